# revision 11
# baseline (speedup 1.0000x reference)
"""CLIP4CAD_HUS_v2 fused forward on 8 Trainium2 NeuronCores.

Data-parallel over batch B=64 (8 batches per core), params replicated.
Per core:
  pass 1 (per batch):  global cross-attention (fp32r K/V projections,
                       block-diagonal scores, exp weights without max
                       subtraction -- scores are O(1) by construction --
                       masked AV + masked denominator matvec)
  batched global tail: out-proj + LN + FFN + LN + modn chain (rows b*G+g)
  pass 2 (per batch):  detail attention with modn-shifted queries
  batched detail tail: out-proj + FFN + LN, gated fusion, outputs

Layouts:
  feature-major tile (128, C, T): [p, c, t] = tensor[c*128+p, t]
  matmul: out[M,N] = lhsT[K,M].T @ rhs[K,N]   (K = partition dim)
Matmul operands are float32r (tf32-class PE precision at bf16 speed).
"""

import contextlib

import numpy as np

import concourse.bass as bass
import concourse.mybir as mybir
import concourse.tile as tile
from concourse import bacc
from concourse.bass_utils import run_bass_kernel_spmd

F32 = mybir.dt.float32
F32R = mybir.dt.float32r
AF = mybir.ActivationFunctionType
OP = mybir.AluOpType
AX = mybir.AxisListType

D, H, G, M = 512, 8, 4, 16
HD = D // H
DF = 4 * D
B, N = 64, 2048
NCORES = 8
NB = B // NCORES          # batches per core
NTB = N // 512            # 512-token blocks per batch
NIC = D // 128            # feature chunks of d=512

_BUILD_CACHE = {}


# ----------------------------------------------------------------- host prep

def _wT_chunks(w):
    """(out,in) torch-Linear weight -> (128, in/128, out) chunk layout."""
    wt = np.ascontiguousarray(np.asarray(w, np.float32).T)      # (in, out)
    ic = wt.shape[0] // 128
    return np.ascontiguousarray(
        wt.reshape(ic, 128, wt.shape[1]).transpose(1, 0, 2))


def _qblock_full(q, cph):
    """q (H*cph, D) -> (128, 4, H*cph) block-diag (chunks accumulated over j).

    out[p, j, h*cph+r] = q[h*cph+r, j*128+p] iff h == 2j + p//64, else 0.
    """
    out = np.zeros((128, NIC, H * cph), np.float32)
    q = np.asarray(q, np.float32)
    for j in range(NIC):
        for pl in range(2):
            h = 2 * j + pl
            rows = slice(pl * 64, (pl + 1) * 64)
            out[rows, j, h * cph:(h + 1) * cph] = \
                q[h * cph:(h + 1) * cph, j * 128 + pl * 64:j * 128 + (pl + 1) * 64].T
    return out


def _diagmask_full(cph):
    """(128, 4, H*cph): 1 iff col's head == 2j + p//64."""
    out = np.zeros((128, NIC, H * cph), np.float32)
    for j in range(NIC):
        for pl in range(2):
            h = 2 * j + pl
            out[pl * 64:(pl + 1) * 64, j, h * cph:(h + 1) * cph] = 1.0
    return out


def _bcast_rows(v, rows):
    v = np.asarray(v, np.float32)
    return np.ascontiguousarray(np.broadcast_to(v, (rows, v.shape[-1])))


def prep_host(params, nb=NB):
    p = {k: np.asarray(v, np.float32) for k, v in params.items()}
    io = {}

    adapt = np.tanh(p['mod_embed'][1] @ p['adapt_w'].T + p['adapt_b'])      # (D,)
    gq_eff = p['gq'][0] + 0.1 * adapt                                       # (G, D)
    dq_eff = p['dq'][0] + 0.1 * adapt                                       # (M, D)

    wq, wk, wv = np.split(p['mha_in_w'], 3, 0)
    bq, bk, bv = np.split(p['mha_in_b'], 3, 0)
    qg = (gq_eff @ wq.T + bq) / np.sqrt(HD)                                 # (G, D) -> rows h*G? no: (G,D)
    qd_base = (dq_eff @ p['det_wq'].T + p['det_bq']) / np.sqrt(HD)          # (M, D)

    # reorder query rows to (h, g): q_hg[h*cph+r, :] = q[r, h-th 64-slice...]
    # NOT a reorder of rows: _qblock_full wants q indexed [h*cph+r, d] where
    # the (h, r) query vector is q[r, :] restricted to head h's d-slice.
    # Build expanded (H*cph, D) with rows (h, r) = original row r.
    qg_hg = np.repeat(qg[None, :, :], H, 0).reshape(H * G, D)
    qd_hg = np.repeat(qd_base[None, :, :], H, 0).reshape(H * M, D)

    io['wk'] = _wT_chunks(wk)
    io['wv'] = _wT_chunks(wv)
    io['wdk'] = _wT_chunks(p['det_wk'])
    io['wdv'] = _wT_chunks(p['det_wv'])
    io['wo'] = _wT_chunks(p['mha_out_w'])
    io['wdo'] = _wT_chunks(p['det_wo'])
    io['cw1'] = _wT_chunks(p['cond_w1'])
    io['cw2'] = _wT_chunks(p['cond_w2'] / np.sqrt(HD))
    io['gw1'] = _wT_chunks(p['gffn_w1'])
    io['gw2'] = _wT_chunks(p['gffn_w2'])
    io['dw1'] = _wT_chunks(p['dffn_w1'])
    io['dw2'] = _wT_chunks(p['dffn_w2'])
    io['gaw1'] = _wT_chunks(p['gate_w1'])
    io['gaw2'] = _wT_chunks(p['gate_w2'])
    io['fw'] = _wT_chunks(p['fus_w'])

    io['qgb'] = _qblock_full(qg_hg, G)                                      # (128,4,32)
    io['qdb'] = _qblock_full(qd_hg, M)                                      # (128,4,128)
    io['dmg'] = _diagmask_full(G)
    io['dmd'] = _diagmask_full(M)
    io['ident'] = np.eye(128, dtype=np.float32)

    bo_eff = p['mha_out_b'] + bv @ p['mha_out_w'].T                         # (D,)
    dbo_eff = p['det_bo'] + p['det_bv'] @ p['det_wo'].T
    io['residg'] = (np.tile(gq_eff, (nb, 1)) + bo_eff).astype(np.float32)
    io['dob'] = _bcast_rows(dbo_eff, nb * M)

    io['gn1g'] = _bcast_rows(p['gn1_g'], nb * G)
    io['gn1b'] = _bcast_rows(p['gn1_b'], nb * G)
    io['gn2g'] = _bcast_rows(p['gn2_g'], nb * G)
    io['gn2b'] = _bcast_rows(p['gn2_b'], nb * G)
    io['dng'] = _bcast_rows(p['dn_g'], nb * M)
    io['dnb'] = _bcast_rows(p['dn_b'], nb * M)
    io['gfb1'] = _bcast_rows(p['gffn_b1'], nb * G)
    io['gfb2'] = _bcast_rows(p['gffn_b2'], nb * G)
    io['dfb1'] = _bcast_rows(p['dffn_b1'], nb * M)
    io['dfb2'] = _bcast_rows(p['dffn_b2'], nb * M)
    io['cb1'] = _bcast_rows(p['cond_b1'], nb)
    cb2 = (p['cond_b2'] / np.sqrt(HD)).reshape(NIC, 128).T                  # (128, 4)
    io['cb2t'] = np.ascontiguousarray(
        np.repeat(cb2[:, :, None], nb, axis=2)).astype(np.float32)          # (128,4,nb)
    io['gab1'] = _bcast_rows(p['gate_b1'], nb)
    io['gab2'] = _bcast_rows(p['gate_b2'], nb)
    io['fb'] = _bcast_rows(p['fus_b'], nb)
    io['fg'] = _bcast_rows(p['fus_g'], nb)
    io['fbb'] = _bcast_rows(p['fus_bb'], nb)

    pool4 = np.zeros((nb * G, nb), np.float32)
    for b in range(nb):
        pool4[b * G:(b + 1) * G, b] = 1.0 / G
    io['pool4'] = pool4
    pool16 = np.zeros((nb * M, nb), np.float32)
    for b in range(nb):
        pool16[b * M:(b + 1) * M, b] = 1.0 / M
    io['pool16'] = pool16
    return io


def prep_core_inputs(X_core, mask_core, shared):
    """Per-core data tensors. X_core (nb, N, D) f32, mask_core (nb, N) bool."""
    nb = X_core.shape[0]
    xt = np.ascontiguousarray(
        X_core.transpose(0, 2, 1).reshape(nb, NIC, 128, N)).astype(np.float32)
    m = mask_core.astype(np.float32)                                        # (nb, N)
    maskv = np.ascontiguousarray(
        m.reshape(nb, N // 128, 128).transpose(0, 2, 1))                    # (nb,128,16)
    maskmm = np.ascontiguousarray(
        np.repeat(maskv[:, :, :, None], 2, axis=3))                         # (nb,128,16,2)
    io = dict(shared)
    io['xt'] = xt
    io['maskv'] = maskv
    io['maskmm'] = maskmm
    return io


# -------------------------------------------------------------- device build

def build(nb=NB):
    if nb in _BUILD_CACHE:
        return _BUILD_CACHE[nb]
    nc = bacc.Bacc()

    def dp(name, shape, dt=F32R):
        return nc.declare_dram_parameter(name, list(shape), dt, isOutput=False)

    xt_d = dp('xt', (nb, NIC, 128, N))
    wk_d = dp('wk', (128, NIC, D)); wv_d = dp('wv', (128, NIC, D))
    wdk_d = dp('wdk', (128, NIC, D)); wdv_d = dp('wdv', (128, NIC, D))
    wo_d = dp('wo', (128, NIC, D)); wdo_d = dp('wdo', (128, NIC, D))
    cw1_d = dp('cw1', (128, NIC, D)); cw2_d = dp('cw2', (128, NIC, D))
    gw1_d = dp('gw1', (128, NIC, DF)); gw2_d = dp('gw2', (128, DF // 128, D))
    dw1_d = dp('dw1', (128, NIC, DF)); dw2_d = dp('dw2', (128, DF // 128, D))
    gaw1_d = dp('gaw1', (128, 2 * NIC, D)); gaw2_d = dp('gaw2', (128, NIC, 2))
    fw_d = dp('fw', (128, 2 * NIC, D))
    qgb_d = dp('qgb', (128, NIC, H * G)); qdb_d = dp('qdb', (128, NIC, H * M))
    dmg_d = dp('dmg', (128, NIC, H * G)); dmd_d = dp('dmd', (128, NIC, H * M))
    ident_d = dp('ident', (128, 128))
    maskv_d = dp('maskv', (nb, 128, N // 128), F32)
    maskmm_d = dp('maskmm', (nb, 128, N // 128, 2))
    residg_d = dp('residg', (nb * G, D), F32)
    dob_d = dp('dob', (nb * M, D), F32)
    gn1g_d = dp('gn1g', (nb * G, D), F32); gn1b_d = dp('gn1b', (nb * G, D), F32)
    gn2g_d = dp('gn2g', (nb * G, D), F32); gn2b_d = dp('gn2b', (nb * G, D), F32)
    dng_d = dp('dng', (nb * M, D), F32); dnb_d = dp('dnb', (nb * M, D), F32)
    gfb1_d = dp('gfb1', (nb * G, DF), F32); gfb2_d = dp('gfb2', (nb * G, D), F32)
    dfb1_d = dp('dfb1', (nb * M, DF), F32); dfb2_d = dp('dfb2', (nb * M, D), F32)
    cb1_d = dp('cb1', (nb, D), F32); cb2t_d = dp('cb2t', (128, NIC, nb), F32)
    gab1_d = dp('gab1', (nb, D), F32); gab2_d = dp('gab2', (nb, 2), F32)
    fb_d = dp('fb', (nb, D), F32); fg_d = dp('fg', (nb, D), F32)
    fbb_d = dp('fbb', (nb, D), F32)
    pool4_d = dp('pool4', (nb * G, nb)); pool16_d = dp('pool16', (nb * M, nb))

    zg_o = nc.declare_dram_parameter('zg', [nb, D], F32, isOutput=True)
    zd_o = nc.declare_dram_parameter('zd', [nb, D], F32, isOutput=True)
    zu_o = nc.declare_dram_parameter('zu', [nb, D], F32, isOutput=True)

    with tile.TileContext(nc) as tc, contextlib.ExitStack() as ctx:
        wpool = ctx.enter_context(tc.tile_pool(name="w", bufs=1))
        ffnw = ctx.enter_context(tc.tile_pool(name="ffnw", bufs=2))
        cpool = ctx.enter_context(tc.tile_pool(name="c", bufs=1))
        xpool = ctx.enter_context(tc.tile_pool(name="x", bufs=2))
        kpool = ctx.enter_context(tc.tile_pool(name="k", bufs=2))
        vpool = ctx.enter_context(tc.tile_pool(name="v", bufs=2))
        epool = ctx.enter_context(tc.tile_pool(name="e", bufs=2))
        spool = ctx.enter_context(tc.tile_pool(name="s", bufs=2))
        tpool = ctx.enter_context(tc.tile_pool(name="t", bufs=1))
        hpool = ctx.enter_context(tc.tile_pool(name="h", bufs=1))
        opool = ctx.enter_context(tc.tile_pool(name="o", bufs=1))
        ppP = ctx.enter_context(tc.tile_pool(name="ppP", bufs=2, space="PSUM"))
        ppS = ctx.enter_context(tc.tile_pool(name="ppS", bufs=2, space="PSUM"))
        ppT = ctx.enter_context(tc.tile_pool(name="ppT", bufs=2, space="PSUM"))
        ppO = ctx.enter_context(tc.tile_pool(name="ppO", bufs=1, space="PSUM"))
        ppD = ctx.enter_context(tc.tile_pool(name="ppD", bufs=1, space="PSUM"))

        def wtile(dram, shape, dt=F32R, pool=None, tag=None):
            t = (pool or wpool).tile(list(shape), dt, tag=tag or dram.name)
            nc.scalar.dma_start(out=t, in_=dram[tuple(slice(None) for _ in shape)])
            return t

        wk = wtile(wk_d, (128, NIC, D))
        wv = wtile(wv_d, (128, NIC, D))
        wdk = wtile(wdk_d, (128, NIC, D))
        wdv = wtile(wdv_d, (128, NIC, D))
        gaw2 = wtile(gaw2_d, (128, NIC, 2))
        qgb = wtile(qgb_d, (128, NIC, H * G))
        qdb = wtile(qdb_d, (128, NIC, H * M))
        dmg = wtile(dmg_d, (128, NIC, H * G))
        dmd = wtile(dmd_d, (128, NIC, H * M))
        ident = wtile(ident_d, (128, 128))
        identf = ident.bitcast(F32)
        pool4 = wtile(pool4_d, (nb * G, nb))
        pool16 = wtile(pool16_d, (nb * M, nb))

        residg = wtile(residg_d, (nb * G, D), F32, cpool)
        dob = wtile(dob_d, (nb * M, D), F32, cpool)
        gn1g = wtile(gn1g_d, (nb * G, D), F32, cpool)
        gn1b = wtile(gn1b_d, (nb * G, D), F32, cpool)
        gn2g = wtile(gn2g_d, (nb * G, D), F32, cpool)
        gn2b = wtile(gn2b_d, (nb * G, D), F32, cpool)
        dng = wtile(dng_d, (nb * M, D), F32, cpool)
        dnb = wtile(dnb_d, (nb * M, D), F32, cpool)
        gfb2 = wtile(gfb2_d, (nb * G, D), F32, cpool)
        dfb2 = wtile(dfb2_d, (nb * M, D), F32, cpool)
        cb1 = wtile(cb1_d, (nb, D), F32, cpool)
        cb2t = wtile(cb2t_d, (128, NIC, nb), F32, cpool)
        gab1 = wtile(gab1_d, (nb, D), F32, cpool)
        gab2 = wtile(gab2_d, (nb, 2), F32, cpool)
        fb = wtile(fb_d, (nb, D), F32, cpool)
        fg = wtile(fg_d, (nb, D), F32, cpool)
        fbb = wtile(fbb_d, (nb, D), F32, cpool)

        maskv, maskmm = [], []
        for b in range(nb):
            mv = cpool.tile([128, N // 128], F32, tag=f"maskv{b}")
            nc.sync.dma_start(out=mv, in_=maskv_d[b])
            maskv.append(mv)
            mm = cpool.tile([128, N // 128, 2], F32R, tag=f"maskmm{b}")
            nc.sync.dma_start(out=mm, in_=maskmm_d[b])
            maskmm.append(mm)

        eps_g = cpool.tile([nb * G, 1], F32)
        nc.vector.memset(eps_g, 1e-5)
        eps_d = cpool.tile([nb * M, 1], F32)
        nc.vector.memset(eps_d, 1e-5)
        eps_b = cpool.tile([nb, 1], F32)
        nc.vector.memset(eps_b, 1e-5)

        # ---------------------------------------------------------- helpers

        def attention_pass(b, wkt, wvt, q_blk, ncols, o_ps, den_ps):
            for tb in range(NTB):
                xblk = xpool.tile([128, NIC, 512], F32R, tag="xblk")
                nc.sync.dma_start(
                    out=xblk,
                    in_=xt_d[b, :, :, tb * 512:(tb + 1) * 512].rearrange(
                        "ic p t -> p ic t"))
                kt = kpool.tile([128, NIC, 512], F32R, tag="kt")
                for oc in range(NIC):
                    psk = ppP.tile([128, 512], F32, tag="proj")
                    for ic in range(NIC):
                        nc.tensor.matmul(
                            psk, wkt[:, ic, oc * 128:(oc + 1) * 128],
                            xblk[:, ic, :], start=(ic == 0), stop=(ic == NIC - 1))
                    nc.vector.tensor_copy(kt[:, oc, :], psk)
                pss = ppS.tile([ncols, 512], F32, tag="scores")
                for j in range(NIC):
                    nc.tensor.matmul(pss, q_blk[:, j, :], kt[:, j, :],
                                     start=(j == 0), stop=(j == NIC - 1))
                eb = epool.tile([ncols, 512], F32R, tag="eblk")
                nc.scalar.activation(out=eb, in_=pss, func=AF.Exp)
                for c in range(4):
                    pst = ppT.tile([128, ncols], F32R, tag="tr")
                    nc.tensor.transpose(
                        pst, eb[:, c * 128:(c + 1) * 128], ident[:ncols, :ncols])
                    et = epool.tile([128, ncols], F32R, tag="et")
                    nc.vector.tensor_copy(et, pst)
                    tcg = tb * 4 + c
                    psv = ppP.tile([128, 512], F32, tag="proj")
                    for ic in range(NIC):
                        nc.tensor.matmul(
                            psv, xblk[:, ic, c * 128:(c + 1) * 128],
                            wvt[:, ic, :], start=(ic == 0), stop=(ic == NIC - 1))
                    vm = vpool.tile([128, 512], F32R, tag="vm")
                    nc.scalar.activation(out=vm, in_=psv, func=AF.Copy,
                                         scale=maskv[b][:, tcg:tcg + 1])
                    first = (tb == 0 and c == 0)
                    last = (tb == NTB - 1 and c == 3)
                    nc.tensor.matmul(o_ps, et, vm, start=first, stop=last)
                    nc.tensor.matmul(den_ps, et, maskmm[b][:, tcg, :],
                                     start=first, stop=last)

        def normalize_and_pack(ncols, cph, o_ps, den_ps, dmask, oT_all, b):
            den = spool.tile([ncols, 1], F32, tag="den")
            nc.vector.tensor_copy(den, den_ps[:, 0:1])
            rec = spool.tile([ncols, 1], F32, tag="rec")
            nc.vector.reciprocal(rec, den)
            osb = spool.tile([ncols, 512], F32, tag="osb")
            nc.vector.tensor_scalar_mul(osb, o_ps, rec)
            for ic in range(NIC):
                pst = ppT.tile([128, ncols], F32, tag="tr")
                nc.tensor.transpose(
                    pst, osb[:, ic * 128:(ic + 1) * 128], identf[:ncols, :ncols])
                ocl = spool.tile([128, ncols], F32, tag="ocl")
                nc.vector.tensor_tensor(out=ocl, in0=pst,
                                        in1=dmask[:, ic, :].bitcast(F32), op=OP.mult)
                red = spool.tile([128, cph], F32, tag="red")
                nc.vector.tensor_reduce(
                    out=red, in_=ocl.rearrange("p (h c) -> p c h", c=cph),
                    axis=AX.X, op=OP.add)
                last = nc.vector.tensor_copy(
                    oT_all[:, ic, b * cph:(b + 1) * cph], red)
            return last

        def transpose_to_fm(src, rows, tag, n_chunks=NIC, src_f32r=False):
            """src (rows, n_chunks*128) sbuf -> (128, n_chunks, rows) f32r."""
            out = tpool.tile([128, n_chunks, rows], F32R, tag=tag)
            for ic in range(n_chunks):
                if src_f32r:
                    pst = ppT.tile([128, rows], F32R, tag="tr")
                    nc.tensor.transpose(pst, src[:, ic * 128:(ic + 1) * 128],
                                        ident[:rows, :rows])
                else:
                    pst = ppT.tile([128, rows], F32, tag="tr")
                    nc.tensor.transpose(pst, src[:, ic * 128:(ic + 1) * 128],
                                        identf[:rows, :rows])
                nc.vector.tensor_copy(out[:, ic, :], pst)
            return out

        def linear_rows_w(lhsT, wget, n_ic, rows, cols):
            ps = ppP.tile([rows, cols], F32, tag="proj")
            for ic in range(n_ic):
                nc.tensor.matmul(ps, lhsT(ic), wget(ic),
                                 start=(ic == 0), stop=(ic == n_ic - 1))
            return ps

        def linear_rows(lhsT, w_tile, n_ic, rows, cols):
            ps = ppP.tile([rows, cols], F32, tag="proj")
            for ic in range(n_ic):
                nc.tensor.matmul(ps, lhsT(ic), w_tile[:, ic, :cols],
                                 start=(ic == 0), stop=(ic == n_ic - 1))
            return ps

        def layernorm(x, rows, eps_t, gamma, beta, resid=None, bias=None,
                      tag="ln"):
            """LN over free dim D. x may be psum. Returns f32 sbuf (rows, D)."""
            pre = tpool.tile([rows, D], F32, tag="lnpre")
            if bias is not None:
                nc.vector.tensor_tensor(out=pre, in0=x, in1=bias, op=OP.add)
            else:
                nc.vector.tensor_copy(pre, x)
            if resid is not None:
                nc.vector.tensor_tensor(out=pre, in0=pre, in1=resid, op=OP.add)
            stats = tpool.tile([rows, 6], F32, tag="lnst")
            nc.vector.bn_stats(out=stats, in_=pre)
            mv = tpool.tile([rows, 2], F32, tag="lnmv")
            nc.vector.bn_aggr(out=mv, in_=stats)
            rstd = tpool.tile([rows, 1], F32, tag="lnrs")
            nc.scalar.activation(out=rstd, in_=mv[:, 1:2], func=AF.Sqrt, bias=eps_t)
            nc.vector.reciprocal(rstd, rstd)
            nc.vector.tensor_scalar(out=pre, in0=pre, scalar1=mv[:, 0:1],
                                    scalar2=rstd, op0=OP.subtract, op1=OP.mult)
            out = tpool.tile([rows, D], F32, tag=tag + "out")
            nc.vector.tensor_tensor(out=pre, in0=pre, in1=gamma, op=OP.mult)
            last = nc.vector.tensor_tensor(out=out, in0=pre, in1=beta, op=OP.add)
            return out, pre, last

        from concourse.tile import add_dep_helper

        def pin(anchor, inst):
            if anchor is not None:
                add_dep_helper(inst.ins, anchor.ins, reason="phase pin")

        def ffn_block(z_sb, rows, w1_d, w2_d, b1_d, tagp, anchor):
            """psum(rows, D) = W2 @ gelu(W1 @ z + b1), bias2 NOT added.
            Weights and b1 streamed from DRAM chunk by chunk; every stream
            DMA is pinned after `anchor` so the scheduler cannot hoist it
            into an earlier phase (slot-wait head-of-line deadlock)."""
            zT = transpose_to_fm(z_sb, rows, tagp + "zT")
            h1 = hpool.tile([rows, DF], F32R, tag=tagp + "h1")
            for og in range(DF // 512):
                b1c = ffnw.tile([rows, 512], F32, tag="bs1")
                pin(anchor, nc.scalar.dma_start(
                    out=b1c, in_=b1_d[:, og * 512:(og + 1) * 512]))
                ps = ppP.tile([rows, 512], F32, tag="proj")
                for ic in range(NIC):
                    w1c = ffnw.tile([128, 512], F32R, tag="ws1")
                    pin(anchor, nc.scalar.dma_start(
                        out=w1c, in_=w1_d[:, ic, og * 512:(og + 1) * 512]))
                    nc.tensor.matmul(ps, zT[:, ic, :], w1c,
                                     start=(ic == 0), stop=(ic == NIC - 1))
                hb = tpool.tile([rows, 512], F32, tag="ffnhb")
                nc.vector.tensor_tensor(out=hb, in0=ps, in1=b1c, op=OP.add)
                nc.scalar.activation(out=h1[:, og * 512:(og + 1) * 512], in_=hb,
                                     func=AF.Gelu)
            h1v = h1.rearrange("r (cc p) -> r cc p", p=128)
            ps2 = ppP.tile([rows, D], F32, tag="proj")
            for cc in range(DF // 128):
                w2c = ffnw.tile([128, 512], F32R, tag="ws2")
                pin(anchor, nc.scalar.dma_start(out=w2c, in_=w2_d[:, cc, :]))
                pst = ppT.tile([128, rows], F32R, tag="tr")
                nc.tensor.transpose(pst, h1v[:, cc, :], ident[:rows, :rows])
                h1T = tpool.tile([128, rows], F32R, tag="ffnh1T")
                nc.vector.tensor_copy(h1T, pst)
                nc.tensor.matmul(ps2, h1T, w2c,
                                 start=(cc == 0), stop=(cc == DF // 128 - 1))
            return ps2

        def _stream_chunks(dram, cols, anchor, tag="ws1"):
            def get(ic):
                t = ffnw.tile([128, cols], F32R, tag=tag)
                pin(anchor, nc.scalar.dma_start(out=t, in_=dram[:, ic, :cols]))
                return t
            return get

        # ---------------- pass 1: global attention ----------------
        o_gT_all = opool.tile([128, NIC, nb * G], F32R, tag="ogT")
        for b in range(nb):
            o_ps = ppO.tile([H * G, 512], F32, tag="av")
            den_ps = ppD.tile([H * G, 2], F32, tag="den")
            attention_pass(b, wk, wv, qgb, H * G, o_ps, den_ps)
            p1_anchor = normalize_and_pack(H * G, G, o_ps, den_ps, dmg,
                                           o_gT_all, b)

        # ---------------- batched global tail ----------------
        zps = linear_rows_w(lambda ic: o_gT_all[:, ic, :],
                            _stream_chunks(wo_d, D, p1_anchor),
                            NIC, nb * G, D)
        z1, _, z1_inst = layernorm(zps, nb * G, eps_g, gn1g, gn1b,
                                   resid=residg, tag="g1")
        gffn_ps = ffn_block(z1, nb * G, gw1_d, gw2_d, gfb1_d, "gf", z1_inst)
        zg_b = tpool.tile([nb * G, D], F32, tag="zgb")
        nc.vector.tensor_tensor(out=zg_b, in0=gffn_ps, in1=gfb2, op=OP.add)
        z_global, _, _zg_inst = layernorm(zg_b, nb * G, eps_g, gn2g, gn2b, resid=z1,
                                tag="g2")
        z_global_r = tpool.tile([nb * G, D], F32R, tag="zgr")
        nc.vector.tensor_copy(z_global_r, z_global)

        psp = ppS.tile([nb, D], F32, tag="scores")
        nc.tensor.matmul(psp, pool4, z_global_r, start=True, stop=True)
        zgp = tpool.tile([nb, D], F32R, tag="zgp")
        nc.vector.tensor_copy(zgp, psp)
        nc.sync.dma_start(out=zg_o[:, :], in_=zgp.bitcast(F32))
        condT = transpose_to_fm(zgp.bitcast(F32), nb, "condT")

        # modn chain
        m1ps = linear_rows_w(lambda ic: condT[:, ic, :],
                             _stream_chunks(cw1_d, D, _zg_inst),
                             NIC, nb, D)
        m1b = tpool.tile([nb, D], F32, tag="m1b")
        nc.vector.tensor_tensor(out=m1b, in0=m1ps, in1=cb1, op=OP.add)
        m1 = tpool.tile([nb, D], F32R, tag="m1")
        m1_inst = nc.scalar.activation(out=m1, in_=m1b, func=AF.Gelu)
        m1T = transpose_to_fm(m1.bitcast(F32), nb, "m1T")
        modnT = tpool.tile([128, NIC, nb], F32, tag="modnT")
        for oc in range(NIC):
            psm = ppS.tile([128, nb], F32, tag="scores")
            for ic in range(NIC):
                cw2c = ffnw.tile([128, 128], F32R, tag="ws2")
                pin(m1_inst, nc.scalar.dma_start(
                    out=cw2c, in_=cw2_d[:, ic, oc * 128:(oc + 1) * 128]))
                nc.tensor.matmul(psm, cw2c,
                                 m1T[:, ic, :], start=(ic == 0),
                                 stop=(ic == NIC - 1))
            nc.vector.tensor_tensor(out=modnT[:, oc, :], in0=psm,
                                    in1=cb2t[:, oc, :], op=OP.add)

        # ---------------- pass 2: detail attention ----------------
        o_dT_all = opool.tile([128, NIC, nb * M], F32R, tag="odT")
        for b in range(nb):
            qde = spool.tile([128, NIC, H * M], F32R, tag="qde")
            for j in range(NIC):
                tmp = spool.tile([128, H * M], F32R, tag="qdtmp")
                nc.vector.tensor_scalar_mul(tmp, dmd[:, j, :],
                                            modnT[:, j, b:b + 1])
                nc.vector.tensor_tensor(out=qde[:, j, :], in0=qdb[:, j, :],
                                        in1=tmp, op=OP.add)
            o_ps = ppO.tile([H * M, 512], F32, tag="av")
            den_ps = ppD.tile([H * M, 2], F32, tag="den")
            attention_pass(b, wdk, wdv, qde, H * M, o_ps, den_ps)
            p2_anchor = normalize_and_pack(H * M, M, o_ps, den_ps, dmd,
                                           o_dT_all, b)

        # ---------------- batched detail tail ----------------
        zdps = linear_rows_w(lambda ic: o_dT_all[:, ic, :],
                             _stream_chunks(wdo_d, D, p2_anchor),
                             NIC, nb * M, D)
        z1d = tpool.tile([nb * M, D], F32, tag="z1d")
        z1d_inst = nc.vector.tensor_tensor(out=z1d, in0=zdps, in1=dob, op=OP.add)
        dffn_ps = ffn_block(z1d, nb * M, dw1_d, dw2_d, dfb1_d, "df", z1d_inst)
        zd_b = tpool.tile([nb * M, D], F32, tag="zdb")
        nc.vector.tensor_tensor(out=zd_b, in0=dffn_ps, in1=dfb2, op=OP.add)
        z_detail, _, zdet_inst = layernorm(zd_b, nb * M, eps_d, dng, dnb, resid=z1d,
                                tag="dn")
        z_detail_r = tpool.tile([nb * M, D], F32R, tag="zdr")
        nc.vector.tensor_copy(z_detail_r, z_detail)

        psdp = ppS.tile([nb, D], F32, tag="scores")
        nc.tensor.matmul(psdp, pool16, z_detail_r, start=True, stop=True)
        zdp = tpool.tile([nb, D], F32R, tag="zdp")
        nc.vector.tensor_copy(zdp, psdp)
        nc.sync.dma_start(out=zd_o[:, :], in_=zdp.bitcast(F32))
        zdpT = transpose_to_fm(zdp.bitcast(F32), nb, "zdpT")

        # ---------------- gated fusion ----------------
        def giT(ic):
            return condT[:, ic, :] if ic < NIC else zdpT[:, ic - NIC, :]

        gaw1g = _stream_chunks(gaw1_d, D, zdet_inst)
        g1ps = linear_rows_w(giT, gaw1g, 2 * NIC, nb, D)
        g1b = tpool.tile([nb, D], F32, tag="g1b")
        nc.vector.tensor_tensor(out=g1b, in0=g1ps, in1=gab1, op=OP.add)
        g1 = tpool.tile([nb, D], F32R, tag="g1")
        nc.scalar.activation(out=g1, in_=g1b, func=AF.Gelu)
        g1T = transpose_to_fm(g1.bitcast(F32), nb, "g1T")
        lgps = linear_rows(lambda ic: g1T[:, ic, :], gaw2, NIC, nb, 2)
        lg = tpool.tile([nb, 2], F32, tag="lg")
        nc.vector.tensor_tensor(out=lg, in0=lgps, in1=gab2, op=OP.add)
        eg = tpool.tile([nb, 2], F32, tag="eg")
        nc.scalar.activation(out=eg, in_=lg, func=AF.Exp)
        egs = tpool.tile([nb, 1], F32, tag="egs")
        nc.vector.tensor_reduce(out=egs, in_=eg, axis=AX.X, op=OP.add)
        nc.vector.reciprocal(egs, egs)
        gate = tpool.tile([nb, 2], F32, tag="gate")
        nc.vector.tensor_scalar_mul(gate, eg, egs)

        zw = tpool.tile([nb, D], F32, tag="zw")
        nc.vector.tensor_scalar_mul(zw, zgp.bitcast(F32), gate[:, 0:1])
        zw2 = tpool.tile([nb, D], F32, tag="zw2")
        nc.vector.tensor_scalar_mul(zw2, zdp.bitcast(F32), gate[:, 1:2])
        nc.vector.tensor_tensor(out=zw, in0=zw, in1=zw2, op=OP.add)

        fwg = _stream_chunks(fw_d, D, zdet_inst)
        fps = linear_rows_w(giT, fwg, 2 * NIC, nb, D)
        fzb = tpool.tile([nb, D], F32, tag="fzb")
        nc.vector.tensor_tensor(out=fzb, in0=fps, in1=fb, op=OP.add)
        fzg = tpool.tile([nb, D], F32, tag="fzg")
        nc.scalar.activation(out=fzg, in_=fzb, func=AF.Gelu)
        fln, _, _ = layernorm(fzg, nb, eps_b, fg, fbb, tag="fln")
        zu = tpool.tile([nb, D], F32, tag="zu")
        nc.vector.tensor_tensor(out=zu, in0=fln, in1=zw, op=OP.add)
        nc.sync.dma_start(out=zu_o[:, :], in_=zu)

    nc.finalize()
    _BUILD_CACHE[nb] = nc
    return nc


# ------------------------------------------------------------------- driver

LAST_EXEC_NS = None


def kernel(X_tokens, mask, params, _trace=False):
    global LAST_EXEC_NS
    X = np.asarray(X_tokens, np.float32)
    mk = np.asarray(mask, bool)
    nb = X.shape[0] // NCORES
    shared = prep_host(params, nb)
    in_maps = [
        prep_core_inputs(X[c * nb:(c + 1) * nb], mk[c * nb:(c + 1) * nb], shared)
        for c in range(NCORES)
    ]
    nc = build(nb)
    res = run_bass_kernel_spmd(nc, in_maps, list(range(NCORES)), trace=_trace)
    LAST_EXEC_NS = res.exec_time_ns
    out = np.empty((3, X.shape[0], D), np.float32)
    for c in range(NCORES):
        r = res.results[c]
        out[0, c * nb:(c + 1) * nb] = r['zg']
        out[1, c * nb:(c + 1) * nb] = r['zd']
        out[2, c * nb:(c + 1) * nb] = r['zu']
    return out


# revision 14
# speedup vs baseline: 1.0588x; 1.0588x over previous
"""CLIP4CAD_HUS_v2 fused forward on 8 Trainium2 NeuronCores.

Data-parallel over batch B=64 (8 batches per core), params replicated.
Per core:
  pass 1 (per batch):  global cross-attention (fp32r K/V projections,
                       block-diagonal scores, exp weights without max
                       subtraction -- scores are O(1) by construction --
                       masked AV + masked denominator matvec)
  batched global tail: out-proj + LN + FFN + LN + modn chain (rows b*G+g)
  pass 2 (per batch):  detail attention with modn-shifted queries
  batched detail tail: out-proj + FFN + LN, gated fusion, outputs

Layouts:
  feature-major tile (128, C, T): [p, c, t] = tensor[c*128+p, t]
  matmul: out[M,N] = lhsT[K,M].T @ rhs[K,N]   (K = partition dim)
Matmul operands are float32r (tf32-class PE precision at bf16 speed).
"""

import contextlib

import numpy as np

import concourse.bass as bass
import concourse.mybir as mybir
import concourse.tile as tile
from concourse import bacc
from concourse.bass_utils import run_bass_kernel_spmd

F32 = mybir.dt.float32
F32R = mybir.dt.float32r
AF = mybir.ActivationFunctionType
OP = mybir.AluOpType
AX = mybir.AxisListType

D, H, G, M = 512, 8, 4, 16
HD = D // H
DF = 4 * D
B, N = 64, 2048
NCORES = 8
NB = B // NCORES          # batches per core
NTB = N // 512            # 512-token blocks per batch
NIC = D // 128            # feature chunks of d=512

_BUILD_CACHE = {}


# ----------------------------------------------------------------- host prep

def _wT_chunks(w):
    """(out,in) torch-Linear weight -> (128, in/128, out) chunk layout."""
    wt = np.ascontiguousarray(np.asarray(w, np.float32).T)      # (in, out)
    ic = wt.shape[0] // 128
    return np.ascontiguousarray(
        wt.reshape(ic, 128, wt.shape[1]).transpose(1, 0, 2))


def _qblock_full(q, cph):
    """q (H*cph, D) -> (128, 4, H*cph) block-diag (chunks accumulated over j).

    out[p, j, h*cph+r] = q[h*cph+r, j*128+p] iff h == 2j + p//64, else 0.
    """
    out = np.zeros((128, NIC, H * cph), np.float32)
    q = np.asarray(q, np.float32)
    for j in range(NIC):
        for pl in range(2):
            h = 2 * j + pl
            rows = slice(pl * 64, (pl + 1) * 64)
            out[rows, j, h * cph:(h + 1) * cph] = \
                q[h * cph:(h + 1) * cph, j * 128 + pl * 64:j * 128 + (pl + 1) * 64].T
    return out


def _diagmask_full(cph):
    """(128, 4, H*cph): 1 iff col's head == 2j + p//64."""
    out = np.zeros((128, NIC, H * cph), np.float32)
    for j in range(NIC):
        for pl in range(2):
            h = 2 * j + pl
            out[pl * 64:(pl + 1) * 64, j, h * cph:(h + 1) * cph] = 1.0
    return out


def _bcast_rows(v, rows):
    v = np.asarray(v, np.float32)
    return np.ascontiguousarray(np.broadcast_to(v, (rows, v.shape[-1])))


def prep_host(params, nb=NB):
    p = {k: np.asarray(v, np.float32) for k, v in params.items()}
    io = {}

    adapt = np.tanh(p['mod_embed'][1] @ p['adapt_w'].T + p['adapt_b'])      # (D,)
    gq_eff = p['gq'][0] + 0.1 * adapt                                       # (G, D)
    dq_eff = p['dq'][0] + 0.1 * adapt                                       # (M, D)

    wq, wk, wv = np.split(p['mha_in_w'], 3, 0)
    bq, bk, bv = np.split(p['mha_in_b'], 3, 0)
    qg = (gq_eff @ wq.T + bq) / np.sqrt(HD)                                 # (G, D) -> rows h*G? no: (G,D)
    qd_base = (dq_eff @ p['det_wq'].T + p['det_bq']) / np.sqrt(HD)          # (M, D)

    # reorder query rows to (h, g): q_hg[h*cph+r, :] = q[r, h-th 64-slice...]
    # NOT a reorder of rows: _qblock_full wants q indexed [h*cph+r, d] where
    # the (h, r) query vector is q[r, :] restricted to head h's d-slice.
    # Build expanded (H*cph, D) with rows (h, r) = original row r.
    qg_hg = np.repeat(qg[None, :, :], H, 0).reshape(H * G, D)
    qd_hg = np.repeat(qd_base[None, :, :], H, 0).reshape(H * M, D)

    io['wk'] = _wT_chunks(wk)
    io['wv'] = _wT_chunks(wv)
    io['wdk'] = _wT_chunks(p['det_wk'])
    io['wdv'] = _wT_chunks(p['det_wv'])
    io['wo'] = _wT_chunks(p['mha_out_w'])
    io['wdo'] = _wT_chunks(p['det_wo'])
    io['cw1'] = _wT_chunks(p['cond_w1'])
    io['cw2'] = _wT_chunks(p['cond_w2'] / np.sqrt(HD))
    io['gw1'] = _wT_chunks(p['gffn_w1'])
    io['gw2'] = _wT_chunks(p['gffn_w2'])
    io['dw1'] = _wT_chunks(p['dffn_w1'])
    io['dw2'] = _wT_chunks(p['dffn_w2'])
    io['gaw1'] = _wT_chunks(p['gate_w1'])
    io['gaw2'] = _wT_chunks(p['gate_w2'])
    io['fw'] = _wT_chunks(p['fus_w'])

    io['qgb'] = _qblock_full(qg_hg, G)                                      # (128,4,32)
    io['qdb'] = _qblock_full(qd_hg, M)                                      # (128,4,128)
    io['dmg'] = _diagmask_full(G)
    io['dmd'] = _diagmask_full(M)
    io['ident'] = np.eye(128, dtype=np.float32)

    bo_eff = p['mha_out_b'] + bv @ p['mha_out_w'].T                         # (D,)
    dbo_eff = p['det_bo'] + p['det_bv'] @ p['det_wo'].T
    io['residg'] = (np.tile(gq_eff, (nb, 1)) + bo_eff).astype(np.float32)
    io['dob'] = _bcast_rows(dbo_eff, nb * M)

    io['gn1g'] = _bcast_rows(p['gn1_g'], nb * G)
    io['gn1b'] = _bcast_rows(p['gn1_b'], nb * G)
    io['gn2g'] = _bcast_rows(p['gn2_g'], nb * G)
    io['gn2b'] = _bcast_rows(p['gn2_b'], nb * G)
    io['dng'] = _bcast_rows(p['dn_g'], nb * M)
    io['dnb'] = _bcast_rows(p['dn_b'], nb * M)
    io['gfb1'] = _bcast_rows(p['gffn_b1'], nb * G)
    io['gfb2'] = _bcast_rows(p['gffn_b2'], nb * G)
    io['dfb1'] = _bcast_rows(p['dffn_b1'], nb * M)
    io['dfb2'] = _bcast_rows(p['dffn_b2'], nb * M)
    io['cb1'] = _bcast_rows(p['cond_b1'], nb)
    cb2 = (p['cond_b2'] / np.sqrt(HD)).reshape(NIC, 128).T                  # (128, 4)
    io['cb2t'] = np.ascontiguousarray(
        np.repeat(cb2[:, :, None], nb, axis=2)).astype(np.float32)          # (128,4,nb)
    io['gab1'] = _bcast_rows(p['gate_b1'], nb)
    io['gab2'] = _bcast_rows(p['gate_b2'], nb)
    io['fb'] = _bcast_rows(p['fus_b'], nb)
    io['fg'] = _bcast_rows(p['fus_g'], nb)
    io['fbb'] = _bcast_rows(p['fus_bb'], nb)

    pool4 = np.zeros((nb * G, nb), np.float32)
    for b in range(nb):
        pool4[b * G:(b + 1) * G, b] = 1.0 / G
    io['pool4'] = pool4
    pool16 = np.zeros((nb * M, nb), np.float32)
    for b in range(nb):
        pool16[b * M:(b + 1) * M, b] = 1.0 / M
    io['pool16'] = pool16
    return io


def prep_core_inputs(X_core, mask_core, shared):
    """Per-core data tensors. X_core (nb, N, D) f32, mask_core (nb, N) bool."""
    nb = X_core.shape[0]
    xt = np.ascontiguousarray(
        X_core.transpose(0, 2, 1).reshape(nb, NIC, 128, N)).astype(np.float32)
    m = mask_core.astype(np.float32)                                        # (nb, N)
    maskv = np.ascontiguousarray(
        m.reshape(nb, N // 128, 128).transpose(0, 2, 1))                    # (nb,128,16)
    maskmm = np.ascontiguousarray(
        np.repeat(maskv[:, :, :, None], 2, axis=3))                         # (nb,128,16,2)
    io = dict(shared)
    io['xt'] = xt
    io['maskv'] = maskv
    io['maskmm'] = maskmm
    return io


# -------------------------------------------------------------- device build

def build(nb=NB):
    if nb in _BUILD_CACHE:
        return _BUILD_CACHE[nb]
    nc = bacc.Bacc()

    def dp(name, shape, dt=F32R):
        return nc.declare_dram_parameter(name, list(shape), dt, isOutput=False)

    xt_d = dp('xt', (nb, NIC, 128, N))
    wk_d = dp('wk', (128, NIC, D)); wv_d = dp('wv', (128, NIC, D))
    wdk_d = dp('wdk', (128, NIC, D)); wdv_d = dp('wdv', (128, NIC, D))
    wo_d = dp('wo', (128, NIC, D)); wdo_d = dp('wdo', (128, NIC, D))
    cw1_d = dp('cw1', (128, NIC, D)); cw2_d = dp('cw2', (128, NIC, D))
    gw1_d = dp('gw1', (128, NIC, DF)); gw2_d = dp('gw2', (128, DF // 128, D))
    dw1_d = dp('dw1', (128, NIC, DF)); dw2_d = dp('dw2', (128, DF // 128, D))
    gaw1_d = dp('gaw1', (128, 2 * NIC, D)); gaw2_d = dp('gaw2', (128, NIC, 2))
    fw_d = dp('fw', (128, 2 * NIC, D))
    qgb_d = dp('qgb', (128, NIC, H * G)); qdb_d = dp('qdb', (128, NIC, H * M))
    dmg_d = dp('dmg', (128, NIC, H * G)); dmd_d = dp('dmd', (128, NIC, H * M))
    ident_d = dp('ident', (128, 128))
    maskv_d = dp('maskv', (nb, 128, N // 128), F32)
    maskmm_d = dp('maskmm', (nb, 128, N // 128, 2))
    residg_d = dp('residg', (nb * G, D), F32)
    dob_d = dp('dob', (nb * M, D), F32)
    gn1g_d = dp('gn1g', (nb * G, D), F32); gn1b_d = dp('gn1b', (nb * G, D), F32)
    gn2g_d = dp('gn2g', (nb * G, D), F32); gn2b_d = dp('gn2b', (nb * G, D), F32)
    dng_d = dp('dng', (nb * M, D), F32); dnb_d = dp('dnb', (nb * M, D), F32)
    gfb1_d = dp('gfb1', (nb * G, DF), F32); gfb2_d = dp('gfb2', (nb * G, D), F32)
    dfb1_d = dp('dfb1', (nb * M, DF), F32); dfb2_d = dp('dfb2', (nb * M, D), F32)
    cb1_d = dp('cb1', (nb, D), F32); cb2t_d = dp('cb2t', (128, NIC, nb), F32)
    gab1_d = dp('gab1', (nb, D), F32); gab2_d = dp('gab2', (nb, 2), F32)
    fb_d = dp('fb', (nb, D), F32); fg_d = dp('fg', (nb, D), F32)
    fbb_d = dp('fbb', (nb, D), F32)
    pool4_d = dp('pool4', (nb * G, nb)); pool16_d = dp('pool16', (nb * M, nb))

    zg_o = nc.declare_dram_parameter('zg', [nb, D], F32, isOutput=True)
    zd_o = nc.declare_dram_parameter('zd', [nb, D], F32, isOutput=True)
    zu_o = nc.declare_dram_parameter('zu', [nb, D], F32, isOutput=True)

    with tile.TileContext(nc) as tc, contextlib.ExitStack() as ctx:
        wpool = ctx.enter_context(tc.tile_pool(name="w", bufs=1))
        ffnw = ctx.enter_context(tc.tile_pool(name="ffnw", bufs=2))
        cpool = ctx.enter_context(tc.tile_pool(name="c", bufs=1))
        xpool = ctx.enter_context(tc.tile_pool(name="x", bufs=2))
        kpool = ctx.enter_context(tc.tile_pool(name="k", bufs=2))
        vpool = ctx.enter_context(tc.tile_pool(name="v", bufs=3))
        epool = ctx.enter_context(tc.tile_pool(name="e", bufs=3))
        spool = ctx.enter_context(tc.tile_pool(name="s", bufs=2))
        tpool = ctx.enter_context(tc.tile_pool(name="t", bufs=1))
        hpool = ctx.enter_context(tc.tile_pool(name="h", bufs=1))
        opool = ctx.enter_context(tc.tile_pool(name="o", bufs=1))
        ppP = ctx.enter_context(tc.tile_pool(name="ppP", bufs=2, space="PSUM"))
        ppS = ctx.enter_context(tc.tile_pool(name="ppS", bufs=1, space="PSUM"))
        ppT = ctx.enter_context(tc.tile_pool(name="ppT", bufs=2, space="PSUM"))
        ppO = ctx.enter_context(tc.tile_pool(name="ppO", bufs=2, space="PSUM"))
        ppD = ctx.enter_context(tc.tile_pool(name="ppD", bufs=1, space="PSUM"))

        def wtile(dram, shape, dt=F32R, pool=None, tag=None):
            t = (pool or wpool).tile(list(shape), dt, tag=tag or dram.name)
            nc.scalar.dma_start(out=t, in_=dram[tuple(slice(None) for _ in shape)])
            return t

        wk = wtile(wk_d, (128, NIC, D))
        wv = wtile(wv_d, (128, NIC, D))
        wdk = wtile(wdk_d, (128, NIC, D))
        wdv = wtile(wdv_d, (128, NIC, D))
        gaw2 = wtile(gaw2_d, (128, NIC, 2))
        qgb = wtile(qgb_d, (128, NIC, H * G))
        qdb = wtile(qdb_d, (128, NIC, H * M))
        dmg = wtile(dmg_d, (128, NIC, H * G))
        dmd = wtile(dmd_d, (128, NIC, H * M))
        ident = wtile(ident_d, (128, 128))
        identf = ident.bitcast(F32)
        pool4 = wtile(pool4_d, (nb * G, nb))
        pool16 = wtile(pool16_d, (nb * M, nb))

        residg = wtile(residg_d, (nb * G, D), F32, cpool)
        dob = wtile(dob_d, (nb * M, D), F32, cpool)
        gn1g = wtile(gn1g_d, (nb * G, D), F32, cpool)
        gn1b = wtile(gn1b_d, (nb * G, D), F32, cpool)
        gn2g = wtile(gn2g_d, (nb * G, D), F32, cpool)
        gn2b = wtile(gn2b_d, (nb * G, D), F32, cpool)
        dng = wtile(dng_d, (nb * M, D), F32, cpool)
        dnb = wtile(dnb_d, (nb * M, D), F32, cpool)
        gfb2 = wtile(gfb2_d, (nb * G, D), F32, cpool)
        dfb2 = wtile(dfb2_d, (nb * M, D), F32, cpool)
        cb1 = wtile(cb1_d, (nb, D), F32, cpool)
        cb2t = wtile(cb2t_d, (128, NIC, nb), F32, cpool)
        gab1 = wtile(gab1_d, (nb, D), F32, cpool)
        gab2 = wtile(gab2_d, (nb, 2), F32, cpool)
        fb = wtile(fb_d, (nb, D), F32, cpool)
        fg = wtile(fg_d, (nb, D), F32, cpool)
        fbb = wtile(fbb_d, (nb, D), F32, cpool)

        maskv, maskmm = [], []
        for b in range(nb):
            mv = cpool.tile([128, N // 128], F32, tag=f"maskv{b}")
            nc.sync.dma_start(out=mv, in_=maskv_d[b])
            maskv.append(mv)
            mm = cpool.tile([128, N // 128, 2], F32R, tag=f"maskmm{b}")
            nc.sync.dma_start(out=mm, in_=maskmm_d[b])
            maskmm.append(mm)

        eps_g = cpool.tile([nb * G, 1], F32)
        nc.vector.memset(eps_g, 1e-5)
        eps_d = cpool.tile([nb * M, 1], F32)
        nc.vector.memset(eps_d, 1e-5)
        eps_b = cpool.tile([nb, 1], F32)
        nc.vector.memset(eps_b, 1e-5)

        # ---------------------------------------------------------- helpers

        def attention_pass(b, wkt, wvt, q_blk, ncols, o_ps, den_ps):
            for tb in range(NTB):
                xblk = xpool.tile([128, NIC, 512], F32R, tag="xblk")
                nc.sync.dma_start(
                    out=xblk,
                    in_=xt_d[b, :, :, tb * 512:(tb + 1) * 512].rearrange(
                        "ic p t -> p ic t"))
                kt = kpool.tile([128, NIC, 512], F32R, tag="kt")
                for oc in range(NIC):
                    psk = ppP.tile([128, 512], F32, tag="proj")
                    for ic in range(NIC):
                        nc.tensor.matmul(
                            psk, wkt[:, ic, oc * 128:(oc + 1) * 128],
                            xblk[:, ic, :], start=(ic == 0), stop=(ic == NIC - 1))
                    nc.vector.tensor_copy(kt[:, oc, :], psk)
                pss = ppS.tile([ncols, 512], F32, tag="scores")
                for j in range(NIC):
                    nc.tensor.matmul(pss, q_blk[:, j, :], kt[:, j, :],
                                     start=(j == 0), stop=(j == NIC - 1))
                eb = epool.tile([ncols, 512], F32R, tag="eblk")
                nc.scalar.activation(out=eb, in_=pss, func=AF.Exp)
                for c in range(4):
                    pst = ppT.tile([128, ncols], F32R, tag="tr")
                    nc.tensor.transpose(
                        pst, eb[:, c * 128:(c + 1) * 128], ident[:ncols, :ncols])
                    et = epool.tile([128, ncols], F32R, tag="et")
                    nc.vector.tensor_copy(et, pst)
                    tcg = tb * 4 + c
                    psv = ppP.tile([128, 512], F32, tag="proj")
                    for ic in range(NIC):
                        nc.tensor.matmul(
                            psv, xblk[:, ic, c * 128:(c + 1) * 128],
                            wvt[:, ic, :], start=(ic == 0), stop=(ic == NIC - 1))
                    vm = vpool.tile([128, 512], F32R, tag="vm")
                    nc.vector.tensor_scalar_mul(vm, psv, maskv[b][:, tcg:tcg + 1])
                    first = (tb == 0 and c == 0)
                    last = (tb == NTB - 1 and c == 3)
                    nc.tensor.matmul(o_ps, et, vm, start=first, stop=last)
                    nc.tensor.matmul(den_ps, et, maskmm[b][:, tcg, :],
                                     start=first, stop=last)

        def normalize_and_pack(ncols, cph, o_ps, den_ps, dmask, oT_all, b):
            den = spool.tile([ncols, 1], F32, tag="den")
            nc.vector.tensor_copy(den, den_ps[:, 0:1])
            rec = spool.tile([ncols, 1], F32, tag="rec")
            nc.vector.reciprocal(rec, den)
            osb = spool.tile([ncols, 512], F32, tag="osb")
            nc.vector.tensor_scalar_mul(osb, o_ps, rec)
            for ic in range(NIC):
                pst = ppT.tile([128, ncols], F32, tag="tr")
                nc.tensor.transpose(
                    pst, osb[:, ic * 128:(ic + 1) * 128], identf[:ncols, :ncols])
                ocl = spool.tile([128, ncols], F32, tag="ocl")
                nc.vector.tensor_tensor(out=ocl, in0=pst,
                                        in1=dmask[:, ic, :].bitcast(F32), op=OP.mult)
                red = spool.tile([128, cph], F32, tag="red")
                nc.vector.tensor_reduce(
                    out=red, in_=ocl.rearrange("p (h c) -> p c h", c=cph),
                    axis=AX.X, op=OP.add)
                last = nc.vector.tensor_copy(
                    oT_all[:, ic, b * cph:(b + 1) * cph], red)
            return last

        def transpose_to_fm(src, rows, tag, n_chunks=NIC, src_f32r=False):
            """src (rows, n_chunks*128) sbuf -> (128, n_chunks, rows) f32r."""
            out = tpool.tile([128, n_chunks, rows], F32R, tag=tag)
            for ic in range(n_chunks):
                if src_f32r:
                    pst = ppT.tile([128, rows], F32R, tag="tr")
                    nc.tensor.transpose(pst, src[:, ic * 128:(ic + 1) * 128],
                                        ident[:rows, :rows])
                else:
                    pst = ppT.tile([128, rows], F32, tag="tr")
                    nc.tensor.transpose(pst, src[:, ic * 128:(ic + 1) * 128],
                                        identf[:rows, :rows])
                nc.vector.tensor_copy(out[:, ic, :], pst)
            return out

        def linear_rows_w(lhsT, wget, n_ic, rows, cols):
            ps = ppP.tile([rows, cols], F32, tag="proj")
            for ic in range(n_ic):
                nc.tensor.matmul(ps, lhsT(ic), wget(ic),
                                 start=(ic == 0), stop=(ic == n_ic - 1))
            return ps

        def linear_rows(lhsT, w_tile, n_ic, rows, cols):
            ps = ppP.tile([rows, cols], F32, tag="proj")
            for ic in range(n_ic):
                nc.tensor.matmul(ps, lhsT(ic), w_tile[:, ic, :cols],
                                 start=(ic == 0), stop=(ic == n_ic - 1))
            return ps

        def layernorm(x, rows, eps_t, gamma, beta, resid=None, bias=None,
                      tag="ln"):
            """LN over free dim D. x may be psum. Returns f32 sbuf (rows, D)."""
            pre = tpool.tile([rows, D], F32, tag="lnpre")
            if bias is not None:
                nc.vector.tensor_tensor(out=pre, in0=x, in1=bias, op=OP.add)
            else:
                nc.vector.tensor_copy(pre, x)
            if resid is not None:
                nc.vector.tensor_tensor(out=pre, in0=pre, in1=resid, op=OP.add)
            stats = tpool.tile([rows, 6], F32, tag="lnst")
            nc.vector.bn_stats(out=stats, in_=pre)
            mv = tpool.tile([rows, 2], F32, tag="lnmv")
            nc.vector.bn_aggr(out=mv, in_=stats)
            rstd = tpool.tile([rows, 1], F32, tag="lnrs")
            nc.scalar.activation(out=rstd, in_=mv[:, 1:2], func=AF.Sqrt, bias=eps_t)
            nc.vector.reciprocal(rstd, rstd)
            nc.vector.tensor_scalar(out=pre, in0=pre, scalar1=mv[:, 0:1],
                                    scalar2=rstd, op0=OP.subtract, op1=OP.mult)
            out = tpool.tile([rows, D], F32, tag=tag + "out")
            nc.vector.tensor_tensor(out=pre, in0=pre, in1=gamma, op=OP.mult)
            last = nc.vector.tensor_tensor(out=out, in0=pre, in1=beta, op=OP.add)
            return out, pre, last

        from concourse.tile import add_dep_helper

        def pin(anchor, inst):
            if anchor is not None:
                add_dep_helper(inst.ins, anchor.ins, reason="phase pin")

        def ffn_block(z_sb, rows, w1_d, w2_d, b1_d, tagp, anchor):
            """psum(rows, D) = W2 @ gelu(W1 @ z + b1), bias2 NOT added.
            Weights and b1 streamed from DRAM chunk by chunk; every stream
            DMA is pinned after `anchor` so the scheduler cannot hoist it
            into an earlier phase (slot-wait head-of-line deadlock)."""
            zT = transpose_to_fm(z_sb, rows, tagp + "zT")
            h1 = hpool.tile([rows, DF], F32R, tag=tagp + "h1")
            for og in range(DF // 512):
                b1c = ffnw.tile([rows, 512], F32, tag="bs1")
                pin(anchor, nc.scalar.dma_start(
                    out=b1c, in_=b1_d[:, og * 512:(og + 1) * 512]))
                ps = ppP.tile([rows, 512], F32, tag="proj")
                for ic in range(NIC):
                    w1c = ffnw.tile([128, 512], F32R, tag="ws1")
                    pin(anchor, nc.scalar.dma_start(
                        out=w1c, in_=w1_d[:, ic, og * 512:(og + 1) * 512]))
                    nc.tensor.matmul(ps, zT[:, ic, :], w1c,
                                     start=(ic == 0), stop=(ic == NIC - 1))
                hb = tpool.tile([rows, 512], F32, tag="ffnhb")
                nc.vector.tensor_tensor(out=hb, in0=ps, in1=b1c, op=OP.add)
                nc.scalar.activation(out=h1[:, og * 512:(og + 1) * 512], in_=hb,
                                     func=AF.Gelu)
            h1v = h1.rearrange("r (cc p) -> r cc p", p=128)
            ps2 = ppP.tile([rows, D], F32, tag="proj")
            for cc in range(DF // 128):
                w2c = ffnw.tile([128, 512], F32R, tag="ws2")
                pin(anchor, nc.scalar.dma_start(out=w2c, in_=w2_d[:, cc, :]))
                pst = ppT.tile([128, rows], F32R, tag="tr")
                nc.tensor.transpose(pst, h1v[:, cc, :], ident[:rows, :rows])
                h1T = tpool.tile([128, rows], F32R, tag="ffnh1T")
                nc.vector.tensor_copy(h1T, pst)
                nc.tensor.matmul(ps2, h1T, w2c,
                                 start=(cc == 0), stop=(cc == DF // 128 - 1))
            return ps2

        def _stream_chunks(dram, cols, anchor, tag="ws1"):
            def get(ic):
                t = ffnw.tile([128, cols], F32R, tag=tag)
                pin(anchor, nc.scalar.dma_start(out=t, in_=dram[:, ic, :cols]))
                return t
            return get

        # ---------------- pass 1: global attention ----------------
        o_gT_all = opool.tile([128, NIC, nb * G], F32R, tag="ogT")
        for b in range(nb):
            o_ps = ppO.tile([H * G, 512], F32, tag="av")
            den_ps = ppD.tile([H * G, 2], F32, tag="den")
            attention_pass(b, wk, wv, qgb, H * G, o_ps, den_ps)
            p1_anchor = normalize_and_pack(H * G, G, o_ps, den_ps, dmg,
                                           o_gT_all, b)

        # ---------------- batched global tail ----------------
        zps = linear_rows_w(lambda ic: o_gT_all[:, ic, :],
                            _stream_chunks(wo_d, D, p1_anchor),
                            NIC, nb * G, D)
        z1, _, z1_inst = layernorm(zps, nb * G, eps_g, gn1g, gn1b,
                                   resid=residg, tag="g1")
        gffn_ps = ffn_block(z1, nb * G, gw1_d, gw2_d, gfb1_d, "gf", z1_inst)
        zg_b = tpool.tile([nb * G, D], F32, tag="zgb")
        nc.vector.tensor_tensor(out=zg_b, in0=gffn_ps, in1=gfb2, op=OP.add)
        z_global, _, _zg_inst = layernorm(zg_b, nb * G, eps_g, gn2g, gn2b, resid=z1,
                                tag="g2")
        z_global_r = tpool.tile([nb * G, D], F32R, tag="zgr")
        nc.vector.tensor_copy(z_global_r, z_global)

        psp = ppS.tile([nb, D], F32, tag="scores")
        nc.tensor.matmul(psp, pool4, z_global_r, start=True, stop=True)
        zgp = tpool.tile([nb, D], F32R, tag="zgp")
        nc.vector.tensor_copy(zgp, psp)
        nc.sync.dma_start(out=zg_o[:, :], in_=zgp.bitcast(F32))
        condT = transpose_to_fm(zgp.bitcast(F32), nb, "condT")

        # modn chain
        m1ps = linear_rows_w(lambda ic: condT[:, ic, :],
                             _stream_chunks(cw1_d, D, _zg_inst),
                             NIC, nb, D)
        m1b = tpool.tile([nb, D], F32, tag="m1b")
        nc.vector.tensor_tensor(out=m1b, in0=m1ps, in1=cb1, op=OP.add)
        m1 = tpool.tile([nb, D], F32R, tag="m1")
        m1_inst = nc.scalar.activation(out=m1, in_=m1b, func=AF.Gelu)
        m1T = transpose_to_fm(m1.bitcast(F32), nb, "m1T")
        modnT = tpool.tile([128, NIC, nb], F32, tag="modnT")
        for oc in range(NIC):
            psm = ppS.tile([128, nb], F32, tag="scores")
            for ic in range(NIC):
                cw2c = ffnw.tile([128, 128], F32R, tag="ws2")
                pin(m1_inst, nc.scalar.dma_start(
                    out=cw2c, in_=cw2_d[:, ic, oc * 128:(oc + 1) * 128]))
                nc.tensor.matmul(psm, cw2c,
                                 m1T[:, ic, :], start=(ic == 0),
                                 stop=(ic == NIC - 1))
            nc.vector.tensor_tensor(out=modnT[:, oc, :], in0=psm,
                                    in1=cb2t[:, oc, :], op=OP.add)

        # ---------------- pass 2: detail attention ----------------
        o_dT_all = opool.tile([128, NIC, nb * M], F32R, tag="odT")
        for b in range(nb):
            qde = spool.tile([128, NIC, H * M], F32R, tag="qde")
            for j in range(NIC):
                tmp = spool.tile([128, H * M], F32R, tag="qdtmp")
                nc.vector.tensor_scalar_mul(tmp, dmd[:, j, :],
                                            modnT[:, j, b:b + 1])
                nc.vector.tensor_tensor(out=qde[:, j, :], in0=qdb[:, j, :],
                                        in1=tmp, op=OP.add)
            o_ps = ppO.tile([H * M, 512], F32, tag="av")
            den_ps = ppD.tile([H * M, 2], F32, tag="den")
            attention_pass(b, wdk, wdv, qde, H * M, o_ps, den_ps)
            p2_anchor = normalize_and_pack(H * M, M, o_ps, den_ps, dmd,
                                           o_dT_all, b)

        # ---------------- batched detail tail ----------------
        zdps = linear_rows_w(lambda ic: o_dT_all[:, ic, :],
                             _stream_chunks(wdo_d, D, p2_anchor),
                             NIC, nb * M, D)
        z1d = tpool.tile([nb * M, D], F32, tag="z1d")
        z1d_inst = nc.vector.tensor_tensor(out=z1d, in0=zdps, in1=dob, op=OP.add)
        dffn_ps = ffn_block(z1d, nb * M, dw1_d, dw2_d, dfb1_d, "df", z1d_inst)
        zd_b = tpool.tile([nb * M, D], F32, tag="zdb")
        nc.vector.tensor_tensor(out=zd_b, in0=dffn_ps, in1=dfb2, op=OP.add)
        z_detail, _, zdet_inst = layernorm(zd_b, nb * M, eps_d, dng, dnb, resid=z1d,
                                tag="dn")
        z_detail_r = tpool.tile([nb * M, D], F32R, tag="zdr")
        nc.vector.tensor_copy(z_detail_r, z_detail)

        psdp = ppS.tile([nb, D], F32, tag="scores")
        nc.tensor.matmul(psdp, pool16, z_detail_r, start=True, stop=True)
        zdp = tpool.tile([nb, D], F32R, tag="zdp")
        nc.vector.tensor_copy(zdp, psdp)
        nc.sync.dma_start(out=zd_o[:, :], in_=zdp.bitcast(F32))
        zdpT = transpose_to_fm(zdp.bitcast(F32), nb, "zdpT")

        # ---------------- gated fusion ----------------
        def giT(ic):
            return condT[:, ic, :] if ic < NIC else zdpT[:, ic - NIC, :]

        gaw1g = _stream_chunks(gaw1_d, D, zdet_inst)
        g1ps = linear_rows_w(giT, gaw1g, 2 * NIC, nb, D)
        g1b = tpool.tile([nb, D], F32, tag="g1b")
        nc.vector.tensor_tensor(out=g1b, in0=g1ps, in1=gab1, op=OP.add)
        g1 = tpool.tile([nb, D], F32R, tag="g1")
        nc.scalar.activation(out=g1, in_=g1b, func=AF.Gelu)
        g1T = transpose_to_fm(g1.bitcast(F32), nb, "g1T")
        lgps = linear_rows(lambda ic: g1T[:, ic, :], gaw2, NIC, nb, 2)
        lg = tpool.tile([nb, 2], F32, tag="lg")
        nc.vector.tensor_tensor(out=lg, in0=lgps, in1=gab2, op=OP.add)
        eg = tpool.tile([nb, 2], F32, tag="eg")
        nc.scalar.activation(out=eg, in_=lg, func=AF.Exp)
        egs = tpool.tile([nb, 1], F32, tag="egs")
        nc.vector.tensor_reduce(out=egs, in_=eg, axis=AX.X, op=OP.add)
        nc.vector.reciprocal(egs, egs)
        gate = tpool.tile([nb, 2], F32, tag="gate")
        nc.vector.tensor_scalar_mul(gate, eg, egs)

        zw = tpool.tile([nb, D], F32, tag="zw")
        nc.vector.tensor_scalar_mul(zw, zgp.bitcast(F32), gate[:, 0:1])
        zw2 = tpool.tile([nb, D], F32, tag="zw2")
        nc.vector.tensor_scalar_mul(zw2, zdp.bitcast(F32), gate[:, 1:2])
        nc.vector.tensor_tensor(out=zw, in0=zw, in1=zw2, op=OP.add)

        fwg = _stream_chunks(fw_d, D, zdet_inst)
        fps = linear_rows_w(giT, fwg, 2 * NIC, nb, D)
        fzb = tpool.tile([nb, D], F32, tag="fzb")
        nc.vector.tensor_tensor(out=fzb, in0=fps, in1=fb, op=OP.add)
        fzg = tpool.tile([nb, D], F32, tag="fzg")
        nc.scalar.activation(out=fzg, in_=fzb, func=AF.Gelu)
        fln, _, _ = layernorm(fzg, nb, eps_b, fg, fbb, tag="fln")
        zu = tpool.tile([nb, D], F32, tag="zu")
        nc.vector.tensor_tensor(out=zu, in0=fln, in1=zw, op=OP.add)
        nc.sync.dma_start(out=zu_o[:, :], in_=zu)

    nc.finalize()
    _BUILD_CACHE[nb] = nc
    return nc


# ------------------------------------------------------------------- driver

LAST_EXEC_NS = None


def kernel(X_tokens, mask, params, _trace=False):
    global LAST_EXEC_NS
    X = np.asarray(X_tokens, np.float32)
    mk = np.asarray(mask, bool)
    nb = X.shape[0] // NCORES
    shared = prep_host(params, nb)
    in_maps = [
        prep_core_inputs(X[c * nb:(c + 1) * nb], mk[c * nb:(c + 1) * nb], shared)
        for c in range(NCORES)
    ]
    nc = build(nb)
    res = run_bass_kernel_spmd(nc, in_maps, list(range(NCORES)), trace=_trace)
    LAST_EXEC_NS = res.exec_time_ns
    out = np.empty((3, X.shape[0], D), np.float32)
    for c in range(NCORES):
        r = res.results[c]
        out[0, c * nb:(c + 1) * nb] = r['zg']
        out[1, c * nb:(c + 1) * nb] = r['zd']
        out[2, c * nb:(c + 1) * nb] = r['zu']
    return out


# revision 16
# speedup vs baseline: 1.1045x; 1.0432x over previous
"""CLIP4CAD_HUS_v2 fused forward on 8 Trainium2 NeuronCores.

Data-parallel over batch B=64 (8 batches per core), params replicated.
Per core:
  pass 1 (per batch):  global cross-attention (fp32r K/V projections,
                       block-diagonal scores, exp weights without max
                       subtraction -- scores are O(1) by construction --
                       masked AV + masked denominator matvec)
  batched global tail: out-proj + LN + FFN + LN + modn chain (rows b*G+g)
  pass 2 (per batch):  detail attention with modn-shifted queries
  batched detail tail: out-proj + FFN + LN, gated fusion, outputs

Layouts:
  feature-major tile (128, C, T): [p, c, t] = tensor[c*128+p, t]
  matmul: out[M,N] = lhsT[K,M].T @ rhs[K,N]   (K = partition dim)
Matmul operands are float32r (tf32-class PE precision at bf16 speed).
"""

import contextlib

import numpy as np

import concourse.bass as bass
import concourse.mybir as mybir
import concourse.tile as tile
from concourse import bacc
from concourse.bass_utils import run_bass_kernel_spmd

F32 = mybir.dt.float32
F32R = mybir.dt.float32r
AF = mybir.ActivationFunctionType
OP = mybir.AluOpType
AX = mybir.AxisListType

D, H, G, M = 512, 8, 4, 16
HD = D // H
DF = 4 * D
B, N = 64, 2048
NCORES = 8
NB = B // NCORES          # batches per core
NTB = N // 512            # 512-token blocks per batch
NIC = D // 128            # feature chunks of d=512

_BUILD_CACHE = {}


# ----------------------------------------------------------------- host prep

def _wT_chunks(w):
    """(out,in) torch-Linear weight -> (128, in/128, out) chunk layout."""
    wt = np.ascontiguousarray(np.asarray(w, np.float32).T)      # (in, out)
    ic = wt.shape[0] // 128
    return np.ascontiguousarray(
        wt.reshape(ic, 128, wt.shape[1]).transpose(1, 0, 2))


def _qblock_full(q, cph):
    """q (H*cph, D) -> (128, 4, H*cph) block-diag (chunks accumulated over j).

    out[p, j, h*cph+r] = q[h*cph+r, j*128+p] iff h == 2j + p//64, else 0.
    """
    out = np.zeros((128, NIC, H * cph), np.float32)
    q = np.asarray(q, np.float32)
    for j in range(NIC):
        for pl in range(2):
            h = 2 * j + pl
            rows = slice(pl * 64, (pl + 1) * 64)
            out[rows, j, h * cph:(h + 1) * cph] = \
                q[h * cph:(h + 1) * cph, j * 128 + pl * 64:j * 128 + (pl + 1) * 64].T
    return out


def _diagmask_full(cph):
    """(128, 4, H*cph): 1 iff col's head == 2j + p//64."""
    out = np.zeros((128, NIC, H * cph), np.float32)
    for j in range(NIC):
        for pl in range(2):
            h = 2 * j + pl
            out[pl * 64:(pl + 1) * 64, j, h * cph:(h + 1) * cph] = 1.0
    return out


def _bcast_rows(v, rows):
    v = np.asarray(v, np.float32)
    return np.ascontiguousarray(np.broadcast_to(v, (rows, v.shape[-1])))


def prep_host(params, nb=NB):
    p = {k: np.asarray(v, np.float32) for k, v in params.items()}
    io = {}

    adapt = np.tanh(p['mod_embed'][1] @ p['adapt_w'].T + p['adapt_b'])      # (D,)
    gq_eff = p['gq'][0] + 0.1 * adapt                                       # (G, D)
    dq_eff = p['dq'][0] + 0.1 * adapt                                       # (M, D)

    wq, wk, wv = np.split(p['mha_in_w'], 3, 0)
    bq, bk, bv = np.split(p['mha_in_b'], 3, 0)
    qg = (gq_eff @ wq.T + bq) / np.sqrt(HD)                                 # (G, D) -> rows h*G? no: (G,D)
    qd_base = (dq_eff @ p['det_wq'].T + p['det_bq']) / np.sqrt(HD)          # (M, D)

    # reorder query rows to (h, g): q_hg[h*cph+r, :] = q[r, h-th 64-slice...]
    # NOT a reorder of rows: _qblock_full wants q indexed [h*cph+r, d] where
    # the (h, r) query vector is q[r, :] restricted to head h's d-slice.
    # Build expanded (H*cph, D) with rows (h, r) = original row r.

    # fold K-projection weights into the (few) query rows: scores = qk @ X^T
    qkg = np.zeros((H * G, D), np.float32)
    qkd = np.zeros((H * M, D), np.float32)
    for h in range(H):
        sl = slice(h * HD, (h + 1) * HD)
        qkg[h * G:(h + 1) * G] = qg[:, sl] @ wk[sl, :]
        qkd[h * M:(h + 1) * M] = qd_base[:, sl] @ p['det_wk'][sl, :]
    io['qkg'] = _wT_chunks(qkg)                                         # (128,4,32)
    io['qkdb'] = _wT_chunks(qkd)                                        # (128,4,128)
    io['wdkr'] = _wT_chunks(p['det_wk'].T)                              # Wdk row-chunks
    io['dmh'] = _diagmask_full(1)                                       # (128,4,8)
    io['ones2'] = np.ones((128, 2), np.float32)
    io['wv'] = _wT_chunks(wv)
    io['wdv'] = _wT_chunks(p['det_wv'])
    io['wo'] = _wT_chunks(p['mha_out_w'])
    io['wdo'] = _wT_chunks(p['det_wo'])
    io['cw1'] = _wT_chunks(p['cond_w1'])
    io['cw2'] = _wT_chunks(p['cond_w2'] / np.sqrt(HD))
    io['gw1'] = _wT_chunks(p['gffn_w1'])
    io['gw2'] = _wT_chunks(p['gffn_w2'])
    io['dw1'] = _wT_chunks(p['dffn_w1'])
    io['dw2'] = _wT_chunks(p['dffn_w2'])
    io['gaw1'] = _wT_chunks(p['gate_w1'])
    io['gaw2'] = _wT_chunks(p['gate_w2'])
    io['fw'] = _wT_chunks(p['fus_w'])

    io['dmg'] = _diagmask_full(G)
    io['dmd'] = _diagmask_full(M)
    io['ident'] = np.eye(128, dtype=np.float32)

    bo_eff = p['mha_out_b'] + bv @ p['mha_out_w'].T                         # (D,)
    dbo_eff = p['det_bo'] + p['det_bv'] @ p['det_wo'].T
    io['residg'] = (np.tile(gq_eff, (nb, 1)) + bo_eff).astype(np.float32)
    io['dob'] = _bcast_rows(dbo_eff, nb * M)

    io['gn1g'] = _bcast_rows(p['gn1_g'], nb * G)
    io['gn1b'] = _bcast_rows(p['gn1_b'], nb * G)
    io['gn2g'] = _bcast_rows(p['gn2_g'], nb * G)
    io['gn2b'] = _bcast_rows(p['gn2_b'], nb * G)
    io['dng'] = _bcast_rows(p['dn_g'], nb * M)
    io['dnb'] = _bcast_rows(p['dn_b'], nb * M)
    io['gfb1'] = _bcast_rows(p['gffn_b1'], nb * G)
    io['gfb2'] = _bcast_rows(p['gffn_b2'], nb * G)
    io['dfb1'] = _bcast_rows(p['dffn_b1'], nb * M)
    io['dfb2'] = _bcast_rows(p['dffn_b2'], nb * M)
    io['cb1'] = _bcast_rows(p['cond_b1'], nb)
    cb2 = (p['cond_b2'] / np.sqrt(HD)).reshape(NIC, 128).T                  # (128, 4)
    io['cb2t'] = np.ascontiguousarray(
        np.repeat(cb2[:, :, None], nb, axis=2)).astype(np.float32)          # (128,4,nb)
    io['gab1'] = _bcast_rows(p['gate_b1'], nb)
    io['gab2'] = _bcast_rows(p['gate_b2'], nb)
    io['fb'] = _bcast_rows(p['fus_b'], nb)
    io['fg'] = _bcast_rows(p['fus_g'], nb)
    io['fbb'] = _bcast_rows(p['fus_bb'], nb)

    pool4 = np.zeros((nb * G, nb), np.float32)
    for b in range(nb):
        pool4[b * G:(b + 1) * G, b] = 1.0 / G
    io['pool4'] = pool4
    pool16 = np.zeros((nb * M, nb), np.float32)
    for b in range(nb):
        pool16[b * M:(b + 1) * M, b] = 1.0 / M
    io['pool16'] = pool16
    return io


def prep_core_inputs(X_core, mask_core, shared):
    """Per-core data tensors. X_core (nb, N, D) f32, mask_core (nb, N) bool."""
    nb = X_core.shape[0]
    xt = np.ascontiguousarray(
        X_core.transpose(0, 2, 1).reshape(nb, NIC, 128, N)).astype(np.float32)
    m = mask_core.astype(np.float32)                                        # (nb, N)
    maskv = np.ascontiguousarray(
        m.reshape(nb, N // 128, 128).transpose(0, 2, 1))                    # (nb,128,16)
    io = dict(shared)
    io['xt'] = xt
    io['xtm'] = np.ascontiguousarray(X_core.astype(np.float32))
    io['maskv'] = maskv
    return io


# -------------------------------------------------------------- device build

def build(nb=NB):
    if nb in _BUILD_CACHE:
        return _BUILD_CACHE[nb]
    nc = bacc.Bacc()

    def dp(name, shape, dt=F32R):
        return nc.declare_dram_parameter(name, list(shape), dt, isOutput=False)

    xt_d = dp('xt', (nb, NIC, 128, N))
    xtm_d = dp('xtm', (nb, N, D))
    wv_d = dp('wv', (128, NIC, D)); wdv_d = dp('wdv', (128, NIC, D))
    qkg_d = dp('qkg', (128, NIC, H * G)); qkdb_d = dp('qkdb', (128, NIC, H * M))
    wdkr_d = dp('wdkr', (128, NIC, D)); dmh_d = dp('dmh', (128, NIC, H))
    ones2_d = dp('ones2', (128, 2))
    wo_d = dp('wo', (128, NIC, D)); wdo_d = dp('wdo', (128, NIC, D))
    cw1_d = dp('cw1', (128, NIC, D)); cw2_d = dp('cw2', (128, NIC, D))
    gw1_d = dp('gw1', (128, NIC, DF)); gw2_d = dp('gw2', (128, DF // 128, D))
    dw1_d = dp('dw1', (128, NIC, DF)); dw2_d = dp('dw2', (128, DF // 128, D))
    gaw1_d = dp('gaw1', (128, 2 * NIC, D)); gaw2_d = dp('gaw2', (128, NIC, 2))
    fw_d = dp('fw', (128, 2 * NIC, D))
    dmg_d = dp('dmg', (128, NIC, H * G)); dmd_d = dp('dmd', (128, NIC, H * M))
    ident_d = dp('ident', (128, 128))
    maskv_d = dp('maskv', (nb, 128, N // 128), F32)
    residg_d = dp('residg', (nb * G, D), F32)
    dob_d = dp('dob', (nb * M, D), F32)
    gn1g_d = dp('gn1g', (nb * G, D), F32); gn1b_d = dp('gn1b', (nb * G, D), F32)
    gn2g_d = dp('gn2g', (nb * G, D), F32); gn2b_d = dp('gn2b', (nb * G, D), F32)
    dng_d = dp('dng', (nb * M, D), F32); dnb_d = dp('dnb', (nb * M, D), F32)
    gfb1_d = dp('gfb1', (nb * G, DF), F32); gfb2_d = dp('gfb2', (nb * G, D), F32)
    dfb1_d = dp('dfb1', (nb * M, DF), F32); dfb2_d = dp('dfb2', (nb * M, D), F32)
    cb1_d = dp('cb1', (nb, D), F32); cb2t_d = dp('cb2t', (128, NIC, nb), F32)
    gab1_d = dp('gab1', (nb, D), F32); gab2_d = dp('gab2', (nb, 2), F32)
    fb_d = dp('fb', (nb, D), F32); fg_d = dp('fg', (nb, D), F32)
    fbb_d = dp('fbb', (nb, D), F32)
    pool4_d = dp('pool4', (nb * G, nb)); pool16_d = dp('pool16', (nb * M, nb))

    zg_o = nc.declare_dram_parameter('zg', [nb, D], F32, isOutput=True)
    zd_o = nc.declare_dram_parameter('zd', [nb, D], F32, isOutput=True)
    zu_o = nc.declare_dram_parameter('zu', [nb, D], F32, isOutput=True)

    with tile.TileContext(nc) as tc, contextlib.ExitStack() as ctx:
        wpool = ctx.enter_context(tc.tile_pool(name="w", bufs=1))
        ffnw = ctx.enter_context(tc.tile_pool(name="ffnw", bufs=2))
        cpool = ctx.enter_context(tc.tile_pool(name="c", bufs=1))
        xpool = ctx.enter_context(tc.tile_pool(name="x", bufs=2))
        kpool = ctx.enter_context(tc.tile_pool(name="k", bufs=2))
        vpool = ctx.enter_context(tc.tile_pool(name="v", bufs=3))
        epool = ctx.enter_context(tc.tile_pool(name="e", bufs=3))
        spool = ctx.enter_context(tc.tile_pool(name="s", bufs=2))
        tpool = ctx.enter_context(tc.tile_pool(name="t", bufs=1))
        hpool = ctx.enter_context(tc.tile_pool(name="h", bufs=1))
        opool = ctx.enter_context(tc.tile_pool(name="o", bufs=1))
        ppP = ctx.enter_context(tc.tile_pool(name="ppP", bufs=2, space="PSUM"))
        ppS = ctx.enter_context(tc.tile_pool(name="ppS", bufs=1, space="PSUM"))
        ppT = ctx.enter_context(tc.tile_pool(name="ppT", bufs=2, space="PSUM"))
        ppO = ctx.enter_context(tc.tile_pool(name="ppO", bufs=2, space="PSUM"))
        ppD = ctx.enter_context(tc.tile_pool(name="ppD", bufs=1, space="PSUM"))

        def wtile(dram, shape, dt=F32R, pool=None, tag=None):
            t = (pool or wpool).tile(list(shape), dt, tag=tag or dram.name)
            nc.scalar.dma_start(out=t, in_=dram[tuple(slice(None) for _ in shape)])
            return t

        wv = wtile(wv_d, (128, NIC, D))
        wdv = wtile(wdv_d, (128, NIC, D))
        wdkr = wtile(wdkr_d, (128, NIC, D))
        qkg = wtile(qkg_d, (128, NIC, H * G))
        qkdb = wtile(qkdb_d, (128, NIC, H * M))
        dmh = wtile(dmh_d, (128, NIC, H))
        ones2 = wtile(ones2_d, (128, 2))
        gaw2 = wtile(gaw2_d, (128, NIC, 2))
        dmg = wtile(dmg_d, (128, NIC, H * G))
        dmd = wtile(dmd_d, (128, NIC, H * M))
        ident = wtile(ident_d, (128, 128))
        identf = ident.bitcast(F32)
        pool4 = wtile(pool4_d, (nb * G, nb))
        pool16 = wtile(pool16_d, (nb * M, nb))

        residg = wtile(residg_d, (nb * G, D), F32, cpool)
        dob = wtile(dob_d, (nb * M, D), F32, cpool)
        gn1g = wtile(gn1g_d, (nb * G, D), F32, cpool)
        gn1b = wtile(gn1b_d, (nb * G, D), F32, cpool)
        gn2g = wtile(gn2g_d, (nb * G, D), F32, cpool)
        gn2b = wtile(gn2b_d, (nb * G, D), F32, cpool)
        dng = wtile(dng_d, (nb * M, D), F32, cpool)
        dnb = wtile(dnb_d, (nb * M, D), F32, cpool)
        gfb2 = wtile(gfb2_d, (nb * G, D), F32, cpool)
        dfb2 = wtile(dfb2_d, (nb * M, D), F32, cpool)
        cb1 = wtile(cb1_d, (nb, D), F32, cpool)
        cb2t = wtile(cb2t_d, (128, NIC, nb), F32, cpool)
        gab1 = wtile(gab1_d, (nb, D), F32, cpool)
        gab2 = wtile(gab2_d, (nb, 2), F32, cpool)
        fb = wtile(fb_d, (nb, D), F32, cpool)
        fg = wtile(fg_d, (nb, D), F32, cpool)
        fbb = wtile(fbb_d, (nb, D), F32, cpool)

        maskv = []
        for b in range(nb):
            mv = cpool.tile([128, N // 128], F32, tag=f"maskv{b}")
            nc.sync.dma_start(out=mv, in_=maskv_d[b])
            maskv.append(mv)

        eps_g = cpool.tile([nb * G, 1], F32)
        nc.vector.memset(eps_g, 1e-5)
        eps_d = cpool.tile([nb * M, 1], F32)
        nc.vector.memset(eps_d, 1e-5)
        eps_b = cpool.tile([nb, 1], F32)
        nc.vector.memset(eps_b, 1e-5)

        # ---------------------------------------------------------- helpers

        def attention_pass(b, qkT, ncols, c_ps, den_ps):
            """scores = qkT.T @ X^T (K-proj folded into queries host-side);
            C += (E.m)^T.T @ X (V-proj deferred: out = C @ Wv^T later)."""
            for tb in range(NTB):
                xblk = xpool.tile([128, NIC, 512], F32R, tag="xblk")
                nc.sync.dma_start(
                    out=xblk,
                    in_=xt_d[b, :, :, tb * 512:(tb + 1) * 512].rearrange(
                        "ic p t -> p ic t"))
                pss = ppS.tile([ncols, 512], F32, tag="scores")
                for j in range(NIC):
                    nc.tensor.matmul(pss, qkT[:, j, :], xblk[:, j, :],
                                     start=(j == 0), stop=(j == NIC - 1))
                eb = epool.tile([ncols, 512], F32R, tag="eblk")
                nc.scalar.activation(out=eb, in_=pss, func=AF.Exp)
                for c in range(4):
                    pst = ppT.tile([128, ncols], F32R, tag="tr")
                    nc.tensor.transpose(
                        pst, eb[:, c * 128:(c + 1) * 128], ident[:ncols, :ncols])
                    tcg = tb * 4 + c
                    et = epool.tile([128, ncols], F32R, tag="et")
                    nc.vector.tensor_scalar_mul(et, pst, maskv[b][:, tcg:tcg + 1])
                    xtm = vpool.tile([128, 512], F32R, tag="xtm")
                    nc.sync.dma_start(
                        out=xtm, in_=xtm_d[b, tcg * 128:(tcg + 1) * 128, :])
                    first = (tb == 0 and c == 0)
                    last = (tb == NTB - 1 and c == 3)
                    nc.tensor.matmul(c_ps, et, xtm, start=first, stop=last)
                    nc.tensor.matmul(den_ps, et, ones2,
                                     start=first, stop=last)

        def finish_attention(ncols, cph, c_ps, den_ps, wvt, dmask, oT_all, b):
            den = spool.tile([ncols, 1], F32, tag="den")
            nc.vector.tensor_copy(den, den_ps[:, 0:1])
            rec = spool.tile([ncols, 1], F32, tag="rec")
            nc.vector.reciprocal(rec, den)
            csb = spool.tile([ncols, 512], F32, tag="csb")
            nc.vector.tensor_scalar_mul(csb, c_ps, rec)
            cT = spool.tile([128, NIC, ncols], F32R, tag="cT")
            for ic in range(NIC):
                pst = ppT.tile([128, ncols], F32, tag="tr")
                nc.tensor.transpose(
                    pst, csb[:, ic * 128:(ic + 1) * 128], identf[:ncols, :ncols])
                nc.vector.tensor_copy(cT[:, ic, :], pst)
            nps = ppP.tile([ncols, 512], F32, tag="proj")
            for ic in range(NIC):
                nc.tensor.matmul(nps, cT[:, ic, :], wvt[:, ic, :],
                                 start=(ic == 0), stop=(ic == NIC - 1))
            osb = spool.tile([ncols, 512], F32, tag="osb")
            nc.vector.tensor_copy(osb, nps)
            for ic in range(NIC):
                pst = ppT.tile([128, ncols], F32, tag="tr")
                nc.tensor.transpose(
                    pst, osb[:, ic * 128:(ic + 1) * 128], identf[:ncols, :ncols])
                ocl = spool.tile([128, ncols], F32, tag="ocl")
                nc.vector.tensor_tensor(out=ocl, in0=pst,
                                        in1=dmask[:, ic, :].bitcast(F32), op=OP.mult)
                red = spool.tile([128, cph], F32, tag="red")
                nc.vector.tensor_reduce(
                    out=red, in_=ocl.rearrange("p (h c) -> p c h", c=cph),
                    axis=AX.X, op=OP.add)
                last = nc.vector.tensor_copy(
                    oT_all[:, ic, b * cph:(b + 1) * cph], red)
            return last

        def transpose_to_fm(src, rows, tag, n_chunks=NIC, src_f32r=False):
            """src (rows, n_chunks*128) sbuf -> (128, n_chunks, rows) f32r."""
            out = tpool.tile([128, n_chunks, rows], F32R, tag=tag)
            for ic in range(n_chunks):
                if src_f32r:
                    pst = ppT.tile([128, rows], F32R, tag="tr")
                    nc.tensor.transpose(pst, src[:, ic * 128:(ic + 1) * 128],
                                        ident[:rows, :rows])
                else:
                    pst = ppT.tile([128, rows], F32, tag="tr")
                    nc.tensor.transpose(pst, src[:, ic * 128:(ic + 1) * 128],
                                        identf[:rows, :rows])
                nc.vector.tensor_copy(out[:, ic, :], pst)
            return out

        def linear_rows_w(lhsT, wget, n_ic, rows, cols):
            ps = ppP.tile([rows, cols], F32, tag="proj")
            for ic in range(n_ic):
                nc.tensor.matmul(ps, lhsT(ic), wget(ic),
                                 start=(ic == 0), stop=(ic == n_ic - 1))
            return ps

        def linear_rows(lhsT, w_tile, n_ic, rows, cols):
            ps = ppP.tile([rows, cols], F32, tag="proj")
            for ic in range(n_ic):
                nc.tensor.matmul(ps, lhsT(ic), w_tile[:, ic, :cols],
                                 start=(ic == 0), stop=(ic == n_ic - 1))
            return ps

        def layernorm(x, rows, eps_t, gamma, beta, resid=None, bias=None,
                      tag="ln"):
            """LN over free dim D. x may be psum. Returns f32 sbuf (rows, D)."""
            pre = tpool.tile([rows, D], F32, tag="lnpre")
            if bias is not None:
                nc.vector.tensor_tensor(out=pre, in0=x, in1=bias, op=OP.add)
            else:
                nc.vector.tensor_copy(pre, x)
            if resid is not None:
                nc.vector.tensor_tensor(out=pre, in0=pre, in1=resid, op=OP.add)
            stats = tpool.tile([rows, 6], F32, tag="lnst")
            nc.vector.bn_stats(out=stats, in_=pre)
            mv = tpool.tile([rows, 2], F32, tag="lnmv")
            nc.vector.bn_aggr(out=mv, in_=stats)
            rstd = tpool.tile([rows, 1], F32, tag="lnrs")
            nc.scalar.activation(out=rstd, in_=mv[:, 1:2], func=AF.Sqrt, bias=eps_t)
            nc.vector.reciprocal(rstd, rstd)
            nc.vector.tensor_scalar(out=pre, in0=pre, scalar1=mv[:, 0:1],
                                    scalar2=rstd, op0=OP.subtract, op1=OP.mult)
            out = tpool.tile([rows, D], F32, tag=tag + "out")
            nc.vector.tensor_tensor(out=pre, in0=pre, in1=gamma, op=OP.mult)
            last = nc.vector.tensor_tensor(out=out, in0=pre, in1=beta, op=OP.add)
            return out, pre, last

        from concourse.tile import add_dep_helper

        def pin(anchor, inst):
            if anchor is not None:
                add_dep_helper(inst.ins, anchor.ins, reason="phase pin")

        def ffn_block(z_sb, rows, w1_d, w2_d, b1_d, tagp, anchor):
            """psum(rows, D) = W2 @ gelu(W1 @ z + b1), bias2 NOT added.
            Weights and b1 streamed from DRAM chunk by chunk; every stream
            DMA is pinned after `anchor` so the scheduler cannot hoist it
            into an earlier phase (slot-wait head-of-line deadlock)."""
            zT = transpose_to_fm(z_sb, rows, tagp + "zT")
            h1 = hpool.tile([rows, DF], F32R, tag=tagp + "h1")
            for og in range(DF // 512):
                b1c = ffnw.tile([rows, 512], F32, tag="bs1")
                pin(anchor, nc.scalar.dma_start(
                    out=b1c, in_=b1_d[:, og * 512:(og + 1) * 512]))
                ps = ppP.tile([rows, 512], F32, tag="proj")
                for ic in range(NIC):
                    w1c = ffnw.tile([128, 512], F32R, tag="ws1")
                    pin(anchor, nc.scalar.dma_start(
                        out=w1c, in_=w1_d[:, ic, og * 512:(og + 1) * 512]))
                    nc.tensor.matmul(ps, zT[:, ic, :], w1c,
                                     start=(ic == 0), stop=(ic == NIC - 1))
                hb = tpool.tile([rows, 512], F32, tag="ffnhb")
                nc.vector.tensor_tensor(out=hb, in0=ps, in1=b1c, op=OP.add)
                nc.scalar.activation(out=h1[:, og * 512:(og + 1) * 512], in_=hb,
                                     func=AF.Gelu)
            h1v = h1.rearrange("r (cc p) -> r cc p", p=128)
            ps2 = ppP.tile([rows, D], F32, tag="proj")
            for cc in range(DF // 128):
                w2c = ffnw.tile([128, 512], F32R, tag="ws2")
                pin(anchor, nc.scalar.dma_start(out=w2c, in_=w2_d[:, cc, :]))
                pst = ppT.tile([128, rows], F32R, tag="tr")
                nc.tensor.transpose(pst, h1v[:, cc, :], ident[:rows, :rows])
                h1T = tpool.tile([128, rows], F32R, tag="ffnh1T")
                nc.vector.tensor_copy(h1T, pst)
                nc.tensor.matmul(ps2, h1T, w2c,
                                 start=(cc == 0), stop=(cc == DF // 128 - 1))
            return ps2

        def _stream_chunks(dram, cols, anchor, tag="ws1"):
            def get(ic):
                t = ffnw.tile([128, cols], F32R, tag=tag)
                pin(anchor, nc.scalar.dma_start(out=t, in_=dram[:, ic, :cols]))
                return t
            return get

        # ---------------- pass 1: global attention ----------------
        o_gT_all = opool.tile([128, NIC, nb * G], F32R, tag="ogT")
        for b in range(nb):
            c_ps = ppO.tile([H * G, 512], F32, tag="av")
            den_ps = ppD.tile([H * G, 2], F32, tag="den")
            attention_pass(b, qkg, H * G, c_ps, den_ps)
            p1_anchor = finish_attention(H * G, G, c_ps, den_ps, wv, dmg,
                                         o_gT_all, b)

        # ---------------- batched global tail ----------------
        zps = linear_rows_w(lambda ic: o_gT_all[:, ic, :],
                            _stream_chunks(wo_d, D, p1_anchor),
                            NIC, nb * G, D)
        z1, _, z1_inst = layernorm(zps, nb * G, eps_g, gn1g, gn1b,
                                   resid=residg, tag="g1")
        gffn_ps = ffn_block(z1, nb * G, gw1_d, gw2_d, gfb1_d, "gf", z1_inst)
        zg_b = tpool.tile([nb * G, D], F32, tag="zgb")
        nc.vector.tensor_tensor(out=zg_b, in0=gffn_ps, in1=gfb2, op=OP.add)
        z_global, _, _zg_inst = layernorm(zg_b, nb * G, eps_g, gn2g, gn2b, resid=z1,
                                tag="g2")
        z_global_r = tpool.tile([nb * G, D], F32R, tag="zgr")
        nc.vector.tensor_copy(z_global_r, z_global)

        psp = ppS.tile([nb, D], F32, tag="scores")
        nc.tensor.matmul(psp, pool4, z_global_r, start=True, stop=True)
        zgp = tpool.tile([nb, D], F32R, tag="zgp")
        nc.vector.tensor_copy(zgp, psp)
        nc.sync.dma_start(out=zg_o[:, :], in_=zgp.bitcast(F32))
        condT = transpose_to_fm(zgp.bitcast(F32), nb, "condT")

        # modn chain
        m1ps = linear_rows_w(lambda ic: condT[:, ic, :],
                             _stream_chunks(cw1_d, D, _zg_inst),
                             NIC, nb, D)
        m1b = tpool.tile([nb, D], F32, tag="m1b")
        nc.vector.tensor_tensor(out=m1b, in0=m1ps, in1=cb1, op=OP.add)
        m1 = tpool.tile([nb, D], F32R, tag="m1")
        m1_inst = nc.scalar.activation(out=m1, in_=m1b, func=AF.Gelu)
        m1T = transpose_to_fm(m1.bitcast(F32), nb, "m1T")
        modnT = tpool.tile([128, NIC, nb], F32, tag="modnT")
        for oc in range(NIC):
            psm = ppS.tile([128, nb], F32, tag="scores")
            for ic in range(NIC):
                cw2c = ffnw.tile([128, 128], F32R, tag="ws2")
                pin(m1_inst, nc.scalar.dma_start(
                    out=cw2c, in_=cw2_d[:, ic, oc * 128:(oc + 1) * 128]))
                nc.tensor.matmul(psm, cw2c,
                                 m1T[:, ic, :], start=(ic == 0),
                                 stop=(ic == NIC - 1))
            nc.vector.tensor_tensor(out=modnT[:, oc, :], in0=psm,
                                    in1=cb2t[:, oc, :], op=OP.add)

        # ---------------- pass 2: detail attention ----------------
        o_dT_all = opool.tile([128, NIC, nb * M], F32R, tag="odT")
        for b in range(nb):
            # fold modn into the folded detail queries:
            # qkd = qkd_base + (modn block-diag) @ Wdk  (broadcast over m)
            mblk = spool.tile([128, NIC, H], F32R, tag="mblk")
            for j in range(NIC):
                nc.vector.tensor_scalar_mul(mblk[:, j, :], dmh[:, j, :],
                                            modnT[:, j, b:b + 1])
            mkT = spool.tile([128, NIC, H], F32R, tag="mkT")
            for ic in range(NIC):
                psm2 = ppT.tile([128, H], F32, tag="tr")
                for j in range(NIC):
                    nc.tensor.matmul(psm2,
                                     wdkr[:, j, ic * 128:(ic + 1) * 128],
                                     mblk[:, j, :], start=(j == 0),
                                     stop=(j == NIC - 1))
                nc.vector.tensor_copy(mkT[:, ic, :], psm2)
            qde = spool.tile([128, NIC, H * M], F32R, tag="qde")
            for ic in range(NIC):
                nc.vector.tensor_tensor(
                    out=qde[:, ic, :].rearrange("p (h m) -> p h m", m=M),
                    in0=qkdb[:, ic, :].rearrange("p (h m) -> p h m", m=M),
                    in1=mkT[:, ic, :].unsqueeze(-1).broadcast_to([128, H, M]),
                    op=OP.add)
            c_ps = ppO.tile([H * M, 512], F32, tag="av")
            den_ps = ppD.tile([H * M, 2], F32, tag="den")
            attention_pass(b, qde, H * M, c_ps, den_ps)
            p2_anchor = finish_attention(H * M, M, c_ps, den_ps, wdv, dmd,
                                         o_dT_all, b)

        # ---------------- batched detail tail ----------------
        zdps = linear_rows_w(lambda ic: o_dT_all[:, ic, :],
                             _stream_chunks(wdo_d, D, p2_anchor),
                             NIC, nb * M, D)
        z1d = tpool.tile([nb * M, D], F32, tag="z1d")
        z1d_inst = nc.vector.tensor_tensor(out=z1d, in0=zdps, in1=dob, op=OP.add)
        dffn_ps = ffn_block(z1d, nb * M, dw1_d, dw2_d, dfb1_d, "df", z1d_inst)
        zd_b = tpool.tile([nb * M, D], F32, tag="zdb")
        nc.vector.tensor_tensor(out=zd_b, in0=dffn_ps, in1=dfb2, op=OP.add)
        z_detail, _, zdet_inst = layernorm(zd_b, nb * M, eps_d, dng, dnb, resid=z1d,
                                tag="dn")
        z_detail_r = tpool.tile([nb * M, D], F32R, tag="zdr")
        nc.vector.tensor_copy(z_detail_r, z_detail)

        psdp = ppS.tile([nb, D], F32, tag="scores")
        nc.tensor.matmul(psdp, pool16, z_detail_r, start=True, stop=True)
        zdp = tpool.tile([nb, D], F32R, tag="zdp")
        nc.vector.tensor_copy(zdp, psdp)
        nc.sync.dma_start(out=zd_o[:, :], in_=zdp.bitcast(F32))
        zdpT = transpose_to_fm(zdp.bitcast(F32), nb, "zdpT")

        # ---------------- gated fusion ----------------
        def giT(ic):
            return condT[:, ic, :] if ic < NIC else zdpT[:, ic - NIC, :]

        gaw1g = _stream_chunks(gaw1_d, D, zdet_inst)
        g1ps = linear_rows_w(giT, gaw1g, 2 * NIC, nb, D)
        g1b = tpool.tile([nb, D], F32, tag="g1b")
        nc.vector.tensor_tensor(out=g1b, in0=g1ps, in1=gab1, op=OP.add)
        g1 = tpool.tile([nb, D], F32R, tag="g1")
        nc.scalar.activation(out=g1, in_=g1b, func=AF.Gelu)
        g1T = transpose_to_fm(g1.bitcast(F32), nb, "g1T")
        lgps = linear_rows(lambda ic: g1T[:, ic, :], gaw2, NIC, nb, 2)
        lg = tpool.tile([nb, 2], F32, tag="lg")
        nc.vector.tensor_tensor(out=lg, in0=lgps, in1=gab2, op=OP.add)
        eg = tpool.tile([nb, 2], F32, tag="eg")
        nc.scalar.activation(out=eg, in_=lg, func=AF.Exp)
        egs = tpool.tile([nb, 1], F32, tag="egs")
        nc.vector.tensor_reduce(out=egs, in_=eg, axis=AX.X, op=OP.add)
        nc.vector.reciprocal(egs, egs)
        gate = tpool.tile([nb, 2], F32, tag="gate")
        nc.vector.tensor_scalar_mul(gate, eg, egs)

        zw = tpool.tile([nb, D], F32, tag="zw")
        nc.vector.tensor_scalar_mul(zw, zgp.bitcast(F32), gate[:, 0:1])
        zw2 = tpool.tile([nb, D], F32, tag="zw2")
        nc.vector.tensor_scalar_mul(zw2, zdp.bitcast(F32), gate[:, 1:2])
        nc.vector.tensor_tensor(out=zw, in0=zw, in1=zw2, op=OP.add)

        fwg = _stream_chunks(fw_d, D, zdet_inst)
        fps = linear_rows_w(giT, fwg, 2 * NIC, nb, D)
        fzb = tpool.tile([nb, D], F32, tag="fzb")
        nc.vector.tensor_tensor(out=fzb, in0=fps, in1=fb, op=OP.add)
        fzg = tpool.tile([nb, D], F32, tag="fzg")
        nc.scalar.activation(out=fzg, in_=fzb, func=AF.Gelu)
        fln, _, _ = layernorm(fzg, nb, eps_b, fg, fbb, tag="fln")
        zu = tpool.tile([nb, D], F32, tag="zu")
        nc.vector.tensor_tensor(out=zu, in0=fln, in1=zw, op=OP.add)
        nc.sync.dma_start(out=zu_o[:, :], in_=zu)

    nc.finalize()
    _BUILD_CACHE[nb] = nc
    return nc


# ------------------------------------------------------------------- driver

LAST_EXEC_NS = None


def kernel(X_tokens, mask, params, _trace=False):
    global LAST_EXEC_NS
    X = np.asarray(X_tokens, np.float32)
    mk = np.asarray(mask, bool)
    nb = X.shape[0] // NCORES
    shared = prep_host(params, nb)
    in_maps = [
        prep_core_inputs(X[c * nb:(c + 1) * nb], mk[c * nb:(c + 1) * nb], shared)
        for c in range(NCORES)
    ]
    nc = build(nb)
    res = run_bass_kernel_spmd(nc, in_maps, list(range(NCORES)), trace=_trace)
    LAST_EXEC_NS = res.exec_time_ns
    out = np.empty((3, X.shape[0], D), np.float32)
    for c in range(NCORES):
        r = res.results[c]
        out[0, c * nb:(c + 1) * nb] = r['zg']
        out[1, c * nb:(c + 1) * nb] = r['zd']
        out[2, c * nb:(c + 1) * nb] = r['zu']
    return out


# revision 18
# speedup vs baseline: 1.2388x; 1.1216x over previous
"""CLIP4CAD_HUS_v2 fused forward on 8 Trainium2 NeuronCores.

Data-parallel over batch B=64 (8 batches per core), params replicated.
Per core:
  pass 1 (per batch):  global cross-attention (fp32r K/V projections,
                       block-diagonal scores, exp weights without max
                       subtraction -- scores are O(1) by construction --
                       masked AV + masked denominator matvec)
  batched global tail: out-proj + LN + FFN + LN + modn chain (rows b*G+g)
  pass 2 (per batch):  detail attention with modn-shifted queries
  batched detail tail: out-proj + FFN + LN, gated fusion, outputs

Layouts:
  feature-major tile (128, C, T): [p, c, t] = tensor[c*128+p, t]
  matmul: out[M,N] = lhsT[K,M].T @ rhs[K,N]   (K = partition dim)
Matmul operands are float32r (tf32-class PE precision at bf16 speed).
"""

import contextlib

import numpy as np

import concourse.bass as bass
import concourse.mybir as mybir
import concourse.tile as tile
from concourse import bacc
from concourse.bass_utils import run_bass_kernel_spmd

F32 = mybir.dt.float32
F32R = mybir.dt.float32r
AF = mybir.ActivationFunctionType
OP = mybir.AluOpType
AX = mybir.AxisListType

D, H, G, M = 512, 8, 4, 16
HD = D // H
DF = 4 * D
B, N = 64, 2048
NCORES = 8
NB = B // NCORES          # batches per core
NTB = N // 512            # 512-token blocks per batch
NIC = D // 128            # feature chunks of d=512

_BUILD_CACHE = {}


# ----------------------------------------------------------------- host prep

def _wT_chunks(w):
    """(out,in) torch-Linear weight -> (128, in/128, out) chunk layout."""
    wt = np.ascontiguousarray(np.asarray(w, np.float32).T)      # (in, out)
    ic = wt.shape[0] // 128
    return np.ascontiguousarray(
        wt.reshape(ic, 128, wt.shape[1]).transpose(1, 0, 2))


def _qblock_full(q, cph):
    """q (H*cph, D) -> (128, 4, H*cph) block-diag (chunks accumulated over j).

    out[p, j, h*cph+r] = q[h*cph+r, j*128+p] iff h == 2j + p//64, else 0.
    """
    out = np.zeros((128, NIC, H * cph), np.float32)
    q = np.asarray(q, np.float32)
    for j in range(NIC):
        for pl in range(2):
            h = 2 * j + pl
            rows = slice(pl * 64, (pl + 1) * 64)
            out[rows, j, h * cph:(h + 1) * cph] = \
                q[h * cph:(h + 1) * cph, j * 128 + pl * 64:j * 128 + (pl + 1) * 64].T
    return out


def _diagmask_full(cph):
    """(128, 4, H*cph): 1 iff col's head == 2j + p//64."""
    out = np.zeros((128, NIC, H * cph), np.float32)
    for j in range(NIC):
        for pl in range(2):
            h = 2 * j + pl
            out[pl * 64:(pl + 1) * 64, j, h * cph:(h + 1) * cph] = 1.0
    return out


def _bcast_rows(v, rows):
    v = np.asarray(v, np.float32)
    return np.ascontiguousarray(np.broadcast_to(v, (rows, v.shape[-1])))


def prep_host(params, nb=NB):
    p = {k: np.asarray(v, np.float32) for k, v in params.items()}
    io = {}

    adapt = np.tanh(p['mod_embed'][1] @ p['adapt_w'].T + p['adapt_b'])      # (D,)
    gq_eff = p['gq'][0] + 0.1 * adapt                                       # (G, D)
    dq_eff = p['dq'][0] + 0.1 * adapt                                       # (M, D)

    wq, wk, wv = np.split(p['mha_in_w'], 3, 0)
    bq, bk, bv = np.split(p['mha_in_b'], 3, 0)
    qg = (gq_eff @ wq.T + bq) / np.sqrt(HD)                                 # (G, D) -> rows h*G? no: (G,D)
    qd_base = (dq_eff @ p['det_wq'].T + p['det_bq']) / np.sqrt(HD)          # (M, D)

    # reorder query rows to (h, g): q_hg[h*cph+r, :] = q[r, h-th 64-slice...]
    # NOT a reorder of rows: _qblock_full wants q indexed [h*cph+r, d] where
    # the (h, r) query vector is q[r, :] restricted to head h's d-slice.
    # Build expanded (H*cph, D) with rows (h, r) = original row r.

    # fold K-projection weights into the (few) query rows: scores = qk @ X^T
    qkg = np.zeros((H * G, D), np.float32)
    qkd = np.zeros((H * M, D), np.float32)
    for h in range(H):
        sl = slice(h * HD, (h + 1) * HD)
        qkg[h * G:(h + 1) * G] = qg[:, sl] @ wk[sl, :]
        qkd[h * M:(h + 1) * M] = qd_base[:, sl] @ p['det_wk'][sl, :]
    io['qkg'] = _wT_chunks(qkg)                                         # (128,4,32)
    io['qkdb'] = _wT_chunks(qkd)                                        # (128,4,128)
    io['wdkr'] = _wT_chunks(p['det_wk'].T)                              # Wdk row-chunks
    io['dmh'] = _diagmask_full(1)                                       # (128,4,8)
    io['ones2'] = np.ones((128, 2), np.float32)
    io['wv'] = _wT_chunks(wv)
    io['wdv'] = _wT_chunks(p['det_wv'])
    io['wo'] = _wT_chunks(p['mha_out_w'])
    io['wdo'] = _wT_chunks(p['det_wo'])
    io['cw1'] = _wT_chunks(p['cond_w1'])
    io['cw2'] = _wT_chunks(p['cond_w2'] / np.sqrt(HD))
    io['gw1'] = _wT_chunks(p['gffn_w1'])
    io['gw2'] = _wT_chunks(p['gffn_w2'])
    io['dw1'] = _wT_chunks(p['dffn_w1'])
    io['dw2'] = _wT_chunks(p['dffn_w2'])
    io['gaw1'] = _wT_chunks(p['gate_w1'])
    io['gaw2'] = _wT_chunks(p['gate_w2'])
    io['fw'] = _wT_chunks(p['fus_w'])

    io['dmg'] = _diagmask_full(G)
    io['dmd'] = _diagmask_full(M)
    io['ident'] = np.eye(128, dtype=np.float32)

    bo_eff = p['mha_out_b'] + bv @ p['mha_out_w'].T                         # (D,)
    dbo_eff = p['det_bo'] + p['det_bv'] @ p['det_wo'].T
    io['residg'] = (np.tile(gq_eff, (nb, 1)) + bo_eff).astype(np.float32)
    io['dob'] = _bcast_rows(dbo_eff, nb * M)

    io['gn1g'] = _bcast_rows(p['gn1_g'], nb * G)
    io['gn1b'] = _bcast_rows(p['gn1_b'], nb * G)
    io['gn2g'] = _bcast_rows(p['gn2_g'], nb * G)
    io['gn2b'] = _bcast_rows(p['gn2_b'], nb * G)
    io['dng'] = _bcast_rows(p['dn_g'], nb * M)
    io['dnb'] = _bcast_rows(p['dn_b'], nb * M)
    io['gfb1'] = _bcast_rows(p['gffn_b1'], nb * G)
    io['gfb2'] = _bcast_rows(p['gffn_b2'], nb * G)
    io['dfb1'] = _bcast_rows(p['dffn_b1'], nb * M)
    io['dfb2'] = _bcast_rows(p['dffn_b2'], nb * M)
    io['cb1'] = _bcast_rows(p['cond_b1'], nb)
    cb2 = (p['cond_b2'] / np.sqrt(HD)).reshape(NIC, 128).T                  # (128, 4)
    io['cb2t'] = np.ascontiguousarray(
        np.repeat(cb2[:, :, None], nb, axis=2)).astype(np.float32)          # (128,4,nb)
    io['gab1'] = _bcast_rows(p['gate_b1'], nb)
    io['gab2'] = _bcast_rows(p['gate_b2'], nb)
    io['fb'] = _bcast_rows(p['fus_b'], nb)
    io['fg'] = _bcast_rows(p['fus_g'], nb)
    io['fbb'] = _bcast_rows(p['fus_bb'], nb)

    pool4 = np.zeros((nb * G, nb), np.float32)
    for b in range(nb):
        pool4[b * G:(b + 1) * G, b] = 1.0 / G
    io['pool4'] = pool4
    pool16 = np.zeros((nb * M, nb), np.float32)
    for b in range(nb):
        pool16[b * M:(b + 1) * M, b] = 1.0 / M
    io['pool16'] = pool16
    return io


def prep_core_inputs(X_core, mask_core, shared):
    """Per-core data tensors. X_core (nb, N, D) f32, mask_core (nb, N) bool."""
    nb = X_core.shape[0]
    xt = np.ascontiguousarray(
        X_core.transpose(0, 2, 1).reshape(nb, NIC, 128, N)).astype(np.float32)
    m = mask_core.astype(np.float32)                                        # (nb, N)
    maskv = np.ascontiguousarray(
        m.reshape(nb, N // 128, 128).transpose(0, 2, 1))                    # (nb,128,16)
    io = dict(shared)
    io['xt'] = xt
    io['xtm'] = np.ascontiguousarray(X_core.astype(np.float32))
    io['maskv'] = maskv
    return io


# -------------------------------------------------------------- device build

def build(nb=NB):
    if nb in _BUILD_CACHE:
        return _BUILD_CACHE[nb]
    nc = bacc.Bacc()

    def dp(name, shape, dt=F32R):
        return nc.declare_dram_parameter(name, list(shape), dt, isOutput=False)

    xt_d = dp('xt', (nb, NIC, 128, N))
    xtm_d = dp('xtm', (nb, N, D))
    wv_d = dp('wv', (128, NIC, D)); wdv_d = dp('wdv', (128, NIC, D))
    qkg_d = dp('qkg', (128, NIC, H * G)); qkdb_d = dp('qkdb', (128, NIC, H * M))
    wdkr_d = dp('wdkr', (128, NIC, D)); dmh_d = dp('dmh', (128, NIC, H))
    ones2_d = dp('ones2', (128, 2))
    wo_d = dp('wo', (128, NIC, D)); wdo_d = dp('wdo', (128, NIC, D))
    cw1_d = dp('cw1', (128, NIC, D)); cw2_d = dp('cw2', (128, NIC, D))
    gw1_d = dp('gw1', (128, NIC, DF)); gw2_d = dp('gw2', (128, DF // 128, D))
    dw1_d = dp('dw1', (128, NIC, DF)); dw2_d = dp('dw2', (128, DF // 128, D))
    gaw1_d = dp('gaw1', (128, 2 * NIC, D)); gaw2_d = dp('gaw2', (128, NIC, 2))
    fw_d = dp('fw', (128, 2 * NIC, D))
    dmg_d = dp('dmg', (128, NIC, H * G)); dmd_d = dp('dmd', (128, NIC, H * M))
    ident_d = dp('ident', (128, 128))
    maskv_d = dp('maskv', (nb, 128, N // 128), F32)
    residg_d = dp('residg', (nb * G, D), F32)
    dob_d = dp('dob', (nb * M, D), F32)
    gn1g_d = dp('gn1g', (nb * G, D), F32); gn1b_d = dp('gn1b', (nb * G, D), F32)
    gn2g_d = dp('gn2g', (nb * G, D), F32); gn2b_d = dp('gn2b', (nb * G, D), F32)
    dng_d = dp('dng', (nb * M, D), F32); dnb_d = dp('dnb', (nb * M, D), F32)
    gfb1_d = dp('gfb1', (nb * G, DF), F32); gfb2_d = dp('gfb2', (nb * G, D), F32)
    dfb1_d = dp('dfb1', (nb * M, DF), F32); dfb2_d = dp('dfb2', (nb * M, D), F32)
    cb1_d = dp('cb1', (nb, D), F32); cb2t_d = dp('cb2t', (128, NIC, nb), F32)
    gab1_d = dp('gab1', (nb, D), F32); gab2_d = dp('gab2', (nb, 2), F32)
    fb_d = dp('fb', (nb, D), F32); fg_d = dp('fg', (nb, D), F32)
    fbb_d = dp('fbb', (nb, D), F32)
    pool4_d = dp('pool4', (nb * G, nb)); pool16_d = dp('pool16', (nb * M, nb))

    zg_o = nc.declare_dram_parameter('zg', [nb, D], F32, isOutput=True)
    zd_o = nc.declare_dram_parameter('zd', [nb, D], F32, isOutput=True)
    zu_o = nc.declare_dram_parameter('zu', [nb, D], F32, isOutput=True)

    with tile.TileContext(nc) as tc, contextlib.ExitStack() as ctx:
        wpool = ctx.enter_context(tc.tile_pool(name="w", bufs=1))
        ffnw = ctx.enter_context(tc.tile_pool(name="ffnw", bufs=2))
        cpool = ctx.enter_context(tc.tile_pool(name="c", bufs=1))
        xpool = ctx.enter_context(tc.tile_pool(name="x", bufs=3))
        kpool = ctx.enter_context(tc.tile_pool(name="k", bufs=2))
        vpool = ctx.enter_context(tc.tile_pool(name="v", bufs=6))
        epool = ctx.enter_context(tc.tile_pool(name="e", bufs=6))
        spool = ctx.enter_context(tc.tile_pool(name="s", bufs=2))
        tpool = ctx.enter_context(tc.tile_pool(name="t", bufs=1))
        hpool = ctx.enter_context(tc.tile_pool(name="h", bufs=1))
        opool = ctx.enter_context(tc.tile_pool(name="o", bufs=1))
        ppP = ctx.enter_context(tc.tile_pool(name="ppP", bufs=1, space="PSUM"))
        ppS = ctx.enter_context(tc.tile_pool(name="ppS", bufs=2, space="PSUM"))
        ppT = ctx.enter_context(tc.tile_pool(name="ppT", bufs=2, space="PSUM"))
        ppO = ctx.enter_context(tc.tile_pool(name="ppO", bufs=2, space="PSUM"))
        ppD = ctx.enter_context(tc.tile_pool(name="ppD", bufs=1, space="PSUM"))

        def wtile(dram, shape, dt=F32R, pool=None, tag=None):
            t = (pool or wpool).tile(list(shape), dt, tag=tag or dram.name)
            nc.scalar.dma_start(out=t, in_=dram[tuple(slice(None) for _ in shape)])
            return t

        wv = wtile(wv_d, (128, NIC, D))
        wdv = wtile(wdv_d, (128, NIC, D))
        wdkr = wtile(wdkr_d, (128, NIC, D))
        qkg = wtile(qkg_d, (128, NIC, H * G))
        qkdb = wtile(qkdb_d, (128, NIC, H * M))
        dmh = wtile(dmh_d, (128, NIC, H))
        ones2 = wtile(ones2_d, (128, 2))
        gaw2 = wtile(gaw2_d, (128, NIC, 2))
        dmg = wtile(dmg_d, (128, NIC, H * G))
        dmd = wtile(dmd_d, (128, NIC, H * M))
        ident = wtile(ident_d, (128, 128))
        identf = ident.bitcast(F32)
        pool4 = wtile(pool4_d, (nb * G, nb))
        pool16 = wtile(pool16_d, (nb * M, nb))

        residg = wtile(residg_d, (nb * G, D), F32, cpool)
        dob = wtile(dob_d, (nb * M, D), F32, cpool)
        gn1g = wtile(gn1g_d, (nb * G, D), F32, cpool)
        gn1b = wtile(gn1b_d, (nb * G, D), F32, cpool)
        gn2g = wtile(gn2g_d, (nb * G, D), F32, cpool)
        gn2b = wtile(gn2b_d, (nb * G, D), F32, cpool)
        dng = wtile(dng_d, (nb * M, D), F32, cpool)
        dnb = wtile(dnb_d, (nb * M, D), F32, cpool)
        gfb2 = wtile(gfb2_d, (nb * G, D), F32, cpool)
        dfb2 = wtile(dfb2_d, (nb * M, D), F32, cpool)
        cb1 = wtile(cb1_d, (nb, D), F32, cpool)
        cb2t = wtile(cb2t_d, (128, NIC, nb), F32, cpool)
        gab1 = wtile(gab1_d, (nb, D), F32, cpool)
        gab2 = wtile(gab2_d, (nb, 2), F32, cpool)
        fb = wtile(fb_d, (nb, D), F32, cpool)
        fg = wtile(fg_d, (nb, D), F32, cpool)
        fbb = wtile(fbb_d, (nb, D), F32, cpool)

        maskv = []
        for b in range(nb):
            mv = cpool.tile([128, N // 128], F32, tag=f"maskv{b}")
            nc.sync.dma_start(out=mv, in_=maskv_d[b])
            maskv.append(mv)

        eps_g = cpool.tile([nb * G, 1], F32)
        nc.vector.memset(eps_g, 1e-5)
        eps_d = cpool.tile([nb * M, 1], F32)
        nc.vector.memset(eps_d, 1e-5)
        eps_b = cpool.tile([nb, 1], F32)
        nc.vector.memset(eps_b, 1e-5)

        # ---------------------------------------------------------- helpers

        def attention_pass(b, qkT, ncols, c_ps, den_ps):
            """scores = qkT.T @ X^T (K-proj folded into queries host-side);
            C += (E.m)^T.T @ X (V-proj deferred: out = C @ Wv^T later)."""
            for tb in range(NTB):
                xblk = xpool.tile([128, NIC, 512], F32R, tag="xblk")
                nc.sync.dma_start(
                    out=xblk,
                    in_=xt_d[b, :, :, tb * 512:(tb + 1) * 512].rearrange(
                        "ic p t -> p ic t"))
                pss = ppS.tile([ncols, 512], F32, tag="scores")
                for j in range(NIC):
                    nc.tensor.matmul(pss, qkT[:, j, :], xblk[:, j, :],
                                     start=(j == 0), stop=(j == NIC - 1))
                eb = epool.tile([ncols, 512], F32R, tag="eblk")
                nc.scalar.activation(out=eb, in_=pss, func=AF.Exp)
                for c in range(4):
                    pst = ppT.tile([128, ncols], F32R, tag="tr")
                    nc.tensor.transpose(
                        pst, eb[:, c * 128:(c + 1) * 128], ident[:ncols, :ncols])
                    tcg = tb * 4 + c
                    et = epool.tile([128, ncols], F32R, tag="et")
                    nc.vector.tensor_scalar_mul(et, pst, maskv[b][:, tcg:tcg + 1])
                    xtm = vpool.tile([128, 512], F32R, tag="xtm")
                    nc.sync.dma_start(
                        out=xtm, in_=xtm_d[b, tcg * 128:(tcg + 1) * 128, :])
                    first = (tb == 0 and c == 0)
                    last = (tb == NTB - 1 and c == 3)
                    nc.tensor.matmul(c_ps, et, xtm, start=first, stop=last)
                    nc.tensor.matmul(den_ps, et, ones2,
                                     start=first, stop=last)

        def finish_attention(ncols, cph, c_ps, den_ps, wvt, dmask, oT_all, b):
            den = spool.tile([ncols, 1], F32, tag="den")
            nc.vector.tensor_copy(den, den_ps[:, 0:1])
            rec = spool.tile([ncols, 1], F32, tag="rec")
            nc.vector.reciprocal(rec, den)
            csb = spool.tile([ncols, 512], F32, tag="csb")
            nc.vector.tensor_scalar_mul(csb, c_ps, rec)
            cT = spool.tile([128, NIC, ncols], F32R, tag="cT")
            for ic in range(NIC):
                pst = ppT.tile([128, ncols], F32, tag="tr")
                nc.tensor.transpose(
                    pst, csb[:, ic * 128:(ic + 1) * 128], identf[:ncols, :ncols])
                nc.vector.tensor_copy(cT[:, ic, :], pst)
            nps = ppP.tile([ncols, 512], F32, tag="proj")
            for ic in range(NIC):
                nc.tensor.matmul(nps, cT[:, ic, :], wvt[:, ic, :],
                                 start=(ic == 0), stop=(ic == NIC - 1))
            osb = spool.tile([ncols, 512], F32, tag="osb")
            nc.vector.tensor_copy(osb, nps)
            for ic in range(NIC):
                pst = ppT.tile([128, ncols], F32, tag="tr")
                nc.tensor.transpose(
                    pst, osb[:, ic * 128:(ic + 1) * 128], identf[:ncols, :ncols])
                ocl = spool.tile([128, ncols], F32, tag="ocl")
                nc.vector.tensor_tensor(out=ocl, in0=pst,
                                        in1=dmask[:, ic, :].bitcast(F32), op=OP.mult)
                red = spool.tile([128, cph], F32, tag="red")
                nc.vector.tensor_reduce(
                    out=red, in_=ocl.rearrange("p (h c) -> p c h", c=cph),
                    axis=AX.X, op=OP.add)
                last = nc.vector.tensor_copy(
                    oT_all[:, ic, b * cph:(b + 1) * cph], red)
            return last

        def transpose_to_fm(src, rows, tag, n_chunks=NIC, src_f32r=False):
            """src (rows, n_chunks*128) sbuf -> (128, n_chunks, rows) f32r."""
            out = tpool.tile([128, n_chunks, rows], F32R, tag=tag)
            for ic in range(n_chunks):
                if src_f32r:
                    pst = ppT.tile([128, rows], F32R, tag="tr")
                    nc.tensor.transpose(pst, src[:, ic * 128:(ic + 1) * 128],
                                        ident[:rows, :rows])
                else:
                    pst = ppT.tile([128, rows], F32, tag="tr")
                    nc.tensor.transpose(pst, src[:, ic * 128:(ic + 1) * 128],
                                        identf[:rows, :rows])
                nc.vector.tensor_copy(out[:, ic, :], pst)
            return out

        def linear_rows_w(lhsT, wget, n_ic, rows, cols):
            ps = ppP.tile([rows, cols], F32, tag="proj")
            for ic in range(n_ic):
                nc.tensor.matmul(ps, lhsT(ic), wget(ic),
                                 start=(ic == 0), stop=(ic == n_ic - 1))
            return ps

        def linear_rows(lhsT, w_tile, n_ic, rows, cols):
            ps = ppP.tile([rows, cols], F32, tag="proj")
            for ic in range(n_ic):
                nc.tensor.matmul(ps, lhsT(ic), w_tile[:, ic, :cols],
                                 start=(ic == 0), stop=(ic == n_ic - 1))
            return ps

        def layernorm(x, rows, eps_t, gamma, beta, resid=None, bias=None,
                      tag="ln"):
            """LN over free dim D. x may be psum. Returns f32 sbuf (rows, D)."""
            pre = tpool.tile([rows, D], F32, tag="lnpre")
            if bias is not None:
                nc.vector.tensor_tensor(out=pre, in0=x, in1=bias, op=OP.add)
            else:
                nc.vector.tensor_copy(pre, x)
            if resid is not None:
                nc.vector.tensor_tensor(out=pre, in0=pre, in1=resid, op=OP.add)
            stats = tpool.tile([rows, 6], F32, tag="lnst")
            nc.vector.bn_stats(out=stats, in_=pre)
            mv = tpool.tile([rows, 2], F32, tag="lnmv")
            nc.vector.bn_aggr(out=mv, in_=stats)
            rstd = tpool.tile([rows, 1], F32, tag="lnrs")
            nc.scalar.activation(out=rstd, in_=mv[:, 1:2], func=AF.Sqrt, bias=eps_t)
            nc.vector.reciprocal(rstd, rstd)
            nc.vector.tensor_scalar(out=pre, in0=pre, scalar1=mv[:, 0:1],
                                    scalar2=rstd, op0=OP.subtract, op1=OP.mult)
            out = tpool.tile([rows, D], F32, tag=tag + "out")
            nc.vector.tensor_tensor(out=pre, in0=pre, in1=gamma, op=OP.mult)
            last = nc.vector.tensor_tensor(out=out, in0=pre, in1=beta, op=OP.add)
            return out, pre, last

        from concourse.tile import add_dep_helper

        def pin(anchor, inst):
            if anchor is not None:
                add_dep_helper(inst.ins, anchor.ins, reason="phase pin")

        def ffn_block(z_sb, rows, w1_d, w2_d, b1_d, tagp, anchor):
            """psum(rows, D) = W2 @ gelu(W1 @ z + b1), bias2 NOT added.
            Weights and b1 streamed from DRAM chunk by chunk; every stream
            DMA is pinned after `anchor` so the scheduler cannot hoist it
            into an earlier phase (slot-wait head-of-line deadlock)."""
            zT = transpose_to_fm(z_sb, rows, tagp + "zT")
            h1 = hpool.tile([rows, DF], F32R, tag=tagp + "h1")
            for og in range(DF // 512):
                b1c = ffnw.tile([rows, 512], F32, tag="bs1")
                pin(anchor, nc.scalar.dma_start(
                    out=b1c, in_=b1_d[:, og * 512:(og + 1) * 512]))
                ps = ppP.tile([rows, 512], F32, tag="proj")
                for ic in range(NIC):
                    w1c = ffnw.tile([128, 512], F32R, tag="ws1")
                    pin(anchor, nc.scalar.dma_start(
                        out=w1c, in_=w1_d[:, ic, og * 512:(og + 1) * 512]))
                    nc.tensor.matmul(ps, zT[:, ic, :], w1c,
                                     start=(ic == 0), stop=(ic == NIC - 1))
                hb = tpool.tile([rows, 512], F32, tag="ffnhb")
                nc.vector.tensor_tensor(out=hb, in0=ps, in1=b1c, op=OP.add)
                nc.scalar.activation(out=h1[:, og * 512:(og + 1) * 512], in_=hb,
                                     func=AF.Gelu)
            h1v = h1.rearrange("r (cc p) -> r cc p", p=128)
            ps2 = ppP.tile([rows, D], F32, tag="proj")
            for cc in range(DF // 128):
                w2c = ffnw.tile([128, 512], F32R, tag="ws2")
                pin(anchor, nc.scalar.dma_start(out=w2c, in_=w2_d[:, cc, :]))
                pst = ppT.tile([128, rows], F32R, tag="tr")
                nc.tensor.transpose(pst, h1v[:, cc, :], ident[:rows, :rows])
                h1T = tpool.tile([128, rows], F32R, tag="ffnh1T")
                nc.vector.tensor_copy(h1T, pst)
                nc.tensor.matmul(ps2, h1T, w2c,
                                 start=(cc == 0), stop=(cc == DF // 128 - 1))
            return ps2

        def _stream_chunks(dram, cols, anchor, tag="ws1"):
            def get(ic):
                t = ffnw.tile([128, cols], F32R, tag=tag)
                pin(anchor, nc.scalar.dma_start(out=t, in_=dram[:, ic, :cols]))
                return t
            return get

        # ---------------- pass 1: global attention ----------------
        o_gT_all = opool.tile([128, NIC, nb * G], F32R, tag="ogT")
        for b in range(nb):
            c_ps = ppO.tile([H * G, 512], F32, tag="av")
            den_ps = ppD.tile([H * G, 2], F32, tag="den")
            attention_pass(b, qkg, H * G, c_ps, den_ps)
            p1_anchor = finish_attention(H * G, G, c_ps, den_ps, wv, dmg,
                                         o_gT_all, b)

        # ---------------- batched global tail ----------------
        zps = linear_rows_w(lambda ic: o_gT_all[:, ic, :],
                            _stream_chunks(wo_d, D, p1_anchor),
                            NIC, nb * G, D)
        z1, _, z1_inst = layernorm(zps, nb * G, eps_g, gn1g, gn1b,
                                   resid=residg, tag="g1")
        gffn_ps = ffn_block(z1, nb * G, gw1_d, gw2_d, gfb1_d, "gf", z1_inst)
        zg_b = tpool.tile([nb * G, D], F32, tag="zgb")
        nc.vector.tensor_tensor(out=zg_b, in0=gffn_ps, in1=gfb2, op=OP.add)
        z_global, _, _zg_inst = layernorm(zg_b, nb * G, eps_g, gn2g, gn2b, resid=z1,
                                tag="g2")
        z_global_r = tpool.tile([nb * G, D], F32R, tag="zgr")
        nc.vector.tensor_copy(z_global_r, z_global)

        psp = ppS.tile([nb, D], F32, tag="scores")
        nc.tensor.matmul(psp, pool4, z_global_r, start=True, stop=True)
        zgp = tpool.tile([nb, D], F32R, tag="zgp")
        nc.vector.tensor_copy(zgp, psp)
        nc.sync.dma_start(out=zg_o[:, :], in_=zgp.bitcast(F32))
        condT = transpose_to_fm(zgp.bitcast(F32), nb, "condT")

        # modn chain
        m1ps = linear_rows_w(lambda ic: condT[:, ic, :],
                             _stream_chunks(cw1_d, D, _zg_inst),
                             NIC, nb, D)
        m1b = tpool.tile([nb, D], F32, tag="m1b")
        nc.vector.tensor_tensor(out=m1b, in0=m1ps, in1=cb1, op=OP.add)
        m1 = tpool.tile([nb, D], F32R, tag="m1")
        m1_inst = nc.scalar.activation(out=m1, in_=m1b, func=AF.Gelu)
        m1T = transpose_to_fm(m1.bitcast(F32), nb, "m1T")
        modnT = tpool.tile([128, NIC, nb], F32, tag="modnT")
        for oc in range(NIC):
            psm = ppS.tile([128, nb], F32, tag="scores")
            for ic in range(NIC):
                cw2c = ffnw.tile([128, 128], F32R, tag="ws2")
                pin(m1_inst, nc.scalar.dma_start(
                    out=cw2c, in_=cw2_d[:, ic, oc * 128:(oc + 1) * 128]))
                nc.tensor.matmul(psm, cw2c,
                                 m1T[:, ic, :], start=(ic == 0),
                                 stop=(ic == NIC - 1))
            nc.vector.tensor_tensor(out=modnT[:, oc, :], in0=psm,
                                    in1=cb2t[:, oc, :], op=OP.add)

        # ---------------- pass 2: detail attention ----------------
        o_dT_all = opool.tile([128, NIC, nb * M], F32R, tag="odT")
        for b in range(nb):
            # fold modn into the folded detail queries:
            # qkd = qkd_base + (modn block-diag) @ Wdk  (broadcast over m)
            mblk = spool.tile([128, NIC, H], F32R, tag="mblk")
            for j in range(NIC):
                nc.vector.tensor_scalar_mul(mblk[:, j, :], dmh[:, j, :],
                                            modnT[:, j, b:b + 1])
            mkT = spool.tile([128, NIC, H], F32R, tag="mkT")
            for ic in range(NIC):
                psm2 = ppT.tile([128, H], F32, tag="tr")
                for j in range(NIC):
                    nc.tensor.matmul(psm2,
                                     wdkr[:, j, ic * 128:(ic + 1) * 128],
                                     mblk[:, j, :], start=(j == 0),
                                     stop=(j == NIC - 1))
                nc.vector.tensor_copy(mkT[:, ic, :], psm2)
            qde = spool.tile([128, NIC, H * M], F32R, tag="qde")
            for ic in range(NIC):
                nc.vector.tensor_tensor(
                    out=qde[:, ic, :].rearrange("p (h m) -> p h m", m=M),
                    in0=qkdb[:, ic, :].rearrange("p (h m) -> p h m", m=M),
                    in1=mkT[:, ic, :].unsqueeze(-1).broadcast_to([128, H, M]),
                    op=OP.add)
            c_ps = ppO.tile([H * M, 512], F32, tag="av")
            den_ps = ppD.tile([H * M, 2], F32, tag="den")
            attention_pass(b, qde, H * M, c_ps, den_ps)
            p2_anchor = finish_attention(H * M, M, c_ps, den_ps, wdv, dmd,
                                         o_dT_all, b)

        # ---------------- batched detail tail ----------------
        zdps = linear_rows_w(lambda ic: o_dT_all[:, ic, :],
                             _stream_chunks(wdo_d, D, p2_anchor),
                             NIC, nb * M, D)
        z1d = tpool.tile([nb * M, D], F32, tag="z1d")
        z1d_inst = nc.vector.tensor_tensor(out=z1d, in0=zdps, in1=dob, op=OP.add)
        dffn_ps = ffn_block(z1d, nb * M, dw1_d, dw2_d, dfb1_d, "df", z1d_inst)
        zd_b = tpool.tile([nb * M, D], F32, tag="zdb")
        nc.vector.tensor_tensor(out=zd_b, in0=dffn_ps, in1=dfb2, op=OP.add)
        z_detail, _, zdet_inst = layernorm(zd_b, nb * M, eps_d, dng, dnb, resid=z1d,
                                tag="dn")
        z_detail_r = tpool.tile([nb * M, D], F32R, tag="zdr")
        nc.vector.tensor_copy(z_detail_r, z_detail)

        psdp = ppS.tile([nb, D], F32, tag="scores")
        nc.tensor.matmul(psdp, pool16, z_detail_r, start=True, stop=True)
        zdp = tpool.tile([nb, D], F32R, tag="zdp")
        nc.vector.tensor_copy(zdp, psdp)
        nc.sync.dma_start(out=zd_o[:, :], in_=zdp.bitcast(F32))
        zdpT = transpose_to_fm(zdp.bitcast(F32), nb, "zdpT")

        # ---------------- gated fusion ----------------
        def giT(ic):
            return condT[:, ic, :] if ic < NIC else zdpT[:, ic - NIC, :]

        gaw1g = _stream_chunks(gaw1_d, D, zdet_inst)
        g1ps = linear_rows_w(giT, gaw1g, 2 * NIC, nb, D)
        g1b = tpool.tile([nb, D], F32, tag="g1b")
        nc.vector.tensor_tensor(out=g1b, in0=g1ps, in1=gab1, op=OP.add)
        g1 = tpool.tile([nb, D], F32R, tag="g1")
        nc.scalar.activation(out=g1, in_=g1b, func=AF.Gelu)
        g1T = transpose_to_fm(g1.bitcast(F32), nb, "g1T")
        lgps = linear_rows(lambda ic: g1T[:, ic, :], gaw2, NIC, nb, 2)
        lg = tpool.tile([nb, 2], F32, tag="lg")
        nc.vector.tensor_tensor(out=lg, in0=lgps, in1=gab2, op=OP.add)
        eg = tpool.tile([nb, 2], F32, tag="eg")
        nc.scalar.activation(out=eg, in_=lg, func=AF.Exp)
        egs = tpool.tile([nb, 1], F32, tag="egs")
        nc.vector.tensor_reduce(out=egs, in_=eg, axis=AX.X, op=OP.add)
        nc.vector.reciprocal(egs, egs)
        gate = tpool.tile([nb, 2], F32, tag="gate")
        nc.vector.tensor_scalar_mul(gate, eg, egs)

        zw = tpool.tile([nb, D], F32, tag="zw")
        nc.vector.tensor_scalar_mul(zw, zgp.bitcast(F32), gate[:, 0:1])
        zw2 = tpool.tile([nb, D], F32, tag="zw2")
        nc.vector.tensor_scalar_mul(zw2, zdp.bitcast(F32), gate[:, 1:2])
        nc.vector.tensor_tensor(out=zw, in0=zw, in1=zw2, op=OP.add)

        fwg = _stream_chunks(fw_d, D, zdet_inst)
        fps = linear_rows_w(giT, fwg, 2 * NIC, nb, D)
        fzb = tpool.tile([nb, D], F32, tag="fzb")
        nc.vector.tensor_tensor(out=fzb, in0=fps, in1=fb, op=OP.add)
        fzg = tpool.tile([nb, D], F32, tag="fzg")
        nc.scalar.activation(out=fzg, in_=fzb, func=AF.Gelu)
        fln, _, _ = layernorm(fzg, nb, eps_b, fg, fbb, tag="fln")
        zu = tpool.tile([nb, D], F32, tag="zu")
        nc.vector.tensor_tensor(out=zu, in0=fln, in1=zw, op=OP.add)
        nc.sync.dma_start(out=zu_o[:, :], in_=zu)

    nc.finalize()
    _BUILD_CACHE[nb] = nc
    return nc


# ------------------------------------------------------------------- driver

LAST_EXEC_NS = None


def kernel(X_tokens, mask, params, _trace=False):
    global LAST_EXEC_NS
    X = np.asarray(X_tokens, np.float32)
    mk = np.asarray(mask, bool)
    nb = X.shape[0] // NCORES
    shared = prep_host(params, nb)
    in_maps = [
        prep_core_inputs(X[c * nb:(c + 1) * nb], mk[c * nb:(c + 1) * nb], shared)
        for c in range(NCORES)
    ]
    nc = build(nb)
    res = run_bass_kernel_spmd(nc, in_maps, list(range(NCORES)), trace=_trace)
    LAST_EXEC_NS = res.exec_time_ns
    out = np.empty((3, X.shape[0], D), np.float32)
    for c in range(NCORES):
        r = res.results[c]
        out[0, c * nb:(c + 1) * nb] = r['zg']
        out[1, c * nb:(c + 1) * nb] = r['zd']
        out[2, c * nb:(c + 1) * nb] = r['zu']
    return out


# revision 20
# speedup vs baseline: 1.5610x; 1.2600x over previous
"""CLIP4CAD_HUS_v2 fused forward on 8 Trainium2 NeuronCores.

Data-parallel over batch B=64 (8 batches per core), params replicated.

Key algebraic restructuring (only G=4 / M=16 queries exist per head, so
the full K/V projections over N=2048 tokens are never materialized):
  scores = (q @ Wk) @ X^T        -- Wk folded into the query rows on host
  attn   = ((E*mask) @ X) @ Wv^T -- Wv applied after the n-contraction
E = exp(scores) without max subtraction (scores are O(1) by construction);
masked softmax denominator via a ones-column matvec on E^T. Per-head
output blocks are extracted with a static diag mask + strided h-sum.

Per core:
  pass 1 (per batch):  global cross-attention as above
  batched global tail: out-proj + LN + FFN + LN + modn chain (rows b*G+g)
  pass 2 (per batch):  detail attention, modn folded into the detail
                       queries via a block-diag matvec against Wdk rows
  batched detail tail: out-proj + FFN + LN, gated fusion, outputs

Layouts:
  feature-major tile (128, C, T): [p, c, t] = tensor[c*128+p, t]
  matmul: out[M,N] = lhsT[K,M].T @ rhs[K,N]   (K = partition dim)
Matmul operands are float32r (tf32-class PE precision at bf16 speed).
"""

import contextlib

import numpy as np

import concourse.bass as bass
import concourse.mybir as mybir
import concourse.tile as tile
from concourse import bacc
from concourse.bass_utils import run_bass_kernel_spmd

F32 = mybir.dt.float32
F32R = mybir.dt.float32r
BF16 = mybir.dt.bfloat16
AF = mybir.ActivationFunctionType
OP = mybir.AluOpType
AX = mybir.AxisListType

D, H, G, M = 512, 8, 4, 16
HD = D // H
DF = 4 * D
B, N = 64, 2048
NCORES = 8
NB = B // NCORES          # batches per core
NTB = N // 512            # 512-token blocks per batch
NIC = D // 128            # feature chunks of d=512

_BUILD_CACHE = {}


# ----------------------------------------------------------------- host prep

def _wT_chunks(w):
    """(out,in) torch-Linear weight -> (128, in/128, out) chunk layout."""
    wt = np.ascontiguousarray(np.asarray(w, np.float32).T)      # (in, out)
    ic = wt.shape[0] // 128
    return np.ascontiguousarray(
        wt.reshape(ic, 128, wt.shape[1]).transpose(1, 0, 2))


def _qblock_full(q, cph):
    """q (H*cph, D) -> (128, 4, H*cph) block-diag (chunks accumulated over j).

    out[p, j, h*cph+r] = q[h*cph+r, j*128+p] iff h == 2j + p//64, else 0.
    """
    out = np.zeros((128, NIC, H * cph), np.float32)
    q = np.asarray(q, np.float32)
    for j in range(NIC):
        for pl in range(2):
            h = 2 * j + pl
            rows = slice(pl * 64, (pl + 1) * 64)
            out[rows, j, h * cph:(h + 1) * cph] = \
                q[h * cph:(h + 1) * cph, j * 128 + pl * 64:j * 128 + (pl + 1) * 64].T
    return out


def _diagmask_full(cph):
    """(128, 4, H*cph): 1 iff col's head == 2j + p//64."""
    out = np.zeros((128, NIC, H * cph), np.float32)
    for j in range(NIC):
        for pl in range(2):
            h = 2 * j + pl
            out[pl * 64:(pl + 1) * 64, j, h * cph:(h + 1) * cph] = 1.0
    return out


def _bcast_rows(v, rows):
    v = np.asarray(v, np.float32)
    return np.ascontiguousarray(np.broadcast_to(v, (rows, v.shape[-1])))


def prep_host(params, nb=NB):
    p = {k: np.asarray(v, np.float32) for k, v in params.items()}
    io = {}

    adapt = np.tanh(p['mod_embed'][1] @ p['adapt_w'].T + p['adapt_b'])      # (D,)
    gq_eff = p['gq'][0] + 0.1 * adapt                                       # (G, D)
    dq_eff = p['dq'][0] + 0.1 * adapt                                       # (M, D)

    wq, wk, wv = np.split(p['mha_in_w'], 3, 0)
    bq, bk, bv = np.split(p['mha_in_b'], 3, 0)
    qg = (gq_eff @ wq.T + bq) / np.sqrt(HD)                                 # (G, D) -> rows h*G? no: (G,D)
    qd_base = (dq_eff @ p['det_wq'].T + p['det_bq']) / np.sqrt(HD)          # (M, D)

    # reorder query rows to (h, g): q_hg[h*cph+r, :] = q[r, h-th 64-slice...]
    # NOT a reorder of rows: _qblock_full wants q indexed [h*cph+r, d] where
    # the (h, r) query vector is q[r, :] restricted to head h's d-slice.
    # Build expanded (H*cph, D) with rows (h, r) = original row r.

    # fold K-projection weights into the (few) query rows: scores = qk @ X^T
    qkg = np.zeros((H * G, D), np.float32)
    qkd = np.zeros((H * M, D), np.float32)
    for h in range(H):
        sl = slice(h * HD, (h + 1) * HD)
        qkg[h * G:(h + 1) * G] = qg[:, sl] @ wk[sl, :]
        qkd[h * M:(h + 1) * M] = qd_base[:, sl] @ p['det_wk'][sl, :]
    import ml_dtypes
    io['qkg'] = _wT_chunks(qkg).astype(ml_dtypes.bfloat16)              # (128,4,32)
    io['qkdb'] = _wT_chunks(qkd).astype(ml_dtypes.bfloat16)             # (128,4,128)
    io['wdkr'] = _wT_chunks(p['det_wk'].T).astype(ml_dtypes.bfloat16)
    io['dmh'] = _diagmask_full(1).astype(ml_dtypes.bfloat16)            # (128,4,8)
    io['ones2'] = np.ones((128, 2), ml_dtypes.bfloat16)
    io['identb'] = np.eye(128, dtype=ml_dtypes.bfloat16)
    io['wv'] = _wT_chunks(wv)
    io['wdv'] = _wT_chunks(p['det_wv'])
    io['wo'] = _wT_chunks(p['mha_out_w'])
    io['wdo'] = _wT_chunks(p['det_wo'])
    io['cw1'] = _wT_chunks(p['cond_w1'])
    io['cw2'] = _wT_chunks(p['cond_w2'] / np.sqrt(HD))
    io['gw1'] = _wT_chunks(p['gffn_w1'])
    io['gw2'] = _wT_chunks(p['gffn_w2'])
    io['dw1'] = _wT_chunks(p['dffn_w1'])
    io['dw2'] = _wT_chunks(p['dffn_w2'])
    io['gaw1'] = _wT_chunks(p['gate_w1'])
    io['gaw2'] = _wT_chunks(p['gate_w2'])
    io['fw'] = _wT_chunks(p['fus_w'])

    io['dmg'] = _diagmask_full(G)
    io['dmd'] = _diagmask_full(M)
    io['ident'] = np.eye(128, dtype=np.float32)

    bo_eff = p['mha_out_b'] + bv @ p['mha_out_w'].T                         # (D,)
    dbo_eff = p['det_bo'] + p['det_bv'] @ p['det_wo'].T
    io['residg'] = (np.tile(gq_eff, (nb, 1)) + bo_eff).astype(np.float32)
    io['dob'] = _bcast_rows(dbo_eff, nb * M)

    io['gn1g'] = _bcast_rows(p['gn1_g'], nb * G)
    io['gn1b'] = _bcast_rows(p['gn1_b'], nb * G)
    io['gn2g'] = _bcast_rows(p['gn2_g'], nb * G)
    io['gn2b'] = _bcast_rows(p['gn2_b'], nb * G)
    io['dng'] = _bcast_rows(p['dn_g'], nb * M)
    io['dnb'] = _bcast_rows(p['dn_b'], nb * M)
    io['gfb1'] = _bcast_rows(p['gffn_b1'], nb * G)
    io['gfb2'] = _bcast_rows(p['gffn_b2'], nb * G)
    io['dfb1'] = _bcast_rows(p['dffn_b1'], nb * M)
    io['dfb2'] = _bcast_rows(p['dffn_b2'], nb * M)
    io['cb1'] = _bcast_rows(p['cond_b1'], nb)
    cb2 = (p['cond_b2'] / np.sqrt(HD)).reshape(NIC, 128).T                  # (128, 4)
    io['cb2t'] = np.ascontiguousarray(
        np.repeat(cb2[:, :, None], nb, axis=2)).astype(np.float32)          # (128,4,nb)
    io['gab1'] = _bcast_rows(p['gate_b1'], nb)
    io['gab2'] = _bcast_rows(p['gate_b2'], nb)
    io['fb'] = _bcast_rows(p['fus_b'], nb)
    io['fg'] = _bcast_rows(p['fus_g'], nb)
    io['fbb'] = _bcast_rows(p['fus_bb'], nb)

    pool4 = np.zeros((nb * G, nb), np.float32)
    for b in range(nb):
        pool4[b * G:(b + 1) * G, b] = 1.0 / G
    io['pool4'] = pool4
    pool16 = np.zeros((nb * M, nb), np.float32)
    for b in range(nb):
        pool16[b * M:(b + 1) * M, b] = 1.0 / M
    io['pool16'] = pool16
    return io


def prep_core_inputs(X_core, mask_core, shared):
    """Per-core data tensors. X_core (nb, N, D) f32, mask_core (nb, N) bool."""
    nb = X_core.shape[0]
    xt = np.ascontiguousarray(
        X_core.transpose(0, 2, 1).reshape(nb, NIC, 128, N)).astype(np.float32)
    m = mask_core.astype(np.float32)                                        # (nb, N)
    maskv = np.ascontiguousarray(
        m.reshape(nb, N // 128, 128).transpose(0, 2, 1))                    # (nb,128,16)
    import ml_dtypes
    io = dict(shared)
    io['xt'] = xt.astype(ml_dtypes.bfloat16)
    io['xtm'] = np.ascontiguousarray(X_core.astype(ml_dtypes.bfloat16))
    io['maskv'] = maskv
    return io


# -------------------------------------------------------------- device build

def build(nb=NB):
    if nb in _BUILD_CACHE:
        return _BUILD_CACHE[nb]
    nc = bacc.Bacc()

    def dp(name, shape, dt=F32R):
        return nc.declare_dram_parameter(name, list(shape), dt, isOutput=False)

    xt_d = dp('xt', (nb, NIC, 128, N), BF16)
    xtm_d = dp('xtm', (nb, N, D), BF16)
    wv_d = dp('wv', (128, NIC, D)); wdv_d = dp('wdv', (128, NIC, D))
    qkg_d = dp('qkg', (128, NIC, H * G), BF16)
    qkdb_d = dp('qkdb', (128, NIC, H * M), BF16)
    wdkr_d = dp('wdkr', (128, NIC, D), BF16); dmh_d = dp('dmh', (128, NIC, H), BF16)
    ones2_d = dp('ones2', (128, 2), BF16)
    identb_d = dp('identb', (128, 128), BF16)
    wo_d = dp('wo', (128, NIC, D)); wdo_d = dp('wdo', (128, NIC, D))
    cw1_d = dp('cw1', (128, NIC, D)); cw2_d = dp('cw2', (128, NIC, D))
    gw1_d = dp('gw1', (128, NIC, DF)); gw2_d = dp('gw2', (128, DF // 128, D))
    dw1_d = dp('dw1', (128, NIC, DF)); dw2_d = dp('dw2', (128, DF // 128, D))
    gaw1_d = dp('gaw1', (128, 2 * NIC, D)); gaw2_d = dp('gaw2', (128, NIC, 2))
    fw_d = dp('fw', (128, 2 * NIC, D))
    dmg_d = dp('dmg', (128, NIC, H * G)); dmd_d = dp('dmd', (128, NIC, H * M))
    ident_d = dp('ident', (128, 128))
    maskv_d = dp('maskv', (nb, 128, N // 128), F32)
    residg_d = dp('residg', (nb * G, D), F32)
    dob_d = dp('dob', (nb * M, D), F32)
    gn1g_d = dp('gn1g', (nb * G, D), F32); gn1b_d = dp('gn1b', (nb * G, D), F32)
    gn2g_d = dp('gn2g', (nb * G, D), F32); gn2b_d = dp('gn2b', (nb * G, D), F32)
    dng_d = dp('dng', (nb * M, D), F32); dnb_d = dp('dnb', (nb * M, D), F32)
    gfb1_d = dp('gfb1', (nb * G, DF), F32); gfb2_d = dp('gfb2', (nb * G, D), F32)
    dfb1_d = dp('dfb1', (nb * M, DF), F32); dfb2_d = dp('dfb2', (nb * M, D), F32)
    cb1_d = dp('cb1', (nb, D), F32); cb2t_d = dp('cb2t', (128, NIC, nb), F32)
    gab1_d = dp('gab1', (nb, D), F32); gab2_d = dp('gab2', (nb, 2), F32)
    fb_d = dp('fb', (nb, D), F32); fg_d = dp('fg', (nb, D), F32)
    fbb_d = dp('fbb', (nb, D), F32)
    pool4_d = dp('pool4', (nb * G, nb)); pool16_d = dp('pool16', (nb * M, nb))

    zg_o = nc.declare_dram_parameter('zg', [nb, D], F32, isOutput=True)
    zd_o = nc.declare_dram_parameter('zd', [nb, D], F32, isOutput=True)
    zu_o = nc.declare_dram_parameter('zu', [nb, D], F32, isOutput=True)

    with tile.TileContext(nc) as tc, contextlib.ExitStack() as ctx:
        wpool = ctx.enter_context(tc.tile_pool(name="w", bufs=1))
        ffnw = ctx.enter_context(tc.tile_pool(name="ffnw", bufs=2))
        cpool = ctx.enter_context(tc.tile_pool(name="c", bufs=1))
        xpool = ctx.enter_context(tc.tile_pool(name="x", bufs=3))
        kpool = ctx.enter_context(tc.tile_pool(name="k", bufs=2))
        vpool = ctx.enter_context(tc.tile_pool(name="v", bufs=6))
        epool = ctx.enter_context(tc.tile_pool(name="e", bufs=6))
        spool = ctx.enter_context(tc.tile_pool(name="s", bufs=2))
        tpool = ctx.enter_context(tc.tile_pool(name="t", bufs=1))
        hpool = ctx.enter_context(tc.tile_pool(name="h", bufs=1))
        opool = ctx.enter_context(tc.tile_pool(name="o", bufs=1))
        ppP = ctx.enter_context(tc.tile_pool(name="ppP", bufs=1, space="PSUM"))
        ppS = ctx.enter_context(tc.tile_pool(name="ppS", bufs=2, space="PSUM"))
        ppT = ctx.enter_context(tc.tile_pool(name="ppT", bufs=2, space="PSUM"))
        ppO = ctx.enter_context(tc.tile_pool(name="ppO", bufs=2, space="PSUM"))
        ppD = ctx.enter_context(tc.tile_pool(name="ppD", bufs=1, space="PSUM"))

        def wtile(dram, shape, dt=F32R, pool=None, tag=None):
            t = (pool or wpool).tile(list(shape), dt, tag=tag or dram.name)
            nc.scalar.dma_start(out=t, in_=dram[tuple(slice(None) for _ in shape)])
            return t

        wv = wtile(wv_d, (128, NIC, D))
        wdv = wtile(wdv_d, (128, NIC, D))
        wdkr = wtile(wdkr_d, (128, NIC, D), BF16)
        qkg = wtile(qkg_d, (128, NIC, H * G), BF16)
        qkdb = wtile(qkdb_d, (128, NIC, H * M), BF16)
        dmh = wtile(dmh_d, (128, NIC, H), BF16)
        ones2 = wtile(ones2_d, (128, 2), BF16)
        identb = wtile(identb_d, (128, 128), BF16)
        gaw2 = wtile(gaw2_d, (128, NIC, 2))
        dmg = wtile(dmg_d, (128, NIC, H * G))
        dmd = wtile(dmd_d, (128, NIC, H * M))
        ident = wtile(ident_d, (128, 128))
        identf = ident.bitcast(F32)
        pool4 = wtile(pool4_d, (nb * G, nb))
        pool16 = wtile(pool16_d, (nb * M, nb))

        residg = wtile(residg_d, (nb * G, D), F32, cpool)
        dob = wtile(dob_d, (nb * M, D), F32, cpool)
        gn1g = wtile(gn1g_d, (nb * G, D), F32, cpool)
        gn1b = wtile(gn1b_d, (nb * G, D), F32, cpool)
        gn2g = wtile(gn2g_d, (nb * G, D), F32, cpool)
        gn2b = wtile(gn2b_d, (nb * G, D), F32, cpool)
        dng = wtile(dng_d, (nb * M, D), F32, cpool)
        dnb = wtile(dnb_d, (nb * M, D), F32, cpool)
        gfb2 = wtile(gfb2_d, (nb * G, D), F32, cpool)
        dfb2 = wtile(dfb2_d, (nb * M, D), F32, cpool)
        cb1 = wtile(cb1_d, (nb, D), F32, cpool)
        cb2t = wtile(cb2t_d, (128, NIC, nb), F32, cpool)
        gab1 = wtile(gab1_d, (nb, D), F32, cpool)
        gab2 = wtile(gab2_d, (nb, 2), F32, cpool)
        fb = wtile(fb_d, (nb, D), F32, cpool)
        fg = wtile(fg_d, (nb, D), F32, cpool)
        fbb = wtile(fbb_d, (nb, D), F32, cpool)

        maskv = []
        for b in range(nb):
            mv = cpool.tile([128, N // 128], F32, tag=f"maskv{b}")
            nc.sync.dma_start(out=mv, in_=maskv_d[b])
            maskv.append(mv)

        eps_g = cpool.tile([nb * G, 1], F32)
        nc.vector.memset(eps_g, 1e-5)
        eps_d = cpool.tile([nb * M, 1], F32)
        nc.vector.memset(eps_d, 1e-5)
        eps_b = cpool.tile([nb, 1], F32)
        nc.vector.memset(eps_b, 1e-5)

        # ---------------------------------------------------------- helpers

        def attention_pass(b, qkT, ncols, c_ps, den_ps):
            """scores = qkT.T @ X^T (K-proj folded into queries host-side);
            C += (E.m)^T.T @ X (V-proj deferred: out = C @ Wv^T later)."""
            for tb in range(NTB):
                xblk = xpool.tile([128, NIC, 512], BF16, tag="xblk")
                nc.sync.dma_start(
                    out=xblk,
                    in_=xt_d[b, :, :, tb * 512:(tb + 1) * 512].rearrange(
                        "ic p t -> p ic t"))
                pss = ppS.tile([ncols, 512], F32, tag="scores")
                for j in range(NIC):
                    nc.tensor.matmul(pss, qkT[:, j, :], xblk[:, j, :],
                                     start=(j == 0), stop=(j == NIC - 1))
                eb = epool.tile([ncols, 512], BF16, tag="eblk")
                nc.scalar.activation(out=eb, in_=pss, func=AF.Exp)
                for c in range(4):
                    pst = ppT.tile([128, ncols], BF16, tag="tr")
                    nc.tensor.transpose(
                        pst, eb[:, c * 128:(c + 1) * 128], identb[:ncols, :ncols])
                    tcg = tb * 4 + c
                    et = epool.tile([128, ncols], BF16, tag="et")
                    nc.vector.tensor_scalar_mul(et, pst, maskv[b][:, tcg:tcg + 1])
                    xtm = vpool.tile([128, 512], BF16, tag="xtm")
                    nc.sync.dma_start(
                        out=xtm, in_=xtm_d[b, tcg * 128:(tcg + 1) * 128, :])
                    first = (tb == 0 and c == 0)
                    last = (tb == NTB - 1 and c == 3)
                    nc.tensor.matmul(c_ps, et, xtm, start=first, stop=last)
                    nc.tensor.matmul(den_ps, et, ones2,
                                     start=first, stop=last)

        def finish_attention(ncols, cph, c_ps, den_ps, wvt, dmask, oT_all, b):
            den = spool.tile([ncols, 1], F32, tag="den")
            nc.vector.tensor_copy(den, den_ps[:, 0:1])
            rec = spool.tile([ncols, 1], F32, tag="rec")
            nc.vector.reciprocal(rec, den)
            csb = spool.tile([ncols, 512], F32, tag="csb")
            nc.vector.tensor_scalar_mul(csb, c_ps, rec)
            cT = spool.tile([128, NIC, ncols], F32R, tag="cT")
            for ic in range(NIC):
                pst = ppT.tile([128, ncols], F32, tag="tr")
                nc.tensor.transpose(
                    pst, csb[:, ic * 128:(ic + 1) * 128], identf[:ncols, :ncols])
                nc.vector.tensor_copy(cT[:, ic, :], pst)
            nps = ppP.tile([ncols, 512], F32, tag="proj")
            for ic in range(NIC):
                nc.tensor.matmul(nps, cT[:, ic, :], wvt[:, ic, :],
                                 start=(ic == 0), stop=(ic == NIC - 1))
            osb = spool.tile([ncols, 512], F32, tag="osb")
            nc.vector.tensor_copy(osb, nps)
            for ic in range(NIC):
                pst = ppT.tile([128, ncols], F32, tag="tr")
                nc.tensor.transpose(
                    pst, osb[:, ic * 128:(ic + 1) * 128], identf[:ncols, :ncols])
                ocl = spool.tile([128, ncols], F32, tag="ocl")
                nc.vector.tensor_tensor(out=ocl, in0=pst,
                                        in1=dmask[:, ic, :].bitcast(F32), op=OP.mult)
                red = spool.tile([128, cph], F32, tag="red")
                nc.vector.tensor_reduce(
                    out=red, in_=ocl.rearrange("p (h c) -> p c h", c=cph),
                    axis=AX.X, op=OP.add)
                last = nc.vector.tensor_copy(
                    oT_all[:, ic, b * cph:(b + 1) * cph], red)
            return last

        def transpose_to_fm(src, rows, tag, n_chunks=NIC, src_f32r=False):
            """src (rows, n_chunks*128) sbuf -> (128, n_chunks, rows) f32r."""
            out = tpool.tile([128, n_chunks, rows], F32R, tag=tag)
            for ic in range(n_chunks):
                if src_f32r:
                    pst = ppT.tile([128, rows], F32R, tag="tr")
                    nc.tensor.transpose(pst, src[:, ic * 128:(ic + 1) * 128],
                                        ident[:rows, :rows])
                else:
                    pst = ppT.tile([128, rows], F32, tag="tr")
                    nc.tensor.transpose(pst, src[:, ic * 128:(ic + 1) * 128],
                                        identf[:rows, :rows])
                nc.vector.tensor_copy(out[:, ic, :], pst)
            return out

        def linear_rows_w(lhsT, wget, n_ic, rows, cols):
            ps = ppP.tile([rows, cols], F32, tag="proj")
            for ic in range(n_ic):
                nc.tensor.matmul(ps, lhsT(ic), wget(ic),
                                 start=(ic == 0), stop=(ic == n_ic - 1))
            return ps

        def linear_rows(lhsT, w_tile, n_ic, rows, cols):
            ps = ppP.tile([rows, cols], F32, tag="proj")
            for ic in range(n_ic):
                nc.tensor.matmul(ps, lhsT(ic), w_tile[:, ic, :cols],
                                 start=(ic == 0), stop=(ic == n_ic - 1))
            return ps

        def layernorm(x, rows, eps_t, gamma, beta, resid=None, bias=None,
                      tag="ln"):
            """LN over free dim D. x may be psum. Returns f32 sbuf (rows, D)."""
            pre = tpool.tile([rows, D], F32, tag="lnpre")
            if bias is not None:
                nc.vector.tensor_tensor(out=pre, in0=x, in1=bias, op=OP.add)
            else:
                nc.vector.tensor_copy(pre, x)
            if resid is not None:
                nc.vector.tensor_tensor(out=pre, in0=pre, in1=resid, op=OP.add)
            stats = tpool.tile([rows, 6], F32, tag="lnst")
            nc.vector.bn_stats(out=stats, in_=pre)
            mv = tpool.tile([rows, 2], F32, tag="lnmv")
            nc.vector.bn_aggr(out=mv, in_=stats)
            rstd = tpool.tile([rows, 1], F32, tag="lnrs")
            nc.scalar.activation(out=rstd, in_=mv[:, 1:2], func=AF.Sqrt, bias=eps_t)
            nc.vector.reciprocal(rstd, rstd)
            nc.vector.tensor_scalar(out=pre, in0=pre, scalar1=mv[:, 0:1],
                                    scalar2=rstd, op0=OP.subtract, op1=OP.mult)
            out = tpool.tile([rows, D], F32, tag=tag + "out")
            nc.vector.tensor_tensor(out=pre, in0=pre, in1=gamma, op=OP.mult)
            last = nc.vector.tensor_tensor(out=out, in0=pre, in1=beta, op=OP.add)
            return out, pre, last

        from concourse.tile import add_dep_helper

        def pin(anchor, inst):
            if anchor is not None:
                add_dep_helper(inst.ins, anchor.ins, reason="phase pin")

        def ffn_block(z_sb, rows, w1_d, w2_d, b1_d, tagp, anchor):
            """psum(rows, D) = W2 @ gelu(W1 @ z + b1), bias2 NOT added.
            Weights and b1 streamed from DRAM chunk by chunk; every stream
            DMA is pinned after `anchor` so the scheduler cannot hoist it
            into an earlier phase (slot-wait head-of-line deadlock)."""
            zT = transpose_to_fm(z_sb, rows, tagp + "zT")
            h1 = hpool.tile([rows, DF], F32R, tag=tagp + "h1")
            for og in range(DF // 512):
                b1c = ffnw.tile([rows, 512], F32, tag="bs1")
                pin(anchor, nc.scalar.dma_start(
                    out=b1c, in_=b1_d[:, og * 512:(og + 1) * 512]))
                ps = ppP.tile([rows, 512], F32, tag="proj")
                for ic in range(NIC):
                    w1c = ffnw.tile([128, 512], F32R, tag="ws1")
                    pin(anchor, nc.scalar.dma_start(
                        out=w1c, in_=w1_d[:, ic, og * 512:(og + 1) * 512]))
                    nc.tensor.matmul(ps, zT[:, ic, :], w1c,
                                     start=(ic == 0), stop=(ic == NIC - 1))
                hb = tpool.tile([rows, 512], F32, tag="ffnhb")
                nc.vector.tensor_tensor(out=hb, in0=ps, in1=b1c, op=OP.add)
                nc.scalar.activation(out=h1[:, og * 512:(og + 1) * 512], in_=hb,
                                     func=AF.Gelu)
            h1v = h1.rearrange("r (cc p) -> r cc p", p=128)
            ps2 = ppP.tile([rows, D], F32, tag="proj")
            for cc in range(DF // 128):
                w2c = ffnw.tile([128, 512], F32R, tag="ws2")
                pin(anchor, nc.scalar.dma_start(out=w2c, in_=w2_d[:, cc, :]))
                pst = ppT.tile([128, rows], F32R, tag="tr")
                nc.tensor.transpose(pst, h1v[:, cc, :], ident[:rows, :rows])
                h1T = tpool.tile([128, rows], F32R, tag="ffnh1T")
                nc.vector.tensor_copy(h1T, pst)
                nc.tensor.matmul(ps2, h1T, w2c,
                                 start=(cc == 0), stop=(cc == DF // 128 - 1))
            return ps2

        def _stream_chunks(dram, cols, anchor, tag="ws1"):
            def get(ic):
                t = ffnw.tile([128, cols], F32R, tag=tag)
                pin(anchor, nc.scalar.dma_start(out=t, in_=dram[:, ic, :cols]))
                return t
            return get

        # ---------------- pass 1: global attention ----------------
        o_gT_all = opool.tile([128, NIC, nb * G], F32R, tag="ogT")
        for b in range(nb):
            c_ps = ppO.tile([H * G, 512], F32, tag="av")
            den_ps = ppD.tile([H * G, 2], F32, tag="den")
            attention_pass(b, qkg, H * G, c_ps, den_ps)
            p1_anchor = finish_attention(H * G, G, c_ps, den_ps, wv, dmg,
                                         o_gT_all, b)

        # ---------------- batched global tail ----------------
        zps = linear_rows_w(lambda ic: o_gT_all[:, ic, :],
                            _stream_chunks(wo_d, D, p1_anchor),
                            NIC, nb * G, D)
        z1, _, z1_inst = layernorm(zps, nb * G, eps_g, gn1g, gn1b,
                                   resid=residg, tag="g1")
        gffn_ps = ffn_block(z1, nb * G, gw1_d, gw2_d, gfb1_d, "gf", z1_inst)
        zg_b = tpool.tile([nb * G, D], F32, tag="zgb")
        nc.vector.tensor_tensor(out=zg_b, in0=gffn_ps, in1=gfb2, op=OP.add)
        z_global, _, _zg_inst = layernorm(zg_b, nb * G, eps_g, gn2g, gn2b, resid=z1,
                                tag="g2")
        z_global_r = tpool.tile([nb * G, D], F32R, tag="zgr")
        nc.vector.tensor_copy(z_global_r, z_global)

        psp = ppS.tile([nb, D], F32, tag="scores")
        nc.tensor.matmul(psp, pool4, z_global_r, start=True, stop=True)
        zgp = tpool.tile([nb, D], F32R, tag="zgp")
        nc.vector.tensor_copy(zgp, psp)
        nc.sync.dma_start(out=zg_o[:, :], in_=zgp.bitcast(F32))
        condT = transpose_to_fm(zgp.bitcast(F32), nb, "condT")

        # modn chain
        m1ps = linear_rows_w(lambda ic: condT[:, ic, :],
                             _stream_chunks(cw1_d, D, _zg_inst),
                             NIC, nb, D)
        m1b = tpool.tile([nb, D], F32, tag="m1b")
        nc.vector.tensor_tensor(out=m1b, in0=m1ps, in1=cb1, op=OP.add)
        m1 = tpool.tile([nb, D], F32R, tag="m1")
        m1_inst = nc.scalar.activation(out=m1, in_=m1b, func=AF.Gelu)
        m1T = transpose_to_fm(m1.bitcast(F32), nb, "m1T")
        modnT = tpool.tile([128, NIC, nb], F32, tag="modnT")
        for oc in range(NIC):
            psm = ppS.tile([128, nb], F32, tag="scores")
            for ic in range(NIC):
                cw2c = ffnw.tile([128, 128], F32R, tag="ws2")
                pin(m1_inst, nc.scalar.dma_start(
                    out=cw2c, in_=cw2_d[:, ic, oc * 128:(oc + 1) * 128]))
                nc.tensor.matmul(psm, cw2c,
                                 m1T[:, ic, :], start=(ic == 0),
                                 stop=(ic == NIC - 1))
            nc.vector.tensor_tensor(out=modnT[:, oc, :], in0=psm,
                                    in1=cb2t[:, oc, :], op=OP.add)

        # ---------------- pass 2: detail attention ----------------
        o_dT_all = opool.tile([128, NIC, nb * M], F32R, tag="odT")
        for b in range(nb):
            # fold modn into the folded detail queries:
            # qkd = qkd_base + (modn block-diag) @ Wdk  (broadcast over m)
            mblk = spool.tile([128, NIC, H], BF16, tag="mblk")
            for j in range(NIC):
                nc.vector.tensor_scalar_mul(mblk[:, j, :], dmh[:, j, :],
                                            modnT[:, j, b:b + 1])
            mkT = spool.tile([128, NIC, H], BF16, tag="mkT")
            for ic in range(NIC):
                psm2 = ppT.tile([128, H], F32, tag="tr")
                for j in range(NIC):
                    nc.tensor.matmul(psm2,
                                     wdkr[:, j, ic * 128:(ic + 1) * 128],
                                     mblk[:, j, :], start=(j == 0),
                                     stop=(j == NIC - 1))
                nc.vector.tensor_copy(mkT[:, ic, :], psm2)
            qde = spool.tile([128, NIC, H * M], BF16, tag="qde")
            for ic in range(NIC):
                nc.vector.tensor_tensor(
                    out=qde[:, ic, :].rearrange("p (h m) -> p h m", m=M),
                    in0=qkdb[:, ic, :].rearrange("p (h m) -> p h m", m=M),
                    in1=mkT[:, ic, :].unsqueeze(-1).broadcast_to([128, H, M]),
                    op=OP.add)
            c_ps = ppO.tile([H * M, 512], F32, tag="av")
            den_ps = ppD.tile([H * M, 2], F32, tag="den")
            attention_pass(b, qde, H * M, c_ps, den_ps)
            p2_anchor = finish_attention(H * M, M, c_ps, den_ps, wdv, dmd,
                                         o_dT_all, b)

        # ---------------- batched detail tail ----------------
        zdps = linear_rows_w(lambda ic: o_dT_all[:, ic, :],
                             _stream_chunks(wdo_d, D, p2_anchor),
                             NIC, nb * M, D)
        z1d = tpool.tile([nb * M, D], F32, tag="z1d")
        z1d_inst = nc.vector.tensor_tensor(out=z1d, in0=zdps, in1=dob, op=OP.add)
        dffn_ps = ffn_block(z1d, nb * M, dw1_d, dw2_d, dfb1_d, "df", z1d_inst)
        zd_b = tpool.tile([nb * M, D], F32, tag="zdb")
        nc.vector.tensor_tensor(out=zd_b, in0=dffn_ps, in1=dfb2, op=OP.add)
        z_detail, _, zdet_inst = layernorm(zd_b, nb * M, eps_d, dng, dnb, resid=z1d,
                                tag="dn")
        z_detail_r = tpool.tile([nb * M, D], F32R, tag="zdr")
        nc.vector.tensor_copy(z_detail_r, z_detail)

        psdp = ppS.tile([nb, D], F32, tag="scores")
        nc.tensor.matmul(psdp, pool16, z_detail_r, start=True, stop=True)
        zdp = tpool.tile([nb, D], F32R, tag="zdp")
        nc.vector.tensor_copy(zdp, psdp)
        nc.sync.dma_start(out=zd_o[:, :], in_=zdp.bitcast(F32))
        zdpT = transpose_to_fm(zdp.bitcast(F32), nb, "zdpT")

        # ---------------- gated fusion ----------------
        def giT(ic):
            return condT[:, ic, :] if ic < NIC else zdpT[:, ic - NIC, :]

        gaw1g = _stream_chunks(gaw1_d, D, zdet_inst)
        g1ps = linear_rows_w(giT, gaw1g, 2 * NIC, nb, D)
        g1b = tpool.tile([nb, D], F32, tag="g1b")
        nc.vector.tensor_tensor(out=g1b, in0=g1ps, in1=gab1, op=OP.add)
        g1 = tpool.tile([nb, D], F32R, tag="g1")
        nc.scalar.activation(out=g1, in_=g1b, func=AF.Gelu)
        g1T = transpose_to_fm(g1.bitcast(F32), nb, "g1T")
        lgps = linear_rows(lambda ic: g1T[:, ic, :], gaw2, NIC, nb, 2)
        lg = tpool.tile([nb, 2], F32, tag="lg")
        nc.vector.tensor_tensor(out=lg, in0=lgps, in1=gab2, op=OP.add)
        eg = tpool.tile([nb, 2], F32, tag="eg")
        nc.scalar.activation(out=eg, in_=lg, func=AF.Exp)
        egs = tpool.tile([nb, 1], F32, tag="egs")
        nc.vector.tensor_reduce(out=egs, in_=eg, axis=AX.X, op=OP.add)
        nc.vector.reciprocal(egs, egs)
        gate = tpool.tile([nb, 2], F32, tag="gate")
        nc.vector.tensor_scalar_mul(gate, eg, egs)

        zw = tpool.tile([nb, D], F32, tag="zw")
        nc.vector.tensor_scalar_mul(zw, zgp.bitcast(F32), gate[:, 0:1])
        zw2 = tpool.tile([nb, D], F32, tag="zw2")
        nc.vector.tensor_scalar_mul(zw2, zdp.bitcast(F32), gate[:, 1:2])
        nc.vector.tensor_tensor(out=zw, in0=zw, in1=zw2, op=OP.add)

        fwg = _stream_chunks(fw_d, D, zdet_inst)
        fps = linear_rows_w(giT, fwg, 2 * NIC, nb, D)
        fzb = tpool.tile([nb, D], F32, tag="fzb")
        nc.vector.tensor_tensor(out=fzb, in0=fps, in1=fb, op=OP.add)
        fzg = tpool.tile([nb, D], F32, tag="fzg")
        nc.scalar.activation(out=fzg, in_=fzb, func=AF.Gelu)
        fln, _, _ = layernorm(fzg, nb, eps_b, fg, fbb, tag="fln")
        zu = tpool.tile([nb, D], F32, tag="zu")
        nc.vector.tensor_tensor(out=zu, in0=fln, in1=zw, op=OP.add)
        nc.sync.dma_start(out=zu_o[:, :], in_=zu)

    nc.finalize()
    _BUILD_CACHE[nb] = nc
    return nc


# ------------------------------------------------------------------- driver

LAST_EXEC_NS = None


def kernel(X_tokens, mask, params, _trace=False):
    global LAST_EXEC_NS
    X = np.asarray(X_tokens, np.float32)
    mk = np.asarray(mask, bool)
    nb = X.shape[0] // NCORES
    shared = prep_host(params, nb)
    in_maps = [
        prep_core_inputs(X[c * nb:(c + 1) * nb], mk[c * nb:(c + 1) * nb], shared)
        for c in range(NCORES)
    ]
    nc = build(nb)
    res = run_bass_kernel_spmd(nc, in_maps, list(range(NCORES)), trace=_trace)
    LAST_EXEC_NS = res.exec_time_ns
    out = np.empty((3, X.shape[0], D), np.float32)
    for c in range(NCORES):
        r = res.results[c]
        out[0, c * nb:(c + 1) * nb] = r['zg']
        out[1, c * nb:(c + 1) * nb] = r['zd']
        out[2, c * nb:(c + 1) * nb] = r['zu']
    return out


# revision 21
# speedup vs baseline: 1.6668x; 1.0678x over previous
"""CLIP4CAD_HUS_v2 fused forward on 8 Trainium2 NeuronCores.

Data-parallel over batch B=64 (8 batches per core), params replicated.

Key algebraic restructuring (only G=4 / M=16 queries exist per head, so
the full K/V projections over N=2048 tokens are never materialized):
  scores = (q @ Wk) @ X^T        -- Wk folded into the query rows on host
  attn   = ((E*mask) @ X) @ Wv^T -- Wv applied after the n-contraction
E = exp(scores) without max subtraction (scores are O(1) by construction);
masked softmax denominator via a ones-column matvec on E^T. Per-head
output blocks are extracted with a static diag mask + strided h-sum.

Per core:
  pass 1 (per batch):  global cross-attention as above
  batched global tail: out-proj + LN + FFN + LN + modn chain (rows b*G+g)
  pass 2 (per batch):  detail attention, modn folded into the detail
                       queries via a block-diag matvec against Wdk rows
  batched detail tail: out-proj + FFN + LN, gated fusion, outputs

Layouts:
  feature-major tile (128, C, T): [p, c, t] = tensor[c*128+p, t]
  matmul: out[M,N] = lhsT[K,M].T @ rhs[K,N]   (K = partition dim)
Matmul operands are float32r (tf32-class PE precision at bf16 speed).
"""

import contextlib

import numpy as np

import concourse.bass as bass
import concourse.mybir as mybir
import concourse.tile as tile
from concourse import bacc
from concourse.bass_utils import run_bass_kernel_spmd

F32 = mybir.dt.float32
F32R = mybir.dt.float32r
BF16 = mybir.dt.bfloat16
AF = mybir.ActivationFunctionType
OP = mybir.AluOpType
AX = mybir.AxisListType

D, H, G, M = 512, 8, 4, 16
HD = D // H
DF = 4 * D
B, N = 64, 2048
NCORES = 8
NB = B // NCORES          # batches per core
NTB = N // 512            # 512-token blocks per batch
NIC = D // 128            # feature chunks of d=512

_BUILD_CACHE = {}


# ----------------------------------------------------------------- host prep

def _wT_chunks(w):
    """(out,in) torch-Linear weight -> (128, in/128, out) chunk layout."""
    wt = np.ascontiguousarray(np.asarray(w, np.float32).T)      # (in, out)
    ic = wt.shape[0] // 128
    return np.ascontiguousarray(
        wt.reshape(ic, 128, wt.shape[1]).transpose(1, 0, 2))


def _qblock_full(q, cph):
    """q (H*cph, D) -> (128, 4, H*cph) block-diag (chunks accumulated over j).

    out[p, j, h*cph+r] = q[h*cph+r, j*128+p] iff h == 2j + p//64, else 0.
    """
    out = np.zeros((128, NIC, H * cph), np.float32)
    q = np.asarray(q, np.float32)
    for j in range(NIC):
        for pl in range(2):
            h = 2 * j + pl
            rows = slice(pl * 64, (pl + 1) * 64)
            out[rows, j, h * cph:(h + 1) * cph] = \
                q[h * cph:(h + 1) * cph, j * 128 + pl * 64:j * 128 + (pl + 1) * 64].T
    return out


def _diagmask_full(cph):
    """(128, 4, H*cph): 1 iff col's head == 2j + p//64."""
    out = np.zeros((128, NIC, H * cph), np.float32)
    for j in range(NIC):
        for pl in range(2):
            h = 2 * j + pl
            out[pl * 64:(pl + 1) * 64, j, h * cph:(h + 1) * cph] = 1.0
    return out


def _bcast_rows(v, rows):
    v = np.asarray(v, np.float32)
    return np.ascontiguousarray(np.broadcast_to(v, (rows, v.shape[-1])))


def prep_host(params, nb=NB):
    p = {k: np.asarray(v, np.float32) for k, v in params.items()}
    io = {}

    adapt = np.tanh(p['mod_embed'][1] @ p['adapt_w'].T + p['adapt_b'])      # (D,)
    gq_eff = p['gq'][0] + 0.1 * adapt                                       # (G, D)
    dq_eff = p['dq'][0] + 0.1 * adapt                                       # (M, D)

    wq, wk, wv = np.split(p['mha_in_w'], 3, 0)
    bq, bk, bv = np.split(p['mha_in_b'], 3, 0)
    qg = (gq_eff @ wq.T + bq) / np.sqrt(HD)                                 # (G, D) -> rows h*G? no: (G,D)
    qd_base = (dq_eff @ p['det_wq'].T + p['det_bq']) / np.sqrt(HD)          # (M, D)

    # reorder query rows to (h, g): q_hg[h*cph+r, :] = q[r, h-th 64-slice...]
    # NOT a reorder of rows: _qblock_full wants q indexed [h*cph+r, d] where
    # the (h, r) query vector is q[r, :] restricted to head h's d-slice.
    # Build expanded (H*cph, D) with rows (h, r) = original row r.

    # fold K-projection weights into the (few) query rows: scores = qk @ X^T
    qkg = np.zeros((H * G, D), np.float32)
    qkd = np.zeros((H * M, D), np.float32)
    for h in range(H):
        sl = slice(h * HD, (h + 1) * HD)
        qkg[h * G:(h + 1) * G] = qg[:, sl] @ wk[sl, :]
        qkd[h * M:(h + 1) * M] = qd_base[:, sl] @ p['det_wk'][sl, :]
    import ml_dtypes
    io['qkg'] = _wT_chunks(qkg).astype(ml_dtypes.bfloat16)              # (128,4,32)
    io['qkdb'] = _wT_chunks(qkd).astype(ml_dtypes.bfloat16)             # (128,4,128)
    io['wdkr'] = _wT_chunks(p['det_wk'].T).astype(ml_dtypes.bfloat16)
    io['dmh'] = _diagmask_full(1).astype(ml_dtypes.bfloat16)            # (128,4,8)
    io['ones2'] = np.ones((128, 2), ml_dtypes.bfloat16)
    io['identb'] = np.eye(128, dtype=ml_dtypes.bfloat16)
    io['wv'] = _wT_chunks(wv)
    io['wdv'] = _wT_chunks(p['det_wv'])
    io['wo'] = _wT_chunks(p['mha_out_w'])
    io['wdo'] = _wT_chunks(p['det_wo'])
    io['cw1'] = _wT_chunks(p['cond_w1'])
    io['cw2'] = _wT_chunks(p['cond_w2'] / np.sqrt(HD))
    io['gw1'] = _wT_chunks(p['gffn_w1'])
    io['gw2'] = _wT_chunks(p['gffn_w2'])
    io['dw1'] = _wT_chunks(p['dffn_w1'])
    io['dw2'] = _wT_chunks(p['dffn_w2'])
    io['gaw1'] = _wT_chunks(p['gate_w1'])
    io['gaw2'] = _wT_chunks(p['gate_w2'])
    io['fw'] = _wT_chunks(p['fus_w'])

    io['dmg'] = _diagmask_full(G)
    io['dmd'] = _diagmask_full(M)
    io['ident'] = np.eye(128, dtype=np.float32)

    bo_eff = p['mha_out_b'] + bv @ p['mha_out_w'].T                         # (D,)
    dbo_eff = p['det_bo'] + p['det_bv'] @ p['det_wo'].T
    io['residg'] = (np.tile(gq_eff, (nb, 1)) + bo_eff).astype(np.float32)
    io['dob'] = _bcast_rows(dbo_eff, nb * M)

    io['gn1g'] = _bcast_rows(p['gn1_g'], nb * G)
    io['gn1b'] = _bcast_rows(p['gn1_b'], nb * G)
    io['gn2g'] = _bcast_rows(p['gn2_g'], nb * G)
    io['gn2b'] = _bcast_rows(p['gn2_b'], nb * G)
    io['dng'] = _bcast_rows(p['dn_g'], nb * M)
    io['dnb'] = _bcast_rows(p['dn_b'], nb * M)
    io['gfb1'] = _bcast_rows(p['gffn_b1'], nb * G)
    io['gfb2'] = _bcast_rows(p['gffn_b2'], nb * G)
    io['dfb1'] = _bcast_rows(p['dffn_b1'], nb * M)
    io['dfb2'] = _bcast_rows(p['dffn_b2'], nb * M)
    io['cb1'] = _bcast_rows(p['cond_b1'], nb)
    cb2 = (p['cond_b2'] / np.sqrt(HD)).reshape(NIC, 128).T                  # (128, 4)
    io['cb2t'] = np.ascontiguousarray(
        np.repeat(cb2[:, :, None], nb, axis=2)).astype(np.float32)          # (128,4,nb)
    io['gab1'] = _bcast_rows(p['gate_b1'], nb)
    io['gab2'] = _bcast_rows(p['gate_b2'], nb)
    io['fb'] = _bcast_rows(p['fus_b'], nb)
    io['fg'] = _bcast_rows(p['fus_g'], nb)
    io['fbb'] = _bcast_rows(p['fus_bb'], nb)

    pool4 = np.zeros((nb * G, nb), np.float32)
    for b in range(nb):
        pool4[b * G:(b + 1) * G, b] = 1.0 / G
    io['pool4'] = pool4
    pool16 = np.zeros((nb * M, nb), np.float32)
    for b in range(nb):
        pool16[b * M:(b + 1) * M, b] = 1.0 / M
    io['pool16'] = pool16
    return io


def prep_core_inputs(X_core, mask_core, shared):
    """Per-core data tensors. X_core (nb, N, D) f32, mask_core (nb, N) bool."""
    nb = X_core.shape[0]
    xt = np.ascontiguousarray(
        X_core.transpose(0, 2, 1).reshape(nb, NIC, 128, N)).astype(np.float32)
    m = mask_core.astype(np.float32)                                        # (nb, N)
    maskv = np.ascontiguousarray(
        m.reshape(nb, N // 128, 128).transpose(0, 2, 1))                    # (nb,128,16)
    import ml_dtypes
    io = dict(shared)
    io['xt'] = xt.astype(ml_dtypes.bfloat16)
    io['xtm'] = np.ascontiguousarray(X_core.astype(ml_dtypes.bfloat16))
    io['maskv'] = maskv
    return io


# -------------------------------------------------------------- device build

def build(nb=NB):
    if nb in _BUILD_CACHE:
        return _BUILD_CACHE[nb]
    nc = bacc.Bacc()

    def dp(name, shape, dt=F32R):
        return nc.declare_dram_parameter(name, list(shape), dt, isOutput=False)

    xt_d = dp('xt', (nb, NIC, 128, N), BF16)
    xtm_d = dp('xtm', (nb, N, D), BF16)
    wv_d = dp('wv', (128, NIC, D)); wdv_d = dp('wdv', (128, NIC, D))
    qkg_d = dp('qkg', (128, NIC, H * G), BF16)
    qkdb_d = dp('qkdb', (128, NIC, H * M), BF16)
    wdkr_d = dp('wdkr', (128, NIC, D), BF16); dmh_d = dp('dmh', (128, NIC, H), BF16)
    ones2_d = dp('ones2', (128, 2), BF16)
    identb_d = dp('identb', (128, 128), BF16)
    wo_d = dp('wo', (128, NIC, D)); wdo_d = dp('wdo', (128, NIC, D))
    cw1_d = dp('cw1', (128, NIC, D)); cw2_d = dp('cw2', (128, NIC, D))
    gw1_d = dp('gw1', (128, NIC, DF)); gw2_d = dp('gw2', (128, DF // 128, D))
    dw1_d = dp('dw1', (128, NIC, DF)); dw2_d = dp('dw2', (128, DF // 128, D))
    gaw1_d = dp('gaw1', (128, 2 * NIC, D)); gaw2_d = dp('gaw2', (128, NIC, 2))
    fw_d = dp('fw', (128, 2 * NIC, D))
    dmg_d = dp('dmg', (128, NIC, H * G)); dmd_d = dp('dmd', (128, NIC, H * M))
    ident_d = dp('ident', (128, 128))
    maskv_d = dp('maskv', (nb, 128, N // 128), F32)
    residg_d = dp('residg', (nb * G, D), F32)
    dob_d = dp('dob', (nb * M, D), F32)
    gn1g_d = dp('gn1g', (nb * G, D), F32); gn1b_d = dp('gn1b', (nb * G, D), F32)
    gn2g_d = dp('gn2g', (nb * G, D), F32); gn2b_d = dp('gn2b', (nb * G, D), F32)
    dng_d = dp('dng', (nb * M, D), F32); dnb_d = dp('dnb', (nb * M, D), F32)
    gfb1_d = dp('gfb1', (nb * G, DF), F32); gfb2_d = dp('gfb2', (nb * G, D), F32)
    dfb1_d = dp('dfb1', (nb * M, DF), F32); dfb2_d = dp('dfb2', (nb * M, D), F32)
    cb1_d = dp('cb1', (nb, D), F32); cb2t_d = dp('cb2t', (128, NIC, nb), F32)
    gab1_d = dp('gab1', (nb, D), F32); gab2_d = dp('gab2', (nb, 2), F32)
    fb_d = dp('fb', (nb, D), F32); fg_d = dp('fg', (nb, D), F32)
    fbb_d = dp('fbb', (nb, D), F32)
    pool4_d = dp('pool4', (nb * G, nb)); pool16_d = dp('pool16', (nb * M, nb))

    zg_o = nc.declare_dram_parameter('zg', [nb, D], F32, isOutput=True)
    zd_o = nc.declare_dram_parameter('zd', [nb, D], F32, isOutput=True)
    zu_o = nc.declare_dram_parameter('zu', [nb, D], F32, isOutput=True)

    with tile.TileContext(nc) as tc, contextlib.ExitStack() as ctx:
        wpool = ctx.enter_context(tc.tile_pool(name="w", bufs=1))
        ffnw = ctx.enter_context(tc.tile_pool(name="ffnw", bufs=2))
        cpool = ctx.enter_context(tc.tile_pool(name="c", bufs=1))
        xpool = ctx.enter_context(tc.tile_pool(name="x", bufs=6))
        kpool = ctx.enter_context(tc.tile_pool(name="k", bufs=2))
        vpool = ctx.enter_context(tc.tile_pool(name="v", bufs=10))
        epool = ctx.enter_context(tc.tile_pool(name="e", bufs=8))
        spool = ctx.enter_context(tc.tile_pool(name="s", bufs=3))
        tpool = ctx.enter_context(tc.tile_pool(name="t", bufs=1))
        hpool = ctx.enter_context(tc.tile_pool(name="h", bufs=1))
        opool = ctx.enter_context(tc.tile_pool(name="o", bufs=1))
        ppP = ctx.enter_context(tc.tile_pool(name="ppP", bufs=1, space="PSUM"))
        ppS = ctx.enter_context(tc.tile_pool(name="ppS", bufs=2, space="PSUM"))
        ppT = ctx.enter_context(tc.tile_pool(name="ppT", bufs=2, space="PSUM"))
        ppO = ctx.enter_context(tc.tile_pool(name="ppO", bufs=2, space="PSUM"))
        ppD = ctx.enter_context(tc.tile_pool(name="ppD", bufs=1, space="PSUM"))

        def wtile(dram, shape, dt=F32R, pool=None, tag=None):
            t = (pool or wpool).tile(list(shape), dt, tag=tag or dram.name)
            nc.scalar.dma_start(out=t, in_=dram[tuple(slice(None) for _ in shape)])
            return t

        wv = wtile(wv_d, (128, NIC, D))
        wdv = wtile(wdv_d, (128, NIC, D))
        wdkr = wtile(wdkr_d, (128, NIC, D), BF16)
        qkg = wtile(qkg_d, (128, NIC, H * G), BF16)
        qkdb = wtile(qkdb_d, (128, NIC, H * M), BF16)
        dmh = wtile(dmh_d, (128, NIC, H), BF16)
        ones2 = wtile(ones2_d, (128, 2), BF16)
        identb = wtile(identb_d, (128, 128), BF16)
        gaw2 = wtile(gaw2_d, (128, NIC, 2))
        dmg = wtile(dmg_d, (128, NIC, H * G))
        dmd = wtile(dmd_d, (128, NIC, H * M))
        ident = wtile(ident_d, (128, 128))
        identf = ident.bitcast(F32)
        pool4 = wtile(pool4_d, (nb * G, nb))
        pool16 = wtile(pool16_d, (nb * M, nb))

        residg = wtile(residg_d, (nb * G, D), F32, cpool)
        dob = wtile(dob_d, (nb * M, D), F32, cpool)
        gn1g = wtile(gn1g_d, (nb * G, D), F32, cpool)
        gn1b = wtile(gn1b_d, (nb * G, D), F32, cpool)
        gn2g = wtile(gn2g_d, (nb * G, D), F32, cpool)
        gn2b = wtile(gn2b_d, (nb * G, D), F32, cpool)
        dng = wtile(dng_d, (nb * M, D), F32, cpool)
        dnb = wtile(dnb_d, (nb * M, D), F32, cpool)
        gfb2 = wtile(gfb2_d, (nb * G, D), F32, cpool)
        dfb2 = wtile(dfb2_d, (nb * M, D), F32, cpool)
        cb1 = wtile(cb1_d, (nb, D), F32, cpool)
        cb2t = wtile(cb2t_d, (128, NIC, nb), F32, cpool)
        gab1 = wtile(gab1_d, (nb, D), F32, cpool)
        gab2 = wtile(gab2_d, (nb, 2), F32, cpool)
        fb = wtile(fb_d, (nb, D), F32, cpool)
        fg = wtile(fg_d, (nb, D), F32, cpool)
        fbb = wtile(fbb_d, (nb, D), F32, cpool)

        maskv = []
        for b in range(nb):
            mv = cpool.tile([128, N // 128], F32, tag=f"maskv{b}")
            nc.sync.dma_start(out=mv, in_=maskv_d[b])
            maskv.append(mv)

        eps_g = cpool.tile([nb * G, 1], F32)
        nc.vector.memset(eps_g, 1e-5)
        eps_d = cpool.tile([nb * M, 1], F32)
        nc.vector.memset(eps_d, 1e-5)
        eps_b = cpool.tile([nb, 1], F32)
        nc.vector.memset(eps_b, 1e-5)

        # ---------------------------------------------------------- helpers

        def attention_pass(b, qkT, ncols, c_ps, den_ps):
            """scores = qkT.T @ X^T (K-proj folded into queries host-side);
            C += (E.m)^T.T @ X (V-proj deferred: out = C @ Wv^T later)."""
            for tb in range(NTB):
                xblk = xpool.tile([128, NIC, 512], BF16, tag="xblk")
                nc.sync.dma_start(
                    out=xblk,
                    in_=xt_d[b, :, :, tb * 512:(tb + 1) * 512].rearrange(
                        "ic p t -> p ic t"))
                pss = ppS.tile([ncols, 512], F32, tag="scores")
                for j in range(NIC):
                    nc.tensor.matmul(pss, qkT[:, j, :], xblk[:, j, :],
                                     start=(j == 0), stop=(j == NIC - 1))
                eb = epool.tile([ncols, 512], BF16, tag="eblk")
                nc.scalar.activation(out=eb, in_=pss, func=AF.Exp)
                for c in range(4):
                    pst = ppT.tile([128, ncols], BF16, tag="tr")
                    nc.tensor.transpose(
                        pst, eb[:, c * 128:(c + 1) * 128], identb[:ncols, :ncols])
                    tcg = tb * 4 + c
                    et = epool.tile([128, ncols], BF16, tag="et")
                    nc.vector.tensor_scalar_mul(et, pst, maskv[b][:, tcg:tcg + 1])
                    xtm = vpool.tile([128, 512], BF16, tag="xtm")
                    nc.sync.dma_start(
                        out=xtm, in_=xtm_d[b, tcg * 128:(tcg + 1) * 128, :])
                    first = (tb == 0 and c == 0)
                    last = (tb == NTB - 1 and c == 3)
                    nc.tensor.matmul(c_ps, et, xtm, start=first, stop=last)
                    nc.tensor.matmul(den_ps, et, ones2,
                                     start=first, stop=last)

        def finish_attention(ncols, cph, c_ps, den_ps, wvt, dmask, oT_all, b):
            den = spool.tile([ncols, 1], F32, tag="den")
            nc.vector.tensor_copy(den, den_ps[:, 0:1])
            rec = spool.tile([ncols, 1], F32, tag="rec")
            nc.vector.reciprocal(rec, den)
            csb = spool.tile([ncols, 512], F32, tag="csb")
            nc.vector.tensor_scalar_mul(csb, c_ps, rec)
            cT = spool.tile([128, NIC, ncols], F32R, tag="cT")
            for ic in range(NIC):
                pst = ppT.tile([128, ncols], F32, tag="tr")
                nc.tensor.transpose(
                    pst, csb[:, ic * 128:(ic + 1) * 128], identf[:ncols, :ncols])
                nc.vector.tensor_copy(cT[:, ic, :], pst)
            nps = ppP.tile([ncols, 512], F32, tag="proj")
            for ic in range(NIC):
                nc.tensor.matmul(nps, cT[:, ic, :], wvt[:, ic, :],
                                 start=(ic == 0), stop=(ic == NIC - 1))
            osb = spool.tile([ncols, 512], F32, tag="osb")
            nc.vector.tensor_copy(osb, nps)
            for ic in range(NIC):
                pst = ppT.tile([128, ncols], F32, tag="tr")
                nc.tensor.transpose(
                    pst, osb[:, ic * 128:(ic + 1) * 128], identf[:ncols, :ncols])
                ocl = spool.tile([128, ncols], F32, tag="ocl")
                nc.vector.tensor_tensor(out=ocl, in0=pst,
                                        in1=dmask[:, ic, :].bitcast(F32), op=OP.mult)
                red = spool.tile([128, cph], F32, tag="red")
                nc.vector.tensor_reduce(
                    out=red, in_=ocl.rearrange("p (h c) -> p c h", c=cph),
                    axis=AX.X, op=OP.add)
                last = nc.vector.tensor_copy(
                    oT_all[:, ic, b * cph:(b + 1) * cph], red)
            return last

        def transpose_to_fm(src, rows, tag, n_chunks=NIC, src_f32r=False):
            """src (rows, n_chunks*128) sbuf -> (128, n_chunks, rows) f32r."""
            out = tpool.tile([128, n_chunks, rows], F32R, tag=tag)
            for ic in range(n_chunks):
                if src_f32r:
                    pst = ppT.tile([128, rows], F32R, tag="tr")
                    nc.tensor.transpose(pst, src[:, ic * 128:(ic + 1) * 128],
                                        ident[:rows, :rows])
                else:
                    pst = ppT.tile([128, rows], F32, tag="tr")
                    nc.tensor.transpose(pst, src[:, ic * 128:(ic + 1) * 128],
                                        identf[:rows, :rows])
                nc.vector.tensor_copy(out[:, ic, :], pst)
            return out

        def linear_rows_w(lhsT, wget, n_ic, rows, cols):
            ps = ppP.tile([rows, cols], F32, tag="proj")
            for ic in range(n_ic):
                nc.tensor.matmul(ps, lhsT(ic), wget(ic),
                                 start=(ic == 0), stop=(ic == n_ic - 1))
            return ps

        def linear_rows(lhsT, w_tile, n_ic, rows, cols):
            ps = ppP.tile([rows, cols], F32, tag="proj")
            for ic in range(n_ic):
                nc.tensor.matmul(ps, lhsT(ic), w_tile[:, ic, :cols],
                                 start=(ic == 0), stop=(ic == n_ic - 1))
            return ps

        def layernorm(x, rows, eps_t, gamma, beta, resid=None, bias=None,
                      tag="ln"):
            """LN over free dim D. x may be psum. Returns f32 sbuf (rows, D)."""
            pre = tpool.tile([rows, D], F32, tag="lnpre")
            if bias is not None:
                nc.vector.tensor_tensor(out=pre, in0=x, in1=bias, op=OP.add)
            else:
                nc.vector.tensor_copy(pre, x)
            if resid is not None:
                nc.vector.tensor_tensor(out=pre, in0=pre, in1=resid, op=OP.add)
            stats = tpool.tile([rows, 6], F32, tag="lnst")
            nc.vector.bn_stats(out=stats, in_=pre)
            mv = tpool.tile([rows, 2], F32, tag="lnmv")
            nc.vector.bn_aggr(out=mv, in_=stats)
            rstd = tpool.tile([rows, 1], F32, tag="lnrs")
            nc.scalar.activation(out=rstd, in_=mv[:, 1:2], func=AF.Sqrt, bias=eps_t)
            nc.vector.reciprocal(rstd, rstd)
            nc.vector.tensor_scalar(out=pre, in0=pre, scalar1=mv[:, 0:1],
                                    scalar2=rstd, op0=OP.subtract, op1=OP.mult)
            out = tpool.tile([rows, D], F32, tag=tag + "out")
            nc.vector.tensor_tensor(out=pre, in0=pre, in1=gamma, op=OP.mult)
            last = nc.vector.tensor_tensor(out=out, in0=pre, in1=beta, op=OP.add)
            return out, pre, last

        from concourse.tile import add_dep_helper

        def pin(anchor, inst):
            if anchor is not None:
                add_dep_helper(inst.ins, anchor.ins, reason="phase pin")

        def ffn_block(z_sb, rows, w1_d, w2_d, b1_d, tagp, anchor):
            """psum(rows, D) = W2 @ gelu(W1 @ z + b1), bias2 NOT added.
            Weights and b1 streamed from DRAM chunk by chunk; every stream
            DMA is pinned after `anchor` so the scheduler cannot hoist it
            into an earlier phase (slot-wait head-of-line deadlock)."""
            zT = transpose_to_fm(z_sb, rows, tagp + "zT")
            h1 = hpool.tile([rows, DF], F32R, tag=tagp + "h1")
            for og in range(DF // 512):
                b1c = ffnw.tile([rows, 512], F32, tag="bs1")
                pin(anchor, nc.scalar.dma_start(
                    out=b1c, in_=b1_d[:, og * 512:(og + 1) * 512]))
                ps = ppP.tile([rows, 512], F32, tag="proj")
                for ic in range(NIC):
                    w1c = ffnw.tile([128, 512], F32R, tag="ws1")
                    pin(anchor, nc.scalar.dma_start(
                        out=w1c, in_=w1_d[:, ic, og * 512:(og + 1) * 512]))
                    nc.tensor.matmul(ps, zT[:, ic, :], w1c,
                                     start=(ic == 0), stop=(ic == NIC - 1))
                hb = tpool.tile([rows, 512], F32, tag="ffnhb")
                nc.vector.tensor_tensor(out=hb, in0=ps, in1=b1c, op=OP.add)
                nc.scalar.activation(out=h1[:, og * 512:(og + 1) * 512], in_=hb,
                                     func=AF.Gelu)
            h1v = h1.rearrange("r (cc p) -> r cc p", p=128)
            ps2 = ppP.tile([rows, D], F32, tag="proj")
            for cc in range(DF // 128):
                w2c = ffnw.tile([128, 512], F32R, tag="ws2")
                pin(anchor, nc.scalar.dma_start(out=w2c, in_=w2_d[:, cc, :]))
                pst = ppT.tile([128, rows], F32R, tag="tr")
                nc.tensor.transpose(pst, h1v[:, cc, :], ident[:rows, :rows])
                h1T = tpool.tile([128, rows], F32R, tag="ffnh1T")
                nc.vector.tensor_copy(h1T, pst)
                nc.tensor.matmul(ps2, h1T, w2c,
                                 start=(cc == 0), stop=(cc == DF // 128 - 1))
            return ps2

        def _stream_chunks(dram, cols, anchor, tag="ws1"):
            def get(ic):
                t = ffnw.tile([128, cols], F32R, tag=tag)
                pin(anchor, nc.scalar.dma_start(out=t, in_=dram[:, ic, :cols]))
                return t
            return get

        # ---------------- pass 1: global attention ----------------
        o_gT_all = opool.tile([128, NIC, nb * G], F32R, tag="ogT")
        for b in range(nb):
            c_ps = ppO.tile([H * G, 512], F32, tag="av")
            den_ps = ppD.tile([H * G, 2], F32, tag="den")
            attention_pass(b, qkg, H * G, c_ps, den_ps)
            p1_anchor = finish_attention(H * G, G, c_ps, den_ps, wv, dmg,
                                         o_gT_all, b)

        # ---------------- batched global tail ----------------
        zps = linear_rows_w(lambda ic: o_gT_all[:, ic, :],
                            _stream_chunks(wo_d, D, p1_anchor),
                            NIC, nb * G, D)
        z1, _, z1_inst = layernorm(zps, nb * G, eps_g, gn1g, gn1b,
                                   resid=residg, tag="g1")
        gffn_ps = ffn_block(z1, nb * G, gw1_d, gw2_d, gfb1_d, "gf", z1_inst)
        zg_b = tpool.tile([nb * G, D], F32, tag="zgb")
        nc.vector.tensor_tensor(out=zg_b, in0=gffn_ps, in1=gfb2, op=OP.add)
        z_global, _, _zg_inst = layernorm(zg_b, nb * G, eps_g, gn2g, gn2b, resid=z1,
                                tag="g2")
        z_global_r = tpool.tile([nb * G, D], F32R, tag="zgr")
        nc.vector.tensor_copy(z_global_r, z_global)

        psp = ppS.tile([nb, D], F32, tag="scores")
        nc.tensor.matmul(psp, pool4, z_global_r, start=True, stop=True)
        zgp = tpool.tile([nb, D], F32R, tag="zgp")
        nc.vector.tensor_copy(zgp, psp)
        nc.sync.dma_start(out=zg_o[:, :], in_=zgp.bitcast(F32))
        condT = transpose_to_fm(zgp.bitcast(F32), nb, "condT")

        # modn chain
        m1ps = linear_rows_w(lambda ic: condT[:, ic, :],
                             _stream_chunks(cw1_d, D, _zg_inst),
                             NIC, nb, D)
        m1b = tpool.tile([nb, D], F32, tag="m1b")
        nc.vector.tensor_tensor(out=m1b, in0=m1ps, in1=cb1, op=OP.add)
        m1 = tpool.tile([nb, D], F32R, tag="m1")
        m1_inst = nc.scalar.activation(out=m1, in_=m1b, func=AF.Gelu)
        m1T = transpose_to_fm(m1.bitcast(F32), nb, "m1T")
        modnT = tpool.tile([128, NIC, nb], F32, tag="modnT")
        for oc in range(NIC):
            psm = ppS.tile([128, nb], F32, tag="scores")
            for ic in range(NIC):
                cw2c = ffnw.tile([128, 128], F32R, tag="ws2")
                pin(m1_inst, nc.scalar.dma_start(
                    out=cw2c, in_=cw2_d[:, ic, oc * 128:(oc + 1) * 128]))
                nc.tensor.matmul(psm, cw2c,
                                 m1T[:, ic, :], start=(ic == 0),
                                 stop=(ic == NIC - 1))
            nc.vector.tensor_tensor(out=modnT[:, oc, :], in0=psm,
                                    in1=cb2t[:, oc, :], op=OP.add)

        # ---------------- pass 2: detail attention ----------------
        o_dT_all = opool.tile([128, NIC, nb * M], F32R, tag="odT")
        for b in range(nb):
            # fold modn into the folded detail queries:
            # qkd = qkd_base + (modn block-diag) @ Wdk  (broadcast over m)
            mblk = spool.tile([128, NIC, H], BF16, tag="mblk")
            for j in range(NIC):
                nc.vector.tensor_scalar_mul(mblk[:, j, :], dmh[:, j, :],
                                            modnT[:, j, b:b + 1])
            mkT = spool.tile([128, NIC, H], BF16, tag="mkT")
            for ic in range(NIC):
                psm2 = ppT.tile([128, H], F32, tag="tr")
                for j in range(NIC):
                    nc.tensor.matmul(psm2,
                                     wdkr[:, j, ic * 128:(ic + 1) * 128],
                                     mblk[:, j, :], start=(j == 0),
                                     stop=(j == NIC - 1))
                nc.vector.tensor_copy(mkT[:, ic, :], psm2)
            qde = spool.tile([128, NIC, H * M], BF16, tag="qde")
            for ic in range(NIC):
                nc.vector.tensor_tensor(
                    out=qde[:, ic, :].rearrange("p (h m) -> p h m", m=M),
                    in0=qkdb[:, ic, :].rearrange("p (h m) -> p h m", m=M),
                    in1=mkT[:, ic, :].unsqueeze(-1).broadcast_to([128, H, M]),
                    op=OP.add)
            c_ps = ppO.tile([H * M, 512], F32, tag="av")
            den_ps = ppD.tile([H * M, 2], F32, tag="den")
            attention_pass(b, qde, H * M, c_ps, den_ps)
            p2_anchor = finish_attention(H * M, M, c_ps, den_ps, wdv, dmd,
                                         o_dT_all, b)

        # ---------------- batched detail tail ----------------
        zdps = linear_rows_w(lambda ic: o_dT_all[:, ic, :],
                             _stream_chunks(wdo_d, D, p2_anchor),
                             NIC, nb * M, D)
        z1d = tpool.tile([nb * M, D], F32, tag="z1d")
        z1d_inst = nc.vector.tensor_tensor(out=z1d, in0=zdps, in1=dob, op=OP.add)
        dffn_ps = ffn_block(z1d, nb * M, dw1_d, dw2_d, dfb1_d, "df", z1d_inst)
        zd_b = tpool.tile([nb * M, D], F32, tag="zdb")
        nc.vector.tensor_tensor(out=zd_b, in0=dffn_ps, in1=dfb2, op=OP.add)
        z_detail, _, zdet_inst = layernorm(zd_b, nb * M, eps_d, dng, dnb, resid=z1d,
                                tag="dn")
        z_detail_r = tpool.tile([nb * M, D], F32R, tag="zdr")
        nc.vector.tensor_copy(z_detail_r, z_detail)

        psdp = ppS.tile([nb, D], F32, tag="scores")
        nc.tensor.matmul(psdp, pool16, z_detail_r, start=True, stop=True)
        zdp = tpool.tile([nb, D], F32R, tag="zdp")
        nc.vector.tensor_copy(zdp, psdp)
        nc.sync.dma_start(out=zd_o[:, :], in_=zdp.bitcast(F32))
        zdpT = transpose_to_fm(zdp.bitcast(F32), nb, "zdpT")

        # ---------------- gated fusion ----------------
        def giT(ic):
            return condT[:, ic, :] if ic < NIC else zdpT[:, ic - NIC, :]

        gaw1g = _stream_chunks(gaw1_d, D, zdet_inst)
        g1ps = linear_rows_w(giT, gaw1g, 2 * NIC, nb, D)
        g1b = tpool.tile([nb, D], F32, tag="g1b")
        nc.vector.tensor_tensor(out=g1b, in0=g1ps, in1=gab1, op=OP.add)
        g1 = tpool.tile([nb, D], F32R, tag="g1")
        nc.scalar.activation(out=g1, in_=g1b, func=AF.Gelu)
        g1T = transpose_to_fm(g1.bitcast(F32), nb, "g1T")
        lgps = linear_rows(lambda ic: g1T[:, ic, :], gaw2, NIC, nb, 2)
        lg = tpool.tile([nb, 2], F32, tag="lg")
        nc.vector.tensor_tensor(out=lg, in0=lgps, in1=gab2, op=OP.add)
        eg = tpool.tile([nb, 2], F32, tag="eg")
        nc.scalar.activation(out=eg, in_=lg, func=AF.Exp)
        egs = tpool.tile([nb, 1], F32, tag="egs")
        nc.vector.tensor_reduce(out=egs, in_=eg, axis=AX.X, op=OP.add)
        nc.vector.reciprocal(egs, egs)
        gate = tpool.tile([nb, 2], F32, tag="gate")
        nc.vector.tensor_scalar_mul(gate, eg, egs)

        zw = tpool.tile([nb, D], F32, tag="zw")
        nc.vector.tensor_scalar_mul(zw, zgp.bitcast(F32), gate[:, 0:1])
        zw2 = tpool.tile([nb, D], F32, tag="zw2")
        nc.vector.tensor_scalar_mul(zw2, zdp.bitcast(F32), gate[:, 1:2])
        nc.vector.tensor_tensor(out=zw, in0=zw, in1=zw2, op=OP.add)

        fwg = _stream_chunks(fw_d, D, zdet_inst)
        fps = linear_rows_w(giT, fwg, 2 * NIC, nb, D)
        fzb = tpool.tile([nb, D], F32, tag="fzb")
        nc.vector.tensor_tensor(out=fzb, in0=fps, in1=fb, op=OP.add)
        fzg = tpool.tile([nb, D], F32, tag="fzg")
        nc.scalar.activation(out=fzg, in_=fzb, func=AF.Gelu)
        fln, _, _ = layernorm(fzg, nb, eps_b, fg, fbb, tag="fln")
        zu = tpool.tile([nb, D], F32, tag="zu")
        nc.vector.tensor_tensor(out=zu, in0=fln, in1=zw, op=OP.add)
        nc.sync.dma_start(out=zu_o[:, :], in_=zu)

    nc.finalize()
    _BUILD_CACHE[nb] = nc
    return nc


# ------------------------------------------------------------------- driver

LAST_EXEC_NS = None


def kernel(X_tokens, mask, params, _trace=False):
    global LAST_EXEC_NS
    X = np.asarray(X_tokens, np.float32)
    mk = np.asarray(mask, bool)
    nb = X.shape[0] // NCORES
    shared = prep_host(params, nb)
    in_maps = [
        prep_core_inputs(X[c * nb:(c + 1) * nb], mk[c * nb:(c + 1) * nb], shared)
        for c in range(NCORES)
    ]
    nc = build(nb)
    res = run_bass_kernel_spmd(nc, in_maps, list(range(NCORES)), trace=_trace)
    LAST_EXEC_NS = res.exec_time_ns
    out = np.empty((3, X.shape[0], D), np.float32)
    for c in range(NCORES):
        r = res.results[c]
        out[0, c * nb:(c + 1) * nb] = r['zg']
        out[1, c * nb:(c + 1) * nb] = r['zd']
        out[2, c * nb:(c + 1) * nb] = r['zu']
    return out


# revision 23
# speedup vs baseline: 1.6718x; 1.0030x over previous
"""CLIP4CAD_HUS_v2 fused forward on 8 Trainium2 NeuronCores.

Data-parallel over batch B=64 (8 batches per core), params replicated.

Key algebraic restructuring (only G=4 / M=16 queries exist per head, so
the full K/V projections over N=2048 tokens are never materialized):
  scores = (q @ Wk) @ X^T        -- Wk folded into the query rows on host
  attn   = ((E*mask) @ X) @ Wv^T -- Wv applied after the n-contraction
E = exp(scores) without max subtraction (scores are O(1) by construction);
masked softmax denominator via a ones-column matvec on E^T. Per-head
output blocks are extracted with a static diag mask + strided h-sum.

Per core:
  pass 1 (per batch):  global cross-attention as above
  batched global tail: out-proj + LN + FFN + LN + modn chain (rows b*G+g)
  pass 2 (per batch):  detail attention, modn folded into the detail
                       queries via a block-diag matvec against Wdk rows
  batched detail tail: out-proj + FFN + LN, gated fusion, outputs

Layouts:
  feature-major tile (128, C, T): [p, c, t] = tensor[c*128+p, t]
  matmul: out[M,N] = lhsT[K,M].T @ rhs[K,N]   (K = partition dim)
Matmul operands are float32r (tf32-class PE precision at bf16 speed).
"""

import contextlib

import numpy as np

import concourse.bass as bass
import concourse.mybir as mybir
import concourse.tile as tile
from concourse import bacc
from concourse.bass_utils import run_bass_kernel_spmd

F32 = mybir.dt.float32
F32R = mybir.dt.float32r
BF16 = mybir.dt.bfloat16
AF = mybir.ActivationFunctionType
OP = mybir.AluOpType
AX = mybir.AxisListType

D, H, G, M = 512, 8, 4, 16
HD = D // H
DF = 4 * D
B, N = 64, 2048
NCORES = 8
NB = B // NCORES          # batches per core
NTB = N // 512            # 512-token blocks per batch
NIC = D // 128            # feature chunks of d=512

_BUILD_CACHE = {}


# ----------------------------------------------------------------- host prep

def _wT_chunks(w):
    """(out,in) torch-Linear weight -> (128, in/128, out) chunk layout."""
    wt = np.ascontiguousarray(np.asarray(w, np.float32).T)      # (in, out)
    ic = wt.shape[0] // 128
    return np.ascontiguousarray(
        wt.reshape(ic, 128, wt.shape[1]).transpose(1, 0, 2))


def _qblock_full(q, cph):
    """q (H*cph, D) -> (128, 4, H*cph) block-diag (chunks accumulated over j).

    out[p, j, h*cph+r] = q[h*cph+r, j*128+p] iff h == 2j + p//64, else 0.
    """
    out = np.zeros((128, NIC, H * cph), np.float32)
    q = np.asarray(q, np.float32)
    for j in range(NIC):
        for pl in range(2):
            h = 2 * j + pl
            rows = slice(pl * 64, (pl + 1) * 64)
            out[rows, j, h * cph:(h + 1) * cph] = \
                q[h * cph:(h + 1) * cph, j * 128 + pl * 64:j * 128 + (pl + 1) * 64].T
    return out


def _diagmask_full(cph):
    """(128, 4, H*cph): 1 iff col's head == 2j + p//64."""
    out = np.zeros((128, NIC, H * cph), np.float32)
    for j in range(NIC):
        for pl in range(2):
            h = 2 * j + pl
            out[pl * 64:(pl + 1) * 64, j, h * cph:(h + 1) * cph] = 1.0
    return out


def _bcast_rows(v, rows):
    v = np.asarray(v, np.float32)
    return np.ascontiguousarray(np.broadcast_to(v, (rows, v.shape[-1])))


def prep_host(params, nb=NB):
    p = {k: np.asarray(v, np.float32) for k, v in params.items()}
    io = {}

    adapt = np.tanh(p['mod_embed'][1] @ p['adapt_w'].T + p['adapt_b'])      # (D,)
    gq_eff = p['gq'][0] + 0.1 * adapt                                       # (G, D)
    dq_eff = p['dq'][0] + 0.1 * adapt                                       # (M, D)

    wq, wk, wv = np.split(p['mha_in_w'], 3, 0)
    bq, bk, bv = np.split(p['mha_in_b'], 3, 0)
    qg = (gq_eff @ wq.T + bq) / np.sqrt(HD)                                 # (G, D) -> rows h*G? no: (G,D)
    qd_base = (dq_eff @ p['det_wq'].T + p['det_bq']) / np.sqrt(HD)          # (M, D)

    # reorder query rows to (h, g): q_hg[h*cph+r, :] = q[r, h-th 64-slice...]
    # NOT a reorder of rows: _qblock_full wants q indexed [h*cph+r, d] where
    # the (h, r) query vector is q[r, :] restricted to head h's d-slice.
    # Build expanded (H*cph, D) with rows (h, r) = original row r.

    # fold K-projection weights into the (few) query rows: scores = qk @ X^T
    qkg = np.zeros((H * G, D), np.float32)
    qkd = np.zeros((H * M, D), np.float32)
    for h in range(H):
        sl = slice(h * HD, (h + 1) * HD)
        qkg[h * G:(h + 1) * G] = qg[:, sl] @ wk[sl, :]
        qkd[h * M:(h + 1) * M] = qd_base[:, sl] @ p['det_wk'][sl, :]
    import ml_dtypes
    io['qkg'] = _wT_chunks(qkg).astype(ml_dtypes.bfloat16)              # (128,4,32)
    io['qkdb'] = _wT_chunks(qkd).astype(ml_dtypes.bfloat16)             # (128,4,128)
    io['wdkr'] = _wT_chunks(p['det_wk'].T).astype(ml_dtypes.bfloat16)
    io['dmh'] = _diagmask_full(1).astype(ml_dtypes.bfloat16)            # (128,4,8)
    io['ones2'] = np.ones((128, 2), ml_dtypes.bfloat16)
    io['identb'] = np.eye(128, dtype=ml_dtypes.bfloat16)
    io['wv'] = _wT_chunks(wv)
    io['wdv'] = _wT_chunks(p['det_wv'])
    io['wo'] = _wT_chunks(p['mha_out_w'])
    io['wdo'] = _wT_chunks(p['det_wo'])
    io['cw1'] = _wT_chunks(p['cond_w1'])
    io['cw2'] = _wT_chunks(p['cond_w2'] / np.sqrt(HD))
    io['gw1'] = _wT_chunks(p['gffn_w1'])
    io['gw2'] = _wT_chunks(p['gffn_w2'])
    io['dw1'] = _wT_chunks(p['dffn_w1'])
    io['dw2'] = _wT_chunks(p['dffn_w2'])
    io['gaw1'] = _wT_chunks(p['gate_w1'])
    io['gaw2'] = _wT_chunks(p['gate_w2'])
    io['fw'] = _wT_chunks(p['fus_w'])

    io['dmg'] = _diagmask_full(G)
    io['dmd'] = _diagmask_full(M)
    io['ident'] = np.eye(128, dtype=np.float32)

    bo_eff = p['mha_out_b'] + bv @ p['mha_out_w'].T                         # (D,)
    dbo_eff = p['det_bo'] + p['det_bv'] @ p['det_wo'].T
    io['residg'] = (np.tile(gq_eff, (nb, 1)) + bo_eff).astype(np.float32)
    io['dob'] = _bcast_rows(dbo_eff, nb * M)

    io['gn1g'] = _bcast_rows(p['gn1_g'], nb * G)
    io['gn1b'] = _bcast_rows(p['gn1_b'], nb * G)
    io['gn2g'] = _bcast_rows(p['gn2_g'], nb * G)
    io['gn2b'] = _bcast_rows(p['gn2_b'], nb * G)
    io['dng'] = _bcast_rows(p['dn_g'], nb * M)
    io['dnb'] = _bcast_rows(p['dn_b'], nb * M)
    io['gfb1'] = _bcast_rows(p['gffn_b1'], nb * G)
    io['gfb2'] = _bcast_rows(p['gffn_b2'], nb * G)
    io['dfb1'] = _bcast_rows(p['dffn_b1'], nb * M)
    io['dfb2'] = _bcast_rows(p['dffn_b2'], nb * M)
    io['cb1'] = _bcast_rows(p['cond_b1'], nb)
    cb2 = (p['cond_b2'] / np.sqrt(HD)).reshape(NIC, 128).T                  # (128, 4)
    io['cb2t'] = np.ascontiguousarray(
        np.repeat(cb2[:, :, None], nb, axis=2)).astype(np.float32)          # (128,4,nb)
    io['gab1'] = _bcast_rows(p['gate_b1'], nb)
    io['gab2'] = _bcast_rows(p['gate_b2'], nb)
    io['fb'] = _bcast_rows(p['fus_b'], nb)
    io['fg'] = _bcast_rows(p['fus_g'], nb)
    io['fbb'] = _bcast_rows(p['fus_bb'], nb)

    pool4 = np.zeros((nb * G, nb), np.float32)
    for b in range(nb):
        pool4[b * G:(b + 1) * G, b] = 1.0 / G
    io['pool4'] = pool4
    pool16 = np.zeros((nb * M, nb), np.float32)
    for b in range(nb):
        pool16[b * M:(b + 1) * M, b] = 1.0 / M
    io['pool16'] = pool16
    return io


def prep_core_inputs(X_core, mask_core, shared):
    """Per-core data tensors. X_core (nb, N, D) f32, mask_core (nb, N) bool."""
    nb = X_core.shape[0]
    xt = np.ascontiguousarray(
        X_core.transpose(0, 2, 1).reshape(nb, NIC, 128, N)).astype(np.float32)
    m = mask_core.astype(np.float32)                                        # (nb, N)
    maskv = np.ascontiguousarray(
        m.reshape(nb, N // 128, 128).transpose(0, 2, 1))                    # (nb,128,16)
    import ml_dtypes
    io = dict(shared)
    io['xt'] = xt.astype(ml_dtypes.bfloat16)
    io['xtm'] = np.ascontiguousarray(X_core.astype(ml_dtypes.bfloat16))
    io['maskv'] = maskv
    return io


# -------------------------------------------------------------- device build

def build(nb=NB):
    if nb in _BUILD_CACHE:
        return _BUILD_CACHE[nb]
    nc = bacc.Bacc()

    def dp(name, shape, dt=F32R):
        return nc.declare_dram_parameter(name, list(shape), dt, isOutput=False)

    xt_d = dp('xt', (nb, NIC, 128, N), BF16)
    xtm_d = dp('xtm', (nb, N, D), BF16)
    wv_d = dp('wv', (128, NIC, D)); wdv_d = dp('wdv', (128, NIC, D))
    qkg_d = dp('qkg', (128, NIC, H * G), BF16)
    qkdb_d = dp('qkdb', (128, NIC, H * M), BF16)
    wdkr_d = dp('wdkr', (128, NIC, D), BF16); dmh_d = dp('dmh', (128, NIC, H), BF16)
    ones2_d = dp('ones2', (128, 2), BF16)
    identb_d = dp('identb', (128, 128), BF16)
    wo_d = dp('wo', (128, NIC, D)); wdo_d = dp('wdo', (128, NIC, D))
    cw1_d = dp('cw1', (128, NIC, D)); cw2_d = dp('cw2', (128, NIC, D))
    gw1_d = dp('gw1', (128, NIC, DF)); gw2_d = dp('gw2', (128, DF // 128, D))
    dw1_d = dp('dw1', (128, NIC, DF)); dw2_d = dp('dw2', (128, DF // 128, D))
    gaw1_d = dp('gaw1', (128, 2 * NIC, D)); gaw2_d = dp('gaw2', (128, NIC, 2))
    fw_d = dp('fw', (128, 2 * NIC, D))
    dmg_d = dp('dmg', (128, NIC, H * G)); dmd_d = dp('dmd', (128, NIC, H * M))
    ident_d = dp('ident', (128, 128))
    maskv_d = dp('maskv', (nb, 128, N // 128), F32)
    residg_d = dp('residg', (nb * G, D), F32)
    dob_d = dp('dob', (nb * M, D), F32)
    gn1g_d = dp('gn1g', (nb * G, D), F32); gn1b_d = dp('gn1b', (nb * G, D), F32)
    gn2g_d = dp('gn2g', (nb * G, D), F32); gn2b_d = dp('gn2b', (nb * G, D), F32)
    dng_d = dp('dng', (nb * M, D), F32); dnb_d = dp('dnb', (nb * M, D), F32)
    gfb1_d = dp('gfb1', (nb * G, DF), F32); gfb2_d = dp('gfb2', (nb * G, D), F32)
    dfb1_d = dp('dfb1', (nb * M, DF), F32); dfb2_d = dp('dfb2', (nb * M, D), F32)
    cb1_d = dp('cb1', (nb, D), F32); cb2t_d = dp('cb2t', (128, NIC, nb), F32)
    gab1_d = dp('gab1', (nb, D), F32); gab2_d = dp('gab2', (nb, 2), F32)
    fb_d = dp('fb', (nb, D), F32); fg_d = dp('fg', (nb, D), F32)
    fbb_d = dp('fbb', (nb, D), F32)
    pool4_d = dp('pool4', (nb * G, nb)); pool16_d = dp('pool16', (nb * M, nb))

    zg_o = nc.declare_dram_parameter('zg', [nb, D], F32, isOutput=True)
    zd_o = nc.declare_dram_parameter('zd', [nb, D], F32, isOutput=True)
    zu_o = nc.declare_dram_parameter('zu', [nb, D], F32, isOutput=True)

    with tile.TileContext(nc) as tc, contextlib.ExitStack() as ctx:
        wpool = ctx.enter_context(tc.tile_pool(name="w", bufs=1))
        ffnw = ctx.enter_context(tc.tile_pool(name="ffnw", bufs=2))
        cpool = ctx.enter_context(tc.tile_pool(name="c", bufs=1))
        xpool = ctx.enter_context(tc.tile_pool(name="x", bufs=7))
        kpool = ctx.enter_context(tc.tile_pool(name="k", bufs=2))
        vpool = ctx.enter_context(tc.tile_pool(name="v", bufs=10))
        epool = ctx.enter_context(tc.tile_pool(name="e", bufs=8))
        spool = ctx.enter_context(tc.tile_pool(name="s", bufs=3))
        tpool = ctx.enter_context(tc.tile_pool(name="t", bufs=1))
        hpool = ctx.enter_context(tc.tile_pool(name="h", bufs=1))
        opool = ctx.enter_context(tc.tile_pool(name="o", bufs=1))
        ppP = ctx.enter_context(tc.tile_pool(name="ppP", bufs=1, space="PSUM"))
        ppS = ctx.enter_context(tc.tile_pool(name="ppS", bufs=2, space="PSUM"))
        ppT = ctx.enter_context(tc.tile_pool(name="ppT", bufs=2, space="PSUM"))
        ppO = ctx.enter_context(tc.tile_pool(name="ppO", bufs=2, space="PSUM"))
        ppD = ctx.enter_context(tc.tile_pool(name="ppD", bufs=1, space="PSUM"))

        def wtile(dram, shape, dt=F32R, pool=None, tag=None):
            t = (pool or wpool).tile(list(shape), dt, tag=tag or dram.name)
            nc.scalar.dma_start(out=t, in_=dram[tuple(slice(None) for _ in shape)])
            return t

        wv = wtile(wv_d, (128, NIC, D))
        wdv = wtile(wdv_d, (128, NIC, D))
        wdkr = wtile(wdkr_d, (128, NIC, D), BF16)
        qkg = wtile(qkg_d, (128, NIC, H * G), BF16)
        qkdb = wtile(qkdb_d, (128, NIC, H * M), BF16)
        dmh = wtile(dmh_d, (128, NIC, H), BF16)
        ones2 = wtile(ones2_d, (128, 2), BF16)
        identb = wtile(identb_d, (128, 128), BF16)
        gaw2 = wtile(gaw2_d, (128, NIC, 2))
        dmg = wtile(dmg_d, (128, NIC, H * G))
        dmd = wtile(dmd_d, (128, NIC, H * M))
        ident = wtile(ident_d, (128, 128))
        identf = ident.bitcast(F32)
        pool4 = wtile(pool4_d, (nb * G, nb))
        pool16 = wtile(pool16_d, (nb * M, nb))

        residg = wtile(residg_d, (nb * G, D), F32, cpool)
        dob = wtile(dob_d, (nb * M, D), F32, cpool)
        gn1g = wtile(gn1g_d, (nb * G, D), F32, cpool)
        gn1b = wtile(gn1b_d, (nb * G, D), F32, cpool)
        gn2g = wtile(gn2g_d, (nb * G, D), F32, cpool)
        gn2b = wtile(gn2b_d, (nb * G, D), F32, cpool)
        dng = wtile(dng_d, (nb * M, D), F32, cpool)
        dnb = wtile(dnb_d, (nb * M, D), F32, cpool)
        gfb2 = wtile(gfb2_d, (nb * G, D), F32, cpool)
        dfb2 = wtile(dfb2_d, (nb * M, D), F32, cpool)
        cb1 = wtile(cb1_d, (nb, D), F32, cpool)
        cb2t = wtile(cb2t_d, (128, NIC, nb), F32, cpool)
        gab1 = wtile(gab1_d, (nb, D), F32, cpool)
        gab2 = wtile(gab2_d, (nb, 2), F32, cpool)
        fb = wtile(fb_d, (nb, D), F32, cpool)
        fg = wtile(fg_d, (nb, D), F32, cpool)
        fbb = wtile(fbb_d, (nb, D), F32, cpool)

        maskv = []
        for b in range(nb):
            mv = cpool.tile([128, N // 128], F32, tag=f"maskv{b}")
            nc.sync.dma_start(out=mv, in_=maskv_d[b])
            maskv.append(mv)

        eps_g = cpool.tile([nb * G, 1], F32)
        nc.vector.memset(eps_g, 1e-5)
        eps_d = cpool.tile([nb * M, 1], F32)
        nc.vector.memset(eps_d, 1e-5)
        eps_b = cpool.tile([nb, 1], F32)
        nc.vector.memset(eps_b, 1e-5)

        # ---------------------------------------------------------- helpers

        def attention_pass(b, qkT, ncols, c_ps, den_ps):
            """scores = qkT.T @ X^T (K-proj folded into queries host-side);
            C += (E.m)^T.T @ X (V-proj deferred: out = C @ Wv^T later)."""
            for tb in range(NTB):
                xblk = xpool.tile([128, NIC, 512], BF16, tag="xblk")
                nc.sync.dma_start(
                    out=xblk,
                    in_=xt_d[b, :, :, tb * 512:(tb + 1) * 512].rearrange(
                        "ic p t -> p ic t"))
                pss = ppS.tile([ncols, 512], F32, tag="scores")
                for j in range(NIC):
                    nc.tensor.matmul(pss, qkT[:, j, :], xblk[:, j, :],
                                     start=(j == 0), stop=(j == NIC - 1))
                eb = epool.tile([ncols, 512], BF16, tag="eblk")
                nc.scalar.activation(out=eb, in_=pss, func=AF.Exp)
                for c in range(4):
                    pst = ppT.tile([128, ncols], BF16, tag="tr")
                    nc.tensor.transpose(
                        pst, eb[:, c * 128:(c + 1) * 128], identb[:ncols, :ncols])
                    tcg = tb * 4 + c
                    et = epool.tile([128, ncols], BF16, tag="et")
                    nc.vector.tensor_scalar_mul(et, pst, maskv[b][:, tcg:tcg + 1])
                    xtm = vpool.tile([128, 512], BF16, tag="xtm")
                    nc.sync.dma_start(
                        out=xtm, in_=xtm_d[b, tcg * 128:(tcg + 1) * 128, :])
                    first = (tb == 0 and c == 0)
                    last = (tb == NTB - 1 and c == 3)
                    nc.tensor.matmul(c_ps, et, xtm, start=first, stop=last)
                    nc.tensor.matmul(den_ps, et, ones2,
                                     start=first, stop=last)

        def finish_attention(ncols, cph, c_ps, den_ps, wvt, dmask, oT_all, b):
            den = spool.tile([ncols, 1], F32, tag="den")
            nc.vector.tensor_copy(den, den_ps[:, 0:1])
            rec = spool.tile([ncols, 1], F32, tag="rec")
            nc.vector.reciprocal(rec, den)
            csb = spool.tile([ncols, 512], F32, tag="csb")
            nc.vector.tensor_scalar_mul(csb, c_ps, rec)
            cT = spool.tile([128, NIC, ncols], F32R, tag="cT")
            for ic in range(NIC):
                pst = ppT.tile([128, ncols], F32, tag="tr")
                nc.tensor.transpose(
                    pst, csb[:, ic * 128:(ic + 1) * 128], identf[:ncols, :ncols])
                nc.vector.tensor_copy(cT[:, ic, :], pst)
            nps = ppP.tile([ncols, 512], F32, tag="proj")
            for ic in range(NIC):
                nc.tensor.matmul(nps, cT[:, ic, :], wvt[:, ic, :],
                                 start=(ic == 0), stop=(ic == NIC - 1))
            osb = spool.tile([ncols, 512], F32, tag="osb")
            nc.vector.tensor_copy(osb, nps)
            for ic in range(NIC):
                pst = ppT.tile([128, ncols], F32, tag="tr")
                nc.tensor.transpose(
                    pst, osb[:, ic * 128:(ic + 1) * 128], identf[:ncols, :ncols])
                ocl = spool.tile([128, ncols], F32, tag="ocl")
                nc.vector.tensor_tensor(out=ocl, in0=pst,
                                        in1=dmask[:, ic, :].bitcast(F32), op=OP.mult)
                red = spool.tile([128, cph], F32, tag="red")
                nc.vector.tensor_reduce(
                    out=red, in_=ocl.rearrange("p (h c) -> p c h", c=cph),
                    axis=AX.X, op=OP.add)
                last = nc.vector.tensor_copy(
                    oT_all[:, ic, b * cph:(b + 1) * cph], red)
            return last

        def transpose_to_fm(src, rows, tag, n_chunks=NIC, src_f32r=False):
            """src (rows, n_chunks*128) sbuf -> (128, n_chunks, rows) f32r."""
            out = tpool.tile([128, n_chunks, rows], F32R, tag=tag)
            for ic in range(n_chunks):
                if src_f32r:
                    pst = ppT.tile([128, rows], F32R, tag="tr")
                    nc.tensor.transpose(pst, src[:, ic * 128:(ic + 1) * 128],
                                        ident[:rows, :rows])
                else:
                    pst = ppT.tile([128, rows], F32, tag="tr")
                    nc.tensor.transpose(pst, src[:, ic * 128:(ic + 1) * 128],
                                        identf[:rows, :rows])
                nc.vector.tensor_copy(out[:, ic, :], pst)
            return out

        def linear_rows_w(lhsT, wget, n_ic, rows, cols):
            ps = ppP.tile([rows, cols], F32, tag="proj")
            for ic in range(n_ic):
                nc.tensor.matmul(ps, lhsT(ic), wget(ic),
                                 start=(ic == 0), stop=(ic == n_ic - 1))
            return ps

        def linear_rows(lhsT, w_tile, n_ic, rows, cols):
            ps = ppP.tile([rows, cols], F32, tag="proj")
            for ic in range(n_ic):
                nc.tensor.matmul(ps, lhsT(ic), w_tile[:, ic, :cols],
                                 start=(ic == 0), stop=(ic == n_ic - 1))
            return ps

        def layernorm(x, rows, eps_t, gamma, beta, resid=None, bias=None,
                      tag="ln"):
            """LN over free dim D. x may be psum. Returns f32 sbuf (rows, D)."""
            pre = tpool.tile([rows, D], F32, tag="lnpre")
            if bias is not None:
                nc.vector.tensor_tensor(out=pre, in0=x, in1=bias, op=OP.add)
            else:
                nc.vector.tensor_copy(pre, x)
            if resid is not None:
                nc.vector.tensor_tensor(out=pre, in0=pre, in1=resid, op=OP.add)
            stats = tpool.tile([rows, 6], F32, tag="lnst")
            nc.vector.bn_stats(out=stats, in_=pre)
            mv = tpool.tile([rows, 2], F32, tag="lnmv")
            nc.vector.bn_aggr(out=mv, in_=stats)
            rstd = tpool.tile([rows, 1], F32, tag="lnrs")
            nc.scalar.activation(out=rstd, in_=mv[:, 1:2], func=AF.Sqrt, bias=eps_t)
            nc.vector.reciprocal(rstd, rstd)
            nc.vector.tensor_scalar(out=pre, in0=pre, scalar1=mv[:, 0:1],
                                    scalar2=rstd, op0=OP.subtract, op1=OP.mult)
            out = tpool.tile([rows, D], F32, tag=tag + "out")
            nc.vector.tensor_tensor(out=pre, in0=pre, in1=gamma, op=OP.mult)
            last = nc.vector.tensor_tensor(out=out, in0=pre, in1=beta, op=OP.add)
            return out, pre, last

        from concourse.tile import add_dep_helper

        def pin(anchor, inst):
            if anchor is not None:
                add_dep_helper(inst.ins, anchor.ins, reason="phase pin")

        def ffn_block(z_sb, rows, w1_d, w2_d, b1_d, tagp, anchor):
            """psum(rows, D) = W2 @ gelu(W1 @ z + b1), bias2 NOT added.
            Weights and b1 streamed from DRAM chunk by chunk; every stream
            DMA is pinned after `anchor` so the scheduler cannot hoist it
            into an earlier phase (slot-wait head-of-line deadlock)."""
            zT = transpose_to_fm(z_sb, rows, tagp + "zT")
            h1 = hpool.tile([rows, DF], F32R, tag=tagp + "h1")
            for og in range(DF // 512):
                b1c = ffnw.tile([rows, 512], F32, tag="bs1")
                pin(anchor, nc.scalar.dma_start(
                    out=b1c, in_=b1_d[:, og * 512:(og + 1) * 512]))
                ps = ppP.tile([rows, 512], F32, tag="proj")
                for ic in range(NIC):
                    w1c = ffnw.tile([128, 512], F32R, tag="ws1")
                    pin(anchor, nc.scalar.dma_start(
                        out=w1c, in_=w1_d[:, ic, og * 512:(og + 1) * 512]))
                    nc.tensor.matmul(ps, zT[:, ic, :], w1c,
                                     start=(ic == 0), stop=(ic == NIC - 1))
                hb = tpool.tile([rows, 512], F32, tag="ffnhb")
                nc.vector.tensor_tensor(out=hb, in0=ps, in1=b1c, op=OP.add)
                nc.scalar.activation(out=h1[:, og * 512:(og + 1) * 512], in_=hb,
                                     func=AF.Gelu)
            h1v = h1.rearrange("r (cc p) -> r cc p", p=128)
            ps2 = ppP.tile([rows, D], F32, tag="proj")
            for cc in range(DF // 128):
                w2c = ffnw.tile([128, 512], F32R, tag="ws2")
                pin(anchor, nc.scalar.dma_start(out=w2c, in_=w2_d[:, cc, :]))
                pst = ppT.tile([128, rows], F32R, tag="tr")
                nc.tensor.transpose(pst, h1v[:, cc, :], ident[:rows, :rows])
                h1T = tpool.tile([128, rows], F32R, tag="ffnh1T")
                nc.vector.tensor_copy(h1T, pst)
                nc.tensor.matmul(ps2, h1T, w2c,
                                 start=(cc == 0), stop=(cc == DF // 128 - 1))
            return ps2

        def _stream_chunks(dram, cols, anchor, tag="ws1"):
            def get(ic):
                t = ffnw.tile([128, cols], F32R, tag=tag)
                pin(anchor, nc.scalar.dma_start(out=t, in_=dram[:, ic, :cols]))
                return t
            return get

        # ---------------- pass 1: global attention ----------------
        o_gT_all = opool.tile([128, NIC, nb * G], F32R, tag="ogT")
        for b in range(nb):
            c_ps = ppO.tile([H * G, 512], F32, tag="av")
            den_ps = ppD.tile([H * G, 2], F32, tag="den")
            attention_pass(b, qkg, H * G, c_ps, den_ps)
            p1_anchor = finish_attention(H * G, G, c_ps, den_ps, wv, dmg,
                                         o_gT_all, b)

        # ---------------- batched global tail ----------------
        zps = linear_rows_w(lambda ic: o_gT_all[:, ic, :],
                            _stream_chunks(wo_d, D, p1_anchor),
                            NIC, nb * G, D)
        z1, _, z1_inst = layernorm(zps, nb * G, eps_g, gn1g, gn1b,
                                   resid=residg, tag="g1")
        gffn_ps = ffn_block(z1, nb * G, gw1_d, gw2_d, gfb1_d, "gf", z1_inst)
        zg_b = tpool.tile([nb * G, D], F32, tag="zgb")
        nc.vector.tensor_tensor(out=zg_b, in0=gffn_ps, in1=gfb2, op=OP.add)
        z_global, _, _zg_inst = layernorm(zg_b, nb * G, eps_g, gn2g, gn2b, resid=z1,
                                tag="g2")
        z_global_r = tpool.tile([nb * G, D], F32R, tag="zgr")
        nc.vector.tensor_copy(z_global_r, z_global)

        psp = ppS.tile([nb, D], F32, tag="scores")
        nc.tensor.matmul(psp, pool4, z_global_r, start=True, stop=True)
        zgp = tpool.tile([nb, D], F32R, tag="zgp")
        nc.vector.tensor_copy(zgp, psp)
        nc.sync.dma_start(out=zg_o[:, :], in_=zgp.bitcast(F32))
        condT = transpose_to_fm(zgp.bitcast(F32), nb, "condT")

        # modn chain
        m1ps = linear_rows_w(lambda ic: condT[:, ic, :],
                             _stream_chunks(cw1_d, D, _zg_inst),
                             NIC, nb, D)
        m1b = tpool.tile([nb, D], F32, tag="m1b")
        nc.vector.tensor_tensor(out=m1b, in0=m1ps, in1=cb1, op=OP.add)
        m1 = tpool.tile([nb, D], F32R, tag="m1")
        m1_inst = nc.scalar.activation(out=m1, in_=m1b, func=AF.Gelu)
        m1T = transpose_to_fm(m1.bitcast(F32), nb, "m1T")
        modnT = tpool.tile([128, NIC, nb], F32, tag="modnT")
        for oc in range(NIC):
            psm = ppS.tile([128, nb], F32, tag="scores")
            for ic in range(NIC):
                cw2c = ffnw.tile([128, 128], F32R, tag="ws2")
                pin(m1_inst, nc.scalar.dma_start(
                    out=cw2c, in_=cw2_d[:, ic, oc * 128:(oc + 1) * 128]))
                nc.tensor.matmul(psm, cw2c,
                                 m1T[:, ic, :], start=(ic == 0),
                                 stop=(ic == NIC - 1))
            nc.vector.tensor_tensor(out=modnT[:, oc, :], in0=psm,
                                    in1=cb2t[:, oc, :], op=OP.add)

        # ---------------- pass 2: detail attention ----------------
        o_dT_all = opool.tile([128, NIC, nb * M], F32R, tag="odT")
        for b in range(nb):
            # fold modn into the folded detail queries:
            # qkd = qkd_base + (modn block-diag) @ Wdk  (broadcast over m)
            mblk = spool.tile([128, NIC, H], BF16, tag="mblk")
            for j in range(NIC):
                nc.vector.tensor_scalar_mul(mblk[:, j, :], dmh[:, j, :],
                                            modnT[:, j, b:b + 1])
            mkT = spool.tile([128, NIC, H], BF16, tag="mkT")
            for ic in range(NIC):
                psm2 = ppT.tile([128, H], F32, tag="tr")
                for j in range(NIC):
                    nc.tensor.matmul(psm2,
                                     wdkr[:, j, ic * 128:(ic + 1) * 128],
                                     mblk[:, j, :], start=(j == 0),
                                     stop=(j == NIC - 1))
                nc.vector.tensor_copy(mkT[:, ic, :], psm2)
            qde = spool.tile([128, NIC, H * M], BF16, tag="qde")
            for ic in range(NIC):
                nc.vector.tensor_tensor(
                    out=qde[:, ic, :].rearrange("p (h m) -> p h m", m=M),
                    in0=qkdb[:, ic, :].rearrange("p (h m) -> p h m", m=M),
                    in1=mkT[:, ic, :].unsqueeze(-1).broadcast_to([128, H, M]),
                    op=OP.add)
            c_ps = ppO.tile([H * M, 512], F32, tag="av")
            den_ps = ppD.tile([H * M, 2], F32, tag="den")
            attention_pass(b, qde, H * M, c_ps, den_ps)
            p2_anchor = finish_attention(H * M, M, c_ps, den_ps, wdv, dmd,
                                         o_dT_all, b)

        # ---------------- batched detail tail ----------------
        zdps = linear_rows_w(lambda ic: o_dT_all[:, ic, :],
                             _stream_chunks(wdo_d, D, p2_anchor),
                             NIC, nb * M, D)
        z1d = tpool.tile([nb * M, D], F32, tag="z1d")
        z1d_inst = nc.vector.tensor_tensor(out=z1d, in0=zdps, in1=dob, op=OP.add)
        dffn_ps = ffn_block(z1d, nb * M, dw1_d, dw2_d, dfb1_d, "df", z1d_inst)
        zd_b = tpool.tile([nb * M, D], F32, tag="zdb")
        nc.vector.tensor_tensor(out=zd_b, in0=dffn_ps, in1=dfb2, op=OP.add)
        z_detail, _, zdet_inst = layernorm(zd_b, nb * M, eps_d, dng, dnb, resid=z1d,
                                tag="dn")
        z_detail_r = tpool.tile([nb * M, D], F32R, tag="zdr")
        nc.vector.tensor_copy(z_detail_r, z_detail)

        psdp = ppS.tile([nb, D], F32, tag="scores")
        nc.tensor.matmul(psdp, pool16, z_detail_r, start=True, stop=True)
        zdp = tpool.tile([nb, D], F32R, tag="zdp")
        nc.vector.tensor_copy(zdp, psdp)
        nc.sync.dma_start(out=zd_o[:, :], in_=zdp.bitcast(F32))
        zdpT = transpose_to_fm(zdp.bitcast(F32), nb, "zdpT")

        # ---------------- gated fusion ----------------
        def giT(ic):
            return condT[:, ic, :] if ic < NIC else zdpT[:, ic - NIC, :]

        gaw1g = _stream_chunks(gaw1_d, D, zdet_inst)
        g1ps = linear_rows_w(giT, gaw1g, 2 * NIC, nb, D)
        g1b = tpool.tile([nb, D], F32, tag="g1b")
        nc.vector.tensor_tensor(out=g1b, in0=g1ps, in1=gab1, op=OP.add)
        g1 = tpool.tile([nb, D], F32R, tag="g1")
        nc.scalar.activation(out=g1, in_=g1b, func=AF.Gelu)
        g1T = transpose_to_fm(g1.bitcast(F32), nb, "g1T")
        lgps = linear_rows(lambda ic: g1T[:, ic, :], gaw2, NIC, nb, 2)
        lg = tpool.tile([nb, 2], F32, tag="lg")
        nc.vector.tensor_tensor(out=lg, in0=lgps, in1=gab2, op=OP.add)
        eg = tpool.tile([nb, 2], F32, tag="eg")
        nc.scalar.activation(out=eg, in_=lg, func=AF.Exp)
        egs = tpool.tile([nb, 1], F32, tag="egs")
        nc.vector.tensor_reduce(out=egs, in_=eg, axis=AX.X, op=OP.add)
        nc.vector.reciprocal(egs, egs)
        gate = tpool.tile([nb, 2], F32, tag="gate")
        nc.vector.tensor_scalar_mul(gate, eg, egs)

        zw = tpool.tile([nb, D], F32, tag="zw")
        nc.vector.tensor_scalar_mul(zw, zgp.bitcast(F32), gate[:, 0:1])
        zw2 = tpool.tile([nb, D], F32, tag="zw2")
        nc.vector.tensor_scalar_mul(zw2, zdp.bitcast(F32), gate[:, 1:2])
        nc.vector.tensor_tensor(out=zw, in0=zw, in1=zw2, op=OP.add)

        fwg = _stream_chunks(fw_d, D, zdet_inst)
        fps = linear_rows_w(giT, fwg, 2 * NIC, nb, D)
        fzb = tpool.tile([nb, D], F32, tag="fzb")
        nc.vector.tensor_tensor(out=fzb, in0=fps, in1=fb, op=OP.add)
        fzg = tpool.tile([nb, D], F32, tag="fzg")
        nc.scalar.activation(out=fzg, in_=fzb, func=AF.Gelu)
        fln, _, _ = layernorm(fzg, nb, eps_b, fg, fbb, tag="fln")
        zu = tpool.tile([nb, D], F32, tag="zu")
        nc.vector.tensor_tensor(out=zu, in0=fln, in1=zw, op=OP.add)
        nc.sync.dma_start(out=zu_o[:, :], in_=zu)

    nc.finalize()
    _BUILD_CACHE[nb] = nc
    return nc


# ------------------------------------------------------------------- driver

LAST_EXEC_NS = None


def kernel(X_tokens, mask, params, _trace=False):
    global LAST_EXEC_NS
    X = np.asarray(X_tokens, np.float32)
    mk = np.asarray(mask, bool)
    nb = X.shape[0] // NCORES
    shared = prep_host(params, nb)
    in_maps = [
        prep_core_inputs(X[c * nb:(c + 1) * nb], mk[c * nb:(c + 1) * nb], shared)
        for c in range(NCORES)
    ]
    nc = build(nb)
    res = run_bass_kernel_spmd(nc, in_maps, list(range(NCORES)), trace=_trace)
    LAST_EXEC_NS = res.exec_time_ns
    out = np.empty((3, X.shape[0], D), np.float32)
    for c in range(NCORES):
        r = res.results[c]
        out[0, c * nb:(c + 1) * nb] = r['zg']
        out[1, c * nb:(c + 1) * nb] = r['zd']
        out[2, c * nb:(c + 1) * nb] = r['zu']
    return out
